# revision 1
# baseline (speedup 1.0000x reference)
"""Trainium2 Bass kernel for nn_Block_87351044866235 (sparse_attention).

Data-parallel over batch: 8 samples -> 8 NeuronCores. Channel-major
layout [C, H*W] on chip; depthwise convs as diagonal fp32r matmuls on
TensorE; 1x1 convs as fp32r matmuls; LN stats via ones-matmuls; q/k gram
via hi/lo bf16 split + DMA-xbar transposes; dynamic-k gate mean via a
scalar AllReduce.
"""
import sys, os

for _p in ("/opt/trn_rl_repo", "/root/.axon_site/_ro/trn_rl_repo"):
    if os.path.isdir(_p) and _p not in sys.path:
        sys.path.append(_p)

import numpy as np
import concourse.bass as bass
import concourse.bacc as bacc
import concourse.tile as tile
from concourse import mybir
from concourse import bass_utils

try:
    from concourse import tile_utils as _tu
    _tu.max_sbuf_usage = 208 * 1024
except Exception:
    pass

dt = mybir.dt
Alu = mybir.AluOpType
Act = mybir.ActivationFunctionType
AX = mybir.AxisListType.X

EMBED, PDIM, HEADS, HID = 192, 96, 8, 256
CPH = PDIM // HEADS  # 12
SLOP = 8
RC = 3    # conv output rows per chunk
BR = 12   # rows per band

F32, F32R, BF16 = dt.float32, dt.float32r, dt.bfloat16


def _ceil(a, b):
    return (a + b - 1) // b


# ----------------------------------------------------------------------------
# host-side weight prep: everything 2D [partitions, free]
# ----------------------------------------------------------------------------

def _prep_weights(p):
    w = {}
    f32r = lambda a: (np.ascontiguousarray(a, np.float32), F32R)
    f32 = lambda a: (np.ascontiguousarray(a, np.float32), F32)
    eps_bn = 1e-5

    w["ident"] = f32(np.eye(128, dtype=np.float32))

    # pos depthwise diag: [96, (t*2+cg)*96]
    pw = p["pos_w"][:, 0]  # [192,3,3]
    pos_d = np.zeros((96, 18 * 96), np.float32)
    for t in range(9):
        dy, dx = t // 3 - 1, t % 3 - 1
        for cg in range(2):
            pos_d[:, (t * 2 + cg) * 96:(t * 2 + cg + 1) * 96] = \
                np.diag(pw[cg * 96:(cg + 1) * 96, dy + 1, dx + 1])
    w["pos_diag"] = f32r(pos_d)
    w["pos_b"] = f32(p["pos_b"].reshape(2, 96).T)  # [96, 2]

    g1v, b1v = p["ln1_g"], p["ln1_b"]
    qw = p["qkv_w"][:, :, 0, 0]  # [288, 96]
    qw_eff = qw * g1v[None, :96]
    w["qkv_wT"] = f32r(np.concatenate(
        [qw_eff[j * 96:(j + 1) * 96].T for j in range(3)], axis=1))  # [96, 3*96]
    w["qkv_bias"] = f32((qw @ b1v[:96]).reshape(3, 96).T)  # [96, 3]

    qdw = p["qkv_dw_w"][:, 0]  # [288,3,3]
    qdw_d = np.zeros((96, 27 * 96), np.float32)
    for t in range(9):
        dy, dx = t // 3 - 1, t % 3 - 1
        for j in range(3):
            qdw_d[:, (t * 3 + j) * 96:(t * 3 + j + 1) * 96] = \
                np.diag(qdw[j * 96:(j + 1) * 96, dy + 1, dx + 1])
    w["qdw_diag"] = f32r(qdw_d)

    gw1 = p["gate_w1"][:, :, 0, 0]  # [96, 192]
    gw1_eff = gw1 * g1v[None, :]
    w["gate_w1T"] = f32r(np.concatenate(
        [gw1_eff[:, cg * 96:(cg + 1) * 96].T for cg in range(2)], axis=1))  # [96, 192]
    w["gate_b1"] = f32((p["gate_b1"] + gw1 @ b1v).reshape(96, 1))
    w["gate_w2T"] = f32r(p["gate_w2"][:, :, 0, 0].T.copy())  # [96,1]
    w["gate_b2"] = f32(p["gate_b2"].reshape(1, 1))

    pj = p["proj_w"][:, :, 0, 0]
    pj1, pj2 = pj[:, :96], pj[:, 96:] * g1v[None, 96:]
    w["proj1T"] = f32r(np.concatenate(
        [pj1[cg * 96:(cg + 1) * 96].T for cg in range(2)], axis=1))  # [96, 192]
    w["proj2T"] = f32r(np.concatenate(
        [pj2[cg * 96:(cg + 1) * 96].T for cg in range(2)], axis=1))
    w["proj_bias"] = f32((pj[:, 96:] @ b1v[96:]).reshape(2, 96).T)  # [96, 2]

    attn_scale = float(p["attn1"][0] + p["attn2"][0] + p["attn3"][0] + p["attn4"][0])
    w["_attn_scale"] = (attn_scale, None)
    w["tempvec"] = f32(np.repeat(p["temperature"].reshape(HEADS), CPH).reshape(96, 1))

    g2v, b2v = p["ln2_g"], p["ln2_b"]
    f1 = p["fc1_w"][:, :, 0, 0]  # [256, 192]
    f1_eff = f1 * g2v[None, :]
    fc1 = np.zeros((96, 4 * 128), np.float32)
    for mg in range(2):
        for cg in range(2):
            fc1[:, (mg * 2 + cg) * 128:(mg * 2 + cg + 1) * 128] = \
                f1_eff[mg * 128:(mg + 1) * 128, cg * 96:(cg + 1) * 96].T
    w["fc1T"] = f32r(fc1)
    w["fc1_bias"] = f32((f1 @ b2v).reshape(2, 128).T)  # [128, 2]

    s1 = p["bn1_g"] / np.sqrt(p["bn1_v"] + eps_bn)
    t1 = p["bn1_b"] - p["bn1_m"] * s1
    s2 = p["bn2_g"] / np.sqrt(p["bn2_v"] + eps_bn)
    t2 = p["bn2_b"] - p["bn2_m"] * s2
    s3 = p["bn3_g"] / np.sqrt(p["bn3_v"] + eps_bn)
    t3 = p["bn3_b"] - p["bn3_m"] * s3

    dw1w, dw2w, dw3w = p["dw1_w"][:, 0], p["dw2_w"][:, 0], p["dw3_w"][:, 0]
    dw1b, dw2b, dw3b = p["dw1_b"], p["dw2_b"], p["dw3_b"]
    s1g = [s1[i * 64:(i + 1) * 64] for i in range(4)]
    t1g = [t1[i * 64:(i + 1) * 64] for i in range(4)]

    pair_d = np.zeros((128, 25 * 128), np.float32)
    for t in range(25):
        dy, dx = t // 5 - 2, t % 5 - 2
        blk = np.zeros((128, 128), np.float32)
        d2 = dw2w[:, dy + 2, dx + 2] * s1g[2]
        if dy == 0 and dx == 0:
            d2 = d2 + s1g[2]
        blk[64:, 64:] = np.diag(d2)
        if -1 <= dy <= 1 and -1 <= dx <= 1:
            d1 = dw1w[:, dy + 1, dx + 1] * s1g[1]
            if dy == 0 and dx == 0:
                d1 = d1 + s1g[1]
            blk[:64, :64] = np.diag(d1)
        pair_d[:, t * 128:(t + 1) * 128] = blk
    w["pair_diag"] = f32r(pair_d)
    bc1 = t1g[1] * dw1w.sum((1, 2)) + dw1b + t1g[1]
    bc2 = t1g[2] * dw2w.sum((1, 2)) + dw2b + t1g[2]
    w["pair_bias"] = f32(np.concatenate([bc1, bc2]).reshape(128, 1))

    # rows 64:128 of v0t2 hold the same data stored shifted +1, so a read at
    # AP offset (dy, dxa) yields tap (dy, dxa-1) for those rows.
    dw3_passes = []
    for dy in range(-3, 4):
        for dxa in (-2, 0, 2):
            dw3_passes.append((dy, dxa, True))
        dw3_passes.append((dy, 3, False))
    dw3_d = np.zeros((128, len(dw3_passes) * 64), np.float32)
    for i, (dy, dxa, hasb) in enumerate(dw3_passes):
        wa = dw3w[:, dy + 3, dxa + 3] * s1g[3]
        if dy == 0 and dxa == 0:
            wa = wa + s1g[3]
        dw3_d[:64, i * 64:(i + 1) * 64] = np.diag(wa)
        if hasb:
            wb = dw3w[:, dy + 3, dxa - 1 + 3] * s1g[3]
            if dy == 0 and dxa - 1 == 0:
                wb = wb + s1g[3]
            dw3_d[64:, i * 64:(i + 1) * 64] = np.diag(wb)
    w["dw3_diag"] = f32r(dw3_d)
    w["_dw3_passes"] = (dw3_passes, None)
    w["dw3_bias"] = f32((t1g[3] * dw3w.sum((1, 2)) + dw3b + t1g[3]).reshape(64, 1))

    d0w, d0b = p["dw0_w"][:, 0, 0, 0], p["dw0_b"]
    w["g0_scale"] = f32(((d0w + 1.0) * s1g[0]).reshape(64, 1))
    w["g0_bias"] = f32(((d0w + 1.0) * t1g[0] + d0b).reshape(64, 1))

    f2 = p["fc2_w"][:, :, 0, 0]  # [192, 256]
    f2a = f2 * s2[None, :]
    f2b = f2 * (t2 * s1)[None, :]
    cstv = f2 @ (t2 * t1)
    fc2a = np.zeros((128, 4 * 96), np.float32)
    for cg in range(2):
        for kg in range(2):
            fc2a[:, (cg * 2 + kg) * 96:(cg * 2 + kg + 1) * 96] = \
                f2a[cg * 96:(cg + 1) * 96, kg * 128:(kg + 1) * 128].T
    w["fc2aT"] = f32r(fc2a)
    w["fc2bT_g0"] = f32r(np.concatenate(
        [f2b[cg * 96:(cg + 1) * 96, 0:64].T for cg in range(2)], axis=1))    # [64, 192]
    w["fc2bT_g12"] = f32r(np.concatenate(
        [f2b[cg * 96:(cg + 1) * 96, 64:192].T for cg in range(2)], axis=1))  # [128, 192]
    w["fc2bT_g3"] = f32r(np.concatenate(
        [f2b[cg * 96:(cg + 1) * 96, 192:256].T for cg in range(2)], axis=1))  # [64, 192]
    w["s3v"] = f32(np.stack([s3[:96], s3[96:]], axis=1))          # [96, 2]
    w["out_bias"] = f32(np.stack([(s3 * 0 + t3 + s3 * cstv * 0)[:96], (t3)[96:]], axis=1))
    # careful: out = s3*(psum + cst) + t3 + xc' ; psum excludes cst, so bias = s3*cst + t3
    ob = s3 * cstv + t3
    w["out_bias"] = f32(np.stack([ob[:96], ob[96:]], axis=1))     # [96, 2]

    sg = np.where(s1 == 0, 1.0, s1)
    padv = -t1 / sg
    w["padv1"] = f32(np.concatenate([padv[64:128], padv[128:192]]).reshape(128, 1))
    w["padv2"] = f32(np.concatenate([padv[192:256], padv[192:256]]).reshape(128, 1))
    w["s1a"] = f32(s1[:128].reshape(128, 1))
    w["s1b"] = f32(s1[128:].reshape(128, 1))
    w["t1a"] = f32(t1[:128].reshape(128, 1))
    w["t1b"] = f32(t1[128:].reshape(128, 1))

    w["ones_st"] = f32r(np.full((96, 128), 1.0 / EMBED, np.float32))
    w["epsv"] = f32(np.full((128, 1), 1e-6, np.float32))
    vm = np.zeros((96, 96), np.float32)
    for h in range(HEADS):
        vm[h * CPH:(h + 1) * CPH, h * CPH:(h + 1) * CPH] = 1.0
    w["vmask"] = f32(vm)
    return w


WSPEC = {
    "ident": ([128, 128], F32), "pos_diag": ([96, 18 * 96], F32R),
    "pos_b": ([96, 2], F32), "qkv_wT": ([96, 3 * 96], F32R),
    "qkv_bias": ([96, 3], F32), "qdw_diag": ([96, 27 * 96], F32R),
    "gate_w1T": ([96, 192], F32R), "gate_b1": ([96, 1], F32),
    "gate_w2T": ([96, 1], F32R), "gate_b2": ([1, 1], F32),
    "proj1T": ([96, 192], F32R), "proj2T": ([96, 192], F32R),
    "proj_bias": ([96, 2], F32), "tempvec": ([96, 1], F32),
    "fc1T": ([96, 4 * 128], F32R), "fc1_bias": ([128, 2], F32),
    "pair_diag": ([128, 25 * 128], F32R), "pair_bias": ([128, 1], F32),
    "dw3_diag": ([128, 28 * 64], F32R), "dw3_bias": ([64, 1], F32),
    "g0_scale": ([64, 1], F32), "g0_bias": ([64, 1], F32),
    "fc2aT": ([128, 4 * 96], F32R), "fc2bT_g0": ([64, 192], F32R),
    "fc2bT_g12": ([128, 192], F32R), "fc2bT_g3": ([64, 192], F32R),
    "s3v": ([96, 2], F32), "out_bias": ([96, 2], F32),
    "padv1": ([128, 1], F32),
    "padv2": ([128, 1], F32),
    "s1a": ([128, 1], F32), "s1b": ([128, 1], F32),
    "t1a": ([128, 1], F32), "t1b": ([128, 1], F32),
    "ones_st": ([96, 128], F32R),
    "epsv": ([128, 1], F32),
    "vmask": ([96, 96], F32),
}


# ----------------------------------------------------------------------------
# device kernel
# ----------------------------------------------------------------------------

def build(nc, H, W, n_cores, attn_scale, dw3_passes):
    S = H * W
    Wp1 = W + 2
    P1B = (BR + 2) * Wp1 + 2 * SLOP   # band buffer (pad1)
    Wp3, Hp3 = W + 6, H + 6
    P3 = Hp3 * Wp3 + 2 * SLOP
    NCH = _ceil(H, RC)
    NB = _ceil(H, BR)
    NSC = _ceil(S, 512)
    GCH = 512 // W                    # gate chunk rows (512 cols)
    NGC_PER_BAND = _ceil(BR, GCH)

    x_t = nc.dram_tensor("x", [H, W, EMBED], F32, kind="ExternalInput")
    out_t = nc.dram_tensor("out", [S, EMBED], F32, kind="ExternalOutput")
    wt = {k: nc.dram_tensor("w_" + k, shp, d, kind="ExternalInput")
          for k, (shp, d) in WSPEC.items()}

    def pd3(r):
        return SLOP + r * Wp3

    with tile.TileContext(nc) as tc:
        C_ONLY_W = ['fc1T', 'fc1_bias', 'pair_diag', 'pair_bias', 'dw3_diag', 'dw3_bias', 'g0_scale', 'g0_bias', 'fc2aT', 'fc2bT_g0', 'fc2bT_g12', 'fc2bT_g3', 's3v', 'out_bias', 's1a', 's1b', 't1a', 't1b', 'padv1', 'padv2']
        with (
            tc.tile_pool(name="dram", bufs=1, space="DRAM") as dram,
            tc.tile_pool(name="persist", bufs=1) as pers,
        ):
            ws = {}

            def _load_w(pool, names):
                for k in names:
                    shp, d = WSPEC[k]
                    tl = pool.tile(shp, d, tag="w_" + k, name="w_" + k)
                    nc.sync.dma_start(out=tl[:], in_=wt[k][:])
                    ws[k] = tl


            yn1_sp = dram.tile([96, S], F32R)
            yn2_sp = dram.tile([96, S], F32R)
            xc_sp = [dram.tile([96, S], F32R, name=f"xc_sp{i}") for i in range(2)]
            v_sp = dram.tile([96, S], F32R)
            xcp_sp = [dram.tile([96, S], F32R, name=f"xcp_sp{i}") for i in range(2)]
            mu2_sp = dram.tile([128, S], F32)
            rstd2_sp = dram.tile([128, S], F32)
            vg0_sp = dram.tile([64, P3], F32R)
            ug0_sp = dram.tile([64, P3], F32R)
            dbg_sp = {nm: dram.tile([128, S], F32, name="dbg_" + nm)
                      for nm in ("uga", "ugb", "vba", "vbb", "z1a", "z1b")} \
                if getattr(build, "DEBUG", False) else None
            cc_in = dram.tile([1, 1], F32)
            cc_out = dram.tile([1, 1], F32)

            gsum = pers.tile([1, NB * NGC_PER_BAND + 8], F32)
            nc.vector.memset(gsum[:], 0.0)
            dynk = pers.tile([96, 1], F32)
            probsT = pers.tile([96, 96], F32R)

            # ================= PHASE A =================
            _wpab_cm = tc.tile_pool(name="wpAB", bufs=1)
            wpab = _wpab_cm.__enter__()
            _load_w(wpab, [k for k in WSPEC if k not in C_ONLY_W])
            ident = ws["ident"]
            with (
                tc.tile_pool(name="pa_band", bufs=2) as pab,
                tc.tile_pool(name="pa_rot", bufs=3) as par,
                tc.tile_pool(name="pa_ps", bufs=2, space="PSUM") as paps,
                tc.tile_pool(name="pa_ps2", bufs=2, space="PSUM") as paps2,
            ):
                for b in range(NB):
                    r0, r1 = b * BR, min((b + 1) * BR, H)
                    xband = [pab.tile([96, P1B], F32R, tag=f"xb{cg}", name=f"xb{cg}") for cg in range(2)]
                    for cg in range(2):
                        nc.vector.memset(xband[cg][:].bitcast(F32), 0.0)
                    for rr in range(max(r0 - 1, 0), min(r1 + 1, H)):
                        xrow = par.tile([W, EMBED], F32, tag="xrow")
                        nc.sync.dma_start(out=xrow[:], in_=x_t[rr])
                        boff = SLOP + (rr - (r0 - 1)) * Wp1 + 1
                        for cg in range(2):
                            tps = paps2.tile([96, W], F32, tag="tps")
                            nc.tensor.transpose(tps[:], xrow[:, cg * 96:(cg + 1) * 96],
                                                ident[:W, :W])
                            nc.scalar.copy(xband[cg][:, boff:boff + W], tps[:])
                    for c0 in range(r0, r1, RC):
                        nr_c = min(RC, H - c0)
                        N = nr_c * Wp1
                        NN = nr_c * W
                        sb0 = SLOP + (c0 - r0 + 1) * Wp1
                        xc_ch = [par.tile([96, RC * W], F32R, tag=f"xc{cg}", name=f"xc{cg}") for cg in range(2)]
                        xsq = [par.tile([96, RC * W], F32R, tag=f"xq{cg}", name=f"xq{cg}") for cg in range(2)]
                        for cg in range(2):
                            ps = paps.tile([96, RC * Wp1], F32, tag="posps")
                            for t in range(9):
                                dy, dx = t // 3 - 1, t % 3 - 1
                                o = sb0 + dy * Wp1 + dx
                                nc.tensor.matmul(
                                    ps[:, :N],
                                    ws["pos_diag"][:, (t * 2 + cg) * 96:(t * 2 + cg + 1) * 96],
                                    xband[cg][:, o:o + N],
                                    start=(t == 0), stop=(t == 8))
                            ps_int = ps[:, :N].rearrange("p (r w) -> p r w", w=Wp1)[:, :, 1:1 + W]
                            xb_int = xband[cg][:, sb0:sb0 + N] \
                                .rearrange("p (r w) -> p r w", w=Wp1)[:, :, 1:1 + W]
                            xcv = xc_ch[cg][:, :NN].rearrange("p (r w) -> p r w", w=W)
                            nc.vector.scalar_tensor_tensor(
                                out=xcv, in0=ps_int, scalar=ws["pos_b"][:, cg:cg + 1],
                                in1=xb_int, op0=Alu.add, op1=Alu.add)
                            nc.scalar.square(xsq[cg][:, :NN], xc_ch[cg][:, :NN])
                        mu_ps = paps.tile([128, RC * W], F32, tag="mups")
                        m2_ps = paps.tile([128, RC * W], F32, tag="m2ps")
                        for cg in range(2):
                            nc.tensor.matmul(mu_ps[:, :NN], ws["ones_st"], xc_ch[cg][:, :NN],
                                             start=(cg == 0), stop=(cg == 1))
                            nc.tensor.matmul(m2_ps[:, :NN], ws["ones_st"], xsq[cg][:, :NN],
                                             start=(cg == 0), stop=(cg == 1))
                        musq = par.tile([128, RC * W], F32, tag="musq")
                        nc.scalar.square(musq[:, :NN], mu_ps[:, :NN])
                        var = par.tile([128, RC * W], F32, tag="var")
                        nc.vector.tensor_tensor(out=var[:, :NN], in0=m2_ps[:, :NN],
                                                in1=musq[:, :NN], op=Alu.subtract)
                        sd = par.tile([128, RC * W], F32, tag="sd")
                        nc.scalar.activation(sd[:, :NN], var[:, :NN], Act.Sqrt, bias=ws["epsv"])
                        rstd = par.tile([128, RC * W], F32, tag="rstd")
                        nc.vector.reciprocal(rstd[:, :NN], sd[:, :NN])
                        for cg in range(2):
                            tdf = par.tile([96, RC * W], F32, tag=f"td{cg}")
                            nc.vector.tensor_tensor(out=tdf[:, :NN], in0=xc_ch[cg][:, :NN],
                                                    in1=mu_ps[:96, :NN], op=Alu.subtract)
                            ynch = par.tile([96, RC * W], F32R, tag=f"yn{cg}")
                            nc.vector.tensor_tensor(out=ynch[:, :NN], in0=tdf[:, :NN],
                                                    in1=rstd[:96, :NN], op=Alu.mult)
                            sp = yn1_sp if cg == 0 else yn2_sp
                            nc.sync.dma_start(out=sp[:, c0 * W:c0 * W + NN],
                                              in_=ynch[:, :NN])
                            nc.sync.dma_start(out=xc_sp[cg][:, c0 * W:c0 * W + NN],
                                              in_=xc_ch[cg][:, :NN])

            # ================= PHASE B =================
            with (
                tc.tile_pool(name="pb_band", bufs=1) as pbb,
                tc.tile_pool(name="pb_rot", bufs=3) as pbr,
                tc.tile_pool(name="gram_ps", bufs=1, space="PSUM") as gpsp,
            ):
                g1_ps = gpsp.tile([96, 384], F32)
                g2_ps = gpsp.tile([96, 288], F32)
                with (
                    tc.tile_pool(name="pb_psg", bufs=1, space="PSUM") as pbpsg,
                    tc.tile_pool(name="pb_ps", bufs=2, space="PSUM") as pbps,
                ):
                    for b in range(NB):
                        r0, r1 = b * BR, min((b + 1) * BR, H)
                        ylo, yhi = max(r0 - 1, 0), min(r1 + 1, H)
                        ynb = [pbb.tile([96, (BR + 2) * W], F32R, tag=f"ynb{cg}", name=f"ynb{cg}")
                               for cg in range(2)]
                        for cg in range(2):
                            sp = yn1_sp if cg == 0 else yn2_sp
                            nc.sync.dma_start(
                                out=ynb[cg][:, (ylo - r0 + 1) * W:(yhi - r0 + 1) * W],
                                in_=sp[:, ylo * W:yhi * W])
                        # gate (512-col chunks over rows [r0, r1))
                        for gi in range(NGC_PER_BAND):
                            gr0 = r0 + gi * GCH
                            if gr0 >= r1:
                                break
                            ngr = min(GCH, r1 - gr0)
                            NG = ngr * W
                            yo = (gr0 - r0 + 1) * W
                            gps = pbpsg.tile([96, 512], F32, tag="gps")
                            for cg in range(2):
                                nc.tensor.matmul(gps[:, :NG],
                                                 ws["gate_w1T"][:, cg * 96:(cg + 1) * 96],
                                                 ynb[cg][:, yo:yo + NG],
                                                 start=(cg == 0), stop=(cg == 1))
                            g1s = pbr.tile([96, 512], F32R, tag="g1s")
                            nc.scalar.activation(g1s[:, :NG], gps[:, :NG], Act.Relu,
                                                 bias=ws["gate_b1"])
                            g2ps = pbpsg.tile([1, 512], F32, tag="g2ps")
                            nc.tensor.matmul(g2ps[:, :NG], ws["gate_w2T"], g1s[:, :NG],
                                             start=True, stop=True)
                            sgt = pbr.tile([1, 512], F32, tag="sgt")
                            idx = b * NGC_PER_BAND + gi
                            nc.scalar.activation(sgt[:, :NG], g2ps[:, :NG], Act.Sigmoid,
                                                 bias=ws["gate_b2"],
                                                 accum_out=gsum[0:1, idx:idx + 1])
                        # qkv0 band
                        qkv0 = [pbb.tile([96, P1B], F32R, tag=f"qk0{j}", name=f"qk0{j}") for j in range(3)]
                        for j in range(3):
                            nc.vector.memset(qkv0[j][:].bitcast(F32), 0.0)
                        for rr in range(ylo, yhi, 2):
                            nrw = min(2, yhi - rr)
                            NQ = nrw * W
                            for j in range(3):
                                qps = pbps.tile([96, 2 * W], F32, tag="qps")
                                nc.tensor.matmul(qps[:, :NQ],
                                                 ws["qkv_wT"][:, j * 96:(j + 1) * 96],
                                                 ynb[0][:, (rr - r0 + 1) * W:(rr - r0 + 1) * W + NQ],
                                                 start=True, stop=True)
                                dst = SLOP + (rr - r0 + 1) * Wp1 + 1
                                dview = qkv0[j][:, dst:dst + nrw * Wp1] \
                                    .rearrange("p (r w) -> p r w", w=Wp1)[:, :, 0:W]
                                nc.scalar.activation(
                                    dview, qps[:, :NQ].rearrange("p (r w) -> p r w", w=W),
                                    Act.Identity, bias=ws["qkv_bias"][:, j:j + 1])
                        # depthwise + hi/lo + transpose staging
                        qkband = pbr.tile([W, BR * 384], BF16, tag="qkband")
                        for c0 in range(r0, r1, RC):
                            nr_c = min(RC, H - c0)
                            N = nr_c * Wp1
                            NN = nr_c * W
                            sb0 = SLOP + (c0 - r0 + 1) * Wp1
                            hilo = {}
                            for j in range(3):
                                ps = pbps.tile([96, RC * Wp1], F32, tag="dwps")
                                for t in range(9):
                                    dy, dx = t // 3 - 1, t % 3 - 1
                                    o = sb0 + dy * Wp1 + dx
                                    nc.tensor.matmul(
                                        ps[:, :N],
                                        ws["qdw_diag"][:, (t * 3 + j) * 96:(t * 3 + j + 1) * 96],
                                        qkv0[j][:, o:o + N],
                                        start=(t == 0), stop=(t == 8))
                                ps_int = ps[:, :N].rearrange("p (r w) -> p r w", w=Wp1)[:, :, 1:1 + W]
                                if j == 2:
                                    vch = pbr.tile([96, RC * W], F32R, tag="vch")
                                    nc.scalar.copy(
                                        vch[:, :NN].rearrange("p (r w) -> p r w", w=W), ps_int)
                                    nc.sync.dma_start(out=v_sp[:, c0 * W:c0 * W + NN],
                                                      in_=vch[:, :NN])
                                else:
                                    hi = pbr.tile([96, RC * W], BF16, tag=f"hi{j}")
                                    lo = pbr.tile([96, RC * W], BF16, tag=f"lo{j}")
                                    hiv = hi[:, :NN].rearrange("p (r w) -> p r w", w=W)
                                    nc.scalar.copy(hiv, ps_int)
                                    nc.vector.tensor_tensor(
                                        out=lo[:, :NN].rearrange("p (r w) -> p r w", w=W),
                                        in0=ps_int, in1=hiv, op=Alu.subtract)
                                    hilo[j] = (hi, lo)
                            for rr in range(c0, c0 + nr_c):
                                ro = (rr - r0) * 384
                                rl = (rr - c0) * W
                                for idx, src in enumerate((hilo[0][0], hilo[1][0],
                                                           hilo[1][1], hilo[0][1])):
                                    nc.sync.dma_start_transpose(
                                        out=qkband[:, ro + idx * 96:ro + (idx + 1) * 96],
                                        in_=src[:, rl:rl + W])
                        for rr in range(r0, r1):
                            ro = (rr - r0) * 384
                            nc.tensor.matmul(g1_ps[:], qkband[:, ro:ro + 96],
                                             qkband[:, ro:ro + 384],
                                             start=(rr == 0), stop=(rr == H - 1))
                            nc.tensor.matmul(g2_ps[:], qkband[:, ro + 96:ro + 192],
                                             qkband[:, ro + 96:ro + 384],
                                             start=(rr == 0), stop=(rr == H - 1))

                # ---- gate mean -> AllReduce -> dynk ----
                gred = pers.tile([1, 1], F32)
                nc.vector.reduce_sum(gred[:], gsum[0:1, 0:NB * NGC_PER_BAND], axis=AX)
                gsc = pers.tile([1, 1], F32)
                nc.vector.tensor_scalar_mul(gsc[:], gred[:], float(CPH) / (n_cores * S))
                nc.sync.dma_start(out=cc_in[:], in_=gsc[:])
                nc.gpsimd.collective_compute(
                    "AllReduce", Alu.add, replica_groups=[list(range(n_cores))],
                    ins=[cc_in.opt()], outs=[cc_out.opt()])
                nc.sync.dma_start(out=dynk[:], in_=cc_out[:].partition_broadcast(96))

                # ---- attn block ----
                with (
                    tc.tile_pool(name="at_ps", bufs=2, space="PSUM") as atps,
                    tc.tile_pool(name="at_sb", bufs=1) as ab,
                ):
                    g1sb = ab.tile([96, 384], F32)
                    nc.scalar.copy(g1sb[:], g1_ps[:])
                    g2sb = ab.tile([96, 288], F32)
                    nc.scalar.copy(g2sb[:], g2_ps[:])
                    lohi_ps = atps.tile([96, 96], F32, tag="atp")
                    nc.tensor.transpose(lohi_ps[:], g2sb[:, 192:288], ident[:96, :96])
                    gq = ab.tile([96, 96], F32)
                    nc.vector.tensor_tensor(out=gq[:], in0=g1sb[:, 96:192],
                                            in1=g1sb[:, 192:288], op=Alu.add)
                    gqk = ab.tile([96, 96], F32)
                    nc.vector.tensor_tensor(out=gqk[:], in0=gq[:], in1=lohi_ps[:], op=Alu.add)
                    idm = ident[:96, :96]
                    tq = ab.tile([96, 96], F32)
                    nc.vector.tensor_tensor(out=tq[:], in0=g1sb[:, 0:96], in1=idm, op=Alu.mult)
                    nq2 = ab.tile([96, 1], F32)
                    nc.vector.reduce_sum(nq2[:], tq[:], axis=AX)
                    ksm = ab.tile([96, 96], F32)
                    nc.vector.scalar_tensor_tensor(out=ksm[:], in0=g2sb[:, 96:192], scalar=2.0,
                                                   in1=g2sb[:, 0:96], op0=Alu.mult, op1=Alu.add)
                    tk = ab.tile([96, 96], F32)
                    nc.vector.tensor_tensor(out=tk[:], in0=ksm[:], in1=idm, op=Alu.mult)
                    nk2 = ab.tile([96, 1], F32)
                    nc.vector.reduce_sum(nk2[:], tk[:], axis=AX)

                    def rsqrt_clamped(nm, src):
                        sq = ab.tile([96, 1], F32, tag=nm + "sq")
                        nc.scalar.sqrt(sq[:], src[:])
                        cl = ab.tile([96, 1], F32, tag=nm + "cl")
                        nc.vector.tensor_scalar_max(cl[:], sq[:], 1e-12)
                        rvv = ab.tile([96, 1], F32, tag=nm)
                        nc.vector.reciprocal(rvv[:], cl[:])
                        return rvv

                    rq = rsqrt_clamped("rq", nq2)
                    rk = rsqrt_clamped("rk", nk2)
                    rqt = ab.tile([96, 1], F32)
                    nc.vector.tensor_tensor(out=rqt[:], in0=rq[:], in1=ws["tempvec"][:],
                                            op=Alu.mult)
                    asr = ab.tile([96, 96], F32)
                    nc.vector.tensor_scalar_mul(asr[:], gqk[:], rqt[:])
                    as_ps = atps.tile([96, 96], F32, tag="atp")
                    nc.tensor.transpose(as_ps[:], asr[:], ident[:96, :96])
                    ast = ab.tile([96, 96], F32)
                    nc.vector.tensor_scalar_mul(ast[:], as_ps[:], rk[:])
                    as2_ps = atps.tile([96, 96], F32, tag="atp")
                    nc.tensor.transpose(as2_ps[:], ast[:], ident[:96, :96])
                    as2 = ab.tile([96, 96], F32)
                    nc.scalar.copy(as2[:], as2_ps[:])
                    # mask off-head-block entries to -60
                    t60 = ab.tile([96, 96], F32)
                    nc.vector.tensor_scalar_add(t60[:], as2[:], 60.0)
                    amf = ab.tile([96, 96], F32)
                    nc.vector.tensor_tensor(out=amf[:], in0=t60[:], in1=ws["vmask"][:],
                                            op=Alu.mult)
                    nc.vector.tensor_scalar_add(amf[:], amf[:], -60.0)
                    # rank+1 over full row via pairwise is_ge
                    rnk3 = ab.tile([96, 96 * 96], F32)
                    a_i = amf[:].unsqueeze(1).broadcast_to([96, 96, 96])
                    a_d = amf[:].unsqueeze(2).broadcast_to([96, 96, 96])
                    rvw = rnk3[:].rearrange("p (i d) -> p i d", d=96)
                    nc.vector.tensor_tensor(out=rvw, in0=a_i, in1=a_d, op=Alu.is_ge)
                    rank1 = ab.tile([96, 96], F32)
                    nc.vector.reduce_sum(rank1[:].unsqueeze(2), rvw, axis=AX)
                    sel = ab.tile([96, 96], F32)
                    nc.vector.tensor_tensor(out=sel[:], in0=rank1[:],
                                            in1=dynk[:].broadcast_to([96, 96]), op=Alu.is_le)
                    am = ab.tile([96, 96], F32)
                    t60b = ab.tile([96, 96], F32)
                    nc.vector.tensor_scalar_add(t60b[:], amf[:], 60.0)
                    nc.vector.tensor_tensor(out=am[:], in0=t60b[:], in1=sel[:], op=Alu.mult)
                    nc.vector.tensor_scalar_add(am[:], am[:], -60.0)
                    mx = ab.tile([96, 1], F32)
                    nc.vector.reduce_max(mx[:], am[:], axis=AX)
                    nmx = ab.tile([96, 1], F32)
                    nc.vector.tensor_scalar_mul(nmx[:], mx[:], -1.0)
                    ex = ab.tile([96, 96], F32)
                    nc.scalar.activation(ex[:], am[:], Act.Exp, bias=nmx[:])
                    sme = ab.tile([96, 1], F32)
                    nc.vector.reduce_sum(sme[:], ex[:], axis=AX)
                    rsm = ab.tile([96, 1], F32)
                    nc.vector.reciprocal(rsm[:], sme[:])
                    probs = ab.tile([96, 96], F32)
                    nc.vector.tensor_scalar_mul(probs[:], ex[:], rsm[:])
                    pt_ps = atps.tile([96, 96], F32, tag="atp2")
                    nc.tensor.transpose(pt_ps[:], probs[:], ident[:96, :96])
                    nc.scalar.copy(probsT[:], pt_ps[:])

            # ================= PHASE B5 =================
            with (
                tc.tile_pool(name="b5_rot", bufs=3) as b5r,
                tc.tile_pool(name="b5_ps", bufs=1, space="PSUM") as b5ps,
            ):
                for ci in range(NSC):
                    o0 = ci * 512
                    NN = min(512, S - o0)
                    vch = b5r.tile([96, 512], F32R, tag="vch")
                    nc.sync.dma_start(out=vch[:, :NN], in_=v_sp[:, o0:o0 + NN])
                    av_ps = b5ps.tile([96, 512], F32, tag="avps")
                    nc.tensor.matmul(av_ps[:, :NN], probsT[:], vch[:, :NN],
                                     start=True, stop=True)
                    avs = b5r.tile([96, 512], F32R, tag="avs")
                    nc.scalar.activation(avs[:, :NN], av_ps[:, :NN], Act.Copy,
                                         scale=attn_scale)
                    x2ch = b5r.tile([96, 512], F32R, tag="x2ch")
                    nc.sync.dma_start(out=x2ch[:, :NN], in_=yn2_sp[:, o0:o0 + NN])
                    xpch = [b5r.tile([96, 512], F32R, tag=f"xp{cg}", name=f"xp{cg}") for cg in range(2)]
                    xsq = [b5r.tile([96, 512], F32R, tag=f"xs{cg}", name=f"xs{cg}") for cg in range(2)]
                    for cg in range(2):
                        xcch = b5r.tile([96, 512], F32R, tag=f"xcc{cg}")
                        nc.sync.dma_start(out=xcch[:, :NN], in_=xc_sp[cg][:, o0:o0 + NN])
                        pj_ps = b5ps.tile([96, 512], F32, tag=f"pjps{cg}")
                        nc.tensor.matmul(pj_ps[:, :NN],
                                         ws["proj1T"][:, cg * 96:(cg + 1) * 96],
                                         avs[:, :NN], start=True, stop=False)
                        nc.tensor.matmul(pj_ps[:, :NN],
                                         ws["proj2T"][:, cg * 96:(cg + 1) * 96],
                                         x2ch[:, :NN], start=False, stop=True)
                        nc.vector.scalar_tensor_tensor(
                            out=xpch[cg][:, :NN], in0=pj_ps[:, :NN],
                            scalar=ws["proj_bias"][:, cg:cg + 1], in1=xcch[:, :NN],
                            op0=Alu.add, op1=Alu.add)
                        nc.sync.dma_start(out=xcp_sp[cg][:, o0:o0 + NN],
                                          in_=xpch[cg][:, :NN])
                        nc.scalar.square(xsq[cg][:, :NN], xpch[cg][:, :NN])
                    mu_ps = b5ps.tile([128, 512], F32, tag="mu2ps")
                    m2_ps = b5ps.tile([128, 512], F32, tag="m22ps")
                    for cg in range(2):
                        nc.tensor.matmul(mu_ps[:, :NN], ws["ones_st"], xpch[cg][:, :NN],
                                         start=(cg == 0), stop=(cg == 1))
                        nc.tensor.matmul(m2_ps[:, :NN], ws["ones_st"], xsq[cg][:, :NN],
                                         start=(cg == 0), stop=(cg == 1))
                    musq = b5r.tile([128, 512], F32, tag="musq2")
                    nc.scalar.square(musq[:, :NN], mu_ps[:, :NN])
                    var = b5r.tile([128, 512], F32, tag="var2")
                    nc.vector.tensor_tensor(out=var[:, :NN], in0=m2_ps[:, :NN],
                                            in1=musq[:, :NN], op=Alu.subtract)
                    sd = b5r.tile([128, 512], F32, tag="sd2")
                    nc.scalar.activation(sd[:, :NN], var[:, :NN], Act.Sqrt, bias=ws["epsv"])
                    rstd = b5r.tile([128, 512], F32, tag="rstd2")
                    nc.vector.reciprocal(rstd[:, :NN], sd[:, :NN])
                    muc = b5r.tile([128, 512], F32, tag="muc")
                    nc.vector.tensor_copy(out=muc[:, :NN], in_=mu_ps[:, :NN])
                    nc.sync.dma_start(out=mu2_sp[:, o0:o0 + NN], in_=muc[:, :NN])
                    nc.sync.dma_start(out=rstd2_sp[:, o0:o0 + NN], in_=rstd[:, :NN])

            _wpab_cm.__exit__(None, None, None)
            # ================= PHASE C =================
            _wpc_cm = tc.tile_pool(name="wpC", bufs=1)
            wpc = _wpc_cm.__enter__()
            _load_w(wpc, C_ONLY_W)
            with tc.tile_pool(name="c_v0", bufs=1) as cv0:
                v0t1 = cv0.tile([128, P3], F32R)
                v0t2 = cv0.tile([128, P3], F32R)
                with (
                    tc.tile_pool(name="c1_rot", bufs=2) as c1r,
                    tc.tile_pool(name="c1_ps", bufs=2, space="PSUM") as c1ps,
                ):
                    nc.vector.memset(v0t1[:].bitcast(F32), 0.0)
                    nc.vector.memset(v0t2[:].bitcast(F32), 0.0)
                    # pad cells must hold -t1/s1 so the bn-folded depthwise
                    # reads zeros in v0_bn space at image borders
                    nc.vector.tensor_scalar_add(v0t1[:], v0t1[:], ws["padv1"])
                    nc.vector.tensor_scalar_add(v0t2[:], v0t2[:], ws["padv2"])
                    for ci in range(NCH):
                        c0 = ci * RC
                        nr_c = min(RC, H - c0)
                        NN = nr_c * W
                        o0 = c0 * W
                        xpch = [c1r.tile([96, RC * W], F32R, tag=f"cx{cg}", name=f"cx{cg}") for cg in range(2)]
                        for cg in range(2):
                            nc.sync.dma_start(out=xpch[cg][:, :NN],
                                              in_=xcp_sp[cg][:, o0:o0 + NN])
                        mub = c1r.tile([128, RC * W], F32, tag="cmu")
                        rsb = c1r.tile([128, RC * W], F32, tag="crs")
                        nc.sync.dma_start(out=mub[:, :NN], in_=mu2_sp[:, o0:o0 + NN])
                        nc.sync.dma_start(out=rsb[:, :NN], in_=rstd2_sp[:, o0:o0 + NN])
                        yn2t = [c1r.tile([96, RC * W], F32R, tag=f"cy{cg}", name=f"cy{cg}") for cg in range(2)]
                        for cg in range(2):
                            td = c1r.tile([96, RC * W], F32, tag=f"ctd{cg}")
                            nc.vector.tensor_tensor(out=td[:, :NN], in0=xpch[cg][:, :NN],
                                                    in1=mub[:96, :NN], op=Alu.subtract)
                            nc.vector.tensor_tensor(out=yn2t[cg][:, :NN], in0=td[:, :NN],
                                                    in1=rsb[:96, :NN], op=Alu.mult)
                        for mg in range(2):
                            fps = c1ps.tile([128, RC * W], F32, tag="fps")
                            for cg in range(2):
                                nc.tensor.matmul(
                                    fps[:, :NN],
                                    ws["fc1T"][:, (mg * 2 + cg) * 128:(mg * 2 + cg + 1) * 128],
                                    yn2t[cg][:, :NN], start=(cg == 0), stop=(cg == 1))
                            if mg == 0:
                                vg0 = c1r.tile([64, RC * W], F32R, tag="vg0")
                                nc.scalar.activation(vg0[:, :NN], fps[0:64, :NN], Act.Gelu,
                                                     bias=ws["fc1_bias"][0:64, 0:1])
                                ug0 = c1r.tile([64, RC * W], F32R, tag="ug0")
                                nc.scalar.activation(ug0[:, :NN], vg0[:, :NN], Act.Gelu,
                                                     bias=ws["g0_bias"], scale=ws["g0_scale"])
                                for r in range(nr_c):
                                    d0 = pd3(3 + c0 + r) + 3
                                    nc.sync.dma_start(out=vg0_sp[:, d0:d0 + W],
                                                      in_=vg0[:, r * W:(r + 1) * W])
                                    nc.sync.dma_start(out=ug0_sp[:, d0:d0 + W],
                                                      in_=ug0[:, r * W:(r + 1) * W])
                                    nc.scalar.activation(
                                        v0t1[0:64, d0:d0 + W],
                                        fps[64:128, r * W:(r + 1) * W], Act.Gelu,
                                        bias=ws["fc1_bias"][64:128, 0:1])
                            else:
                                for r in range(nr_c):
                                    d0 = pd3(3 + c0 + r) + 3
                                    nc.scalar.activation(
                                        v0t1[64:128, d0:d0 + W],
                                        fps[0:64, r * W:(r + 1) * W], Act.Gelu,
                                        bias=ws["fc1_bias"][0:64, 1:2])
                                    nc.scalar.activation(
                                        v0t2[0:64, d0:d0 + W],
                                        fps[64:128, r * W:(r + 1) * W], Act.Gelu,
                                        bias=ws["fc1_bias"][64:128, 1:2])
                                    nc.scalar.activation(
                                        v0t2[64:128, d0 + 1:d0 + 1 + W],
                                        fps[64:128, r * W:(r + 1) * W], Act.Gelu,
                                        bias=ws["fc1_bias"][64:128, 1:2])

                with (
                    tc.tile_pool(name="c2_rot", bufs=2) as c2r,
                    tc.tile_pool(name="c2_ps", bufs=2, space="PSUM") as c2ps,
                ):
                    for ci in range(NCH):
                        c0 = ci * RC
                        nr_c = min(RC, H - c0)
                        N = nr_c * Wp3
                        NN = nr_c * W
                        sb0 = pd3(3 + c0)
                        ps_a = c2ps.tile([128, RC * Wp3], F32, tag="psa")
                        for t in range(25):
                            dy, dx = t // 5 - 2, t % 5 - 2
                            o = sb0 + dy * Wp3 + dx
                            nc.tensor.matmul(ps_a[:, :N],
                                             ws["pair_diag"][:, t * 128:(t + 1) * 128],
                                             v0t1[:, o:o + N],
                                             start=(t == 0), stop=(t == 24))
                        ps_b = c2ps.tile([64, RC * Wp3], F32, tag="psb")
                        for i, (dy, dxa, hasb) in enumerate(dw3_passes):
                            o = sb0 + dy * Wp3 + dxa
                            nc.tensor.matmul(ps_b[:, :N],
                                             ws["dw3_diag"][:, i * 64:(i + 1) * 64],
                                             v0t2[:, o:o + N],
                                             start=(i == 0), stop=(i == len(dw3_passes) - 1))

                        def inner(ap_flat, lo, hi):
                            # interior view of a PSUM chunk (starts at free 0)
                            return ap_flat[lo:hi, :N].rearrange(
                                "p (r w) -> p r w", w=Wp3)[:, :, 3:3 + W]

                        def inner_v0(ap_flat, lo, hi):
                            # interior view of the padded v0 buffers at this chunk
                            return ap_flat[lo:hi, sb0:sb0 + N].rearrange(
                                "p (r w) -> p r w", w=Wp3)[:, :, 3:3 + W]

                        ug_a = c2r.tile([128, RC * W], F32R, tag="uga")
                        ug_b = c2r.tile([128, RC * W], F32R, tag="ugb")
                        vb_a = c2r.tile([128, RC * W], F32, tag="vba")
                        vb_b = c2r.tile([128, RC * W], F32, tag="vbb")
                        g0v = c2r.tile([64, RC * W], F32R, tag="g0v")
                        src3 = vg0_sp[:, sb0:sb0 + N].rearrange(
                            "p (r w) -> p r w", w=Wp3)[:, :, 3:3 + W]
                        nc.sync.dma_start(
                            out=g0v[:, :NN].rearrange("p (r w) -> p r w", w=W), in_=src3)
                        usrc3 = ug0_sp[:, sb0:sb0 + N].rearrange(
                            "p (r w) -> p r w", w=Wp3)[:, :, 3:3 + W]
                        nc.sync.dma_start(
                            out=ug_a[0:64, :NN].rearrange("p (r w) -> p r w", w=W), in_=usrc3)
                        nc.scalar.activation(
                            ug_a[64:128, :NN].rearrange("p (r w) -> p r w", w=W),
                            inner(ps_a, 0, 64), Act.Gelu, bias=ws["pair_bias"][0:64])
                        nc.scalar.activation(
                            ug_b[0:64, :NN].rearrange("p (r w) -> p r w", w=W),
                            inner(ps_a, 64, 128), Act.Gelu, bias=ws["pair_bias"][64:128])
                        nc.scalar.activation(
                            ug_b[64:128, :NN].rearrange("p (r w) -> p r w", w=W),
                            inner(ps_b, 0, 64), Act.Gelu, bias=ws["dw3_bias"])
                        nc.vector.tensor_scalar(out=vb_a[0:64, :NN], in0=g0v[:, :NN],
                                                scalar1=ws["s1a"][0:64],
                                                scalar2=ws["t1a"][0:64],
                                                op0=Alu.mult, op1=Alu.add)
                        nc.vector.tensor_scalar(out=vb_a[64:128, :NN],
                                                in0=inner_v0(v0t1, 0, 64),
                                                scalar1=ws["s1a"][64:128],
                                                scalar2=ws["t1a"][64:128],
                                                op0=Alu.mult, op1=Alu.add)
                        nc.vector.tensor_scalar(out=vb_b[0:64, :NN],
                                                in0=inner_v0(v0t1, 64, 128),
                                                scalar1=ws["s1b"][0:64],
                                                scalar2=ws["t1b"][0:64],
                                                op0=Alu.mult, op1=Alu.add)
                        nc.vector.tensor_scalar(out=vb_b[64:128, :NN],
                                                in0=inner_v0(v0t2, 0, 64),
                                                scalar1=ws["s1b"][64:128],
                                                scalar2=ws["t1b"][64:128],
                                                op0=Alu.mult, op1=Alu.add)
                        z1a = c2r.tile([128, RC * W], F32R, tag="z1a")
                        z1b = c2r.tile([128, RC * W], F32R, tag="z1b")
                        nc.vector.tensor_tensor(out=z1a[:, :NN], in0=ug_a[:, :NN],
                                                in1=vb_a[:, :NN], op=Alu.mult)
                        nc.vector.tensor_tensor(out=z1b[:, :NN], in0=ug_b[:, :NN],
                                                in1=vb_b[:, :NN], op=Alu.mult)
                        if dbg_sp is not None:
                            for nm, tl in (("uga", ug_a), ("ugb", ug_b), ("vba", vb_a),
                                           ("vbb", vb_b), ("z1a", z1a), ("z1b", z1b)):
                                nc.sync.dma_start(
                                    out=dbg_sp[nm][:, c0 * W:c0 * W + NN],
                                    in_=tl[:, :NN].bitcast(F32))
                        for cg in range(2):
                            ops = c2ps.tile([96, RC * W], F32, tag=f"ops{cg}")
                            nc.tensor.matmul(ops[:, :NN],
                                             ws["fc2aT"][:, (cg * 2) * 96:(cg * 2 + 1) * 96],
                                             z1a[:, :NN], start=True, stop=False)
                            nc.tensor.matmul(ops[:, :NN],
                                             ws["fc2aT"][:, (cg * 2 + 1) * 96:(cg * 2 + 2) * 96],
                                             z1b[:, :NN], start=False, stop=False)
                            nc.tensor.matmul(ops[:, :NN],
                                             ws["fc2bT_g0"][:, cg * 96:(cg + 1) * 96],
                                             g0v[:, :NN], start=False, stop=False)
                            opsv = ops[:, :NN].rearrange("p (r w) -> p r w", w=W)
                            nc.tensor.matmul(opsv,
                                             ws["fc2bT_g12"][:, cg * 96:(cg + 1) * 96],
                                             inner_v0(v0t1, 0, 128), start=False, stop=False)
                            nc.tensor.matmul(opsv,
                                             ws["fc2bT_g3"][:, cg * 96:(cg + 1) * 96],
                                             inner_v0(v0t2, 0, 64), start=False, stop=True)
                            xrch = c2r.tile([96, RC * W], F32R, tag=f"xr{cg}", bufs=1)
                            nc.sync.dma_start(out=xrch[:, :NN],
                                              in_=xcp_sp[cg][:, c0 * W:c0 * W + NN])
                            ob = c2r.tile([96, RC * W], F32, tag=f"ob{cg}", bufs=1)
                            nc.vector.tensor_scalar(out=ob[:, :NN], in0=ops[:, :NN],
                                                    scalar1=ws["s3v"][:, cg:cg + 1],
                                                    scalar2=ws["out_bias"][:, cg:cg + 1],
                                                    op0=Alu.mult, op1=Alu.add)
                            oc = c2r.tile([96, RC * W], F32, tag=f"oc{cg}", bufs=1)
                            nc.vector.tensor_tensor(out=oc[:, :NN], in0=ob[:, :NN],
                                                    in1=xrch[:, :NN], op=Alu.add)
                            nc.sync.dma_start(
                                out=out_t[c0 * W:c0 * W + NN,
                                          cg * 96:(cg + 1) * 96].rearrange("n c -> c n"),
                                in_=oc[:, :NN])
            _wpc_cm.__exit__(None, None, None)
    return out_t.name


# ----------------------------------------------------------------------------
# host entry
# ----------------------------------------------------------------------------

_CACHE = {}


def make_program(H, W, n_cores, attn_scale, dw3_passes):
    key = (H, W, n_cores, round(attn_scale, 9))
    if key in _CACHE:
        return _CACHE[key]
    nc = bacc.Bacc("TRN2", target_bir_lowering=False, debug=False, num_devices=n_cores)
    out_name = build(nc, H, W, n_cores, attn_scale, dw3_passes)
    nc.compile()
    _CACHE[key] = (nc, out_name)
    return nc, out_name


def make_in_maps(inputs):
    x = np.asarray(inputs["x"], np.float32)
    B = x.shape[0]
    wdict = _prep_weights({k: np.asarray(v) for k, v in inputs.items()})
    base = {}
    for k, (shp, d) in WSPEC.items():
        base["w_" + k] = wdict[k][0].reshape(shp)
    in_maps = []
    for b in range(B):
        m = dict(base)
        m["x"] = np.ascontiguousarray(x[b])
        in_maps.append(m)
    return in_maps, wdict


def kernel(**inputs):
    x = np.asarray(inputs["x"], np.float32)
    B, H, W, C = x.shape
    in_maps, wdict = make_in_maps(inputs)
    nc, out_name = make_program(H, W, B, wdict["_attn_scale"][0],
                                wdict["_dw3_passes"][0])
    res = bass_utils.run_bass_kernel_spmd(nc, in_maps, core_ids=list(range(B)))
    return np.stack([res.results[b][out_name].reshape(H, W, C) for b in range(B)])



# revision 15
# speedup vs baseline: 4.8818x; 4.8818x over previous
"""Trainium2 Bass kernel for nn_Block_87351044866235 (sparse_attention).

Data-parallel over batch: 8 samples -> 8 NeuronCores. Channel-major
layout [C, H*W] on chip; depthwise convs as diagonal fp32r matmuls on
TensorE; 1x1 convs as fp32r matmuls; LN stats via ones-matmuls; q/k gram
via hi/lo bf16 split + DMA-xbar transposes; dynamic-k gate mean via a
scalar AllReduce.
"""
import sys, os

for _p in ("/opt/trn_rl_repo", "/root/.axon_site/_ro/trn_rl_repo"):
    if os.path.isdir(_p) and _p not in sys.path:
        sys.path.append(_p)

import numpy as np
import ml_dtypes
import concourse.bass as bass
import concourse.bacc as bacc
import concourse.tile as tile
from concourse import mybir
from concourse import bass_utils

try:
    from concourse import tile_utils as _tu
    _tu.max_sbuf_usage = 208 * 1024
except Exception:
    pass

dt = mybir.dt
Alu = mybir.AluOpType
Act = mybir.ActivationFunctionType
AX = mybir.AxisListType.X

EMBED, PDIM, HEADS, HID = 192, 96, 8, 256
CPH = PDIM // HEADS  # 12
SLOP = 8
RC = 3    # conv output rows per chunk
BR = 12   # rows per band

F32, F32R, BF16 = dt.float32, dt.float32r, dt.bfloat16


def _ceil(a, b):
    return (a + b - 1) // b


# ----------------------------------------------------------------------------
# host-side weight prep: everything 2D [partitions, free]
# ----------------------------------------------------------------------------

def _prep_weights(p):
    w = {}
    f32r = lambda a: (np.ascontiguousarray(a, np.float32), F32R)
    f32 = lambda a: (np.ascontiguousarray(a, np.float32), F32)
    eps_bn = 1e-5

    w["ident"] = f32(np.eye(128, dtype=np.float32))
    w["identb"] = (np.eye(128, dtype=np.float32).astype(ml_dtypes.bfloat16), BF16)

    # pos depthwise diag: [96, (t*2+cg)*96]
    pw = p["pos_w"][:, 0]  # [192,3,3]
    pos_d = np.zeros((96, 18 * 96), np.float32)
    for t in range(9):
        dy, dx = t // 3 - 1, t % 3 - 1
        for cg in range(2):
            pos_d[:, (t * 2 + cg) * 96:(t * 2 + cg + 1) * 96] = \
                np.diag(pw[cg * 96:(cg + 1) * 96, dy + 1, dx + 1])
    w["pos_diag"] = f32r(pos_d)
    w["pos_b"] = f32(p["pos_b"].reshape(2, 96).T)  # [96, 2]

    g1v, b1v = p["ln1_g"], p["ln1_b"]
    qw = p["qkv_w"][:, :, 0, 0]  # [288, 96]
    qw_eff = qw * g1v[None, :96]
    w["qkv_wT"] = f32r(np.concatenate(
        [qw_eff[j * 96:(j + 1) * 96].T for j in range(3)], axis=1))  # [96, 3*96]
    w["qkv_bias"] = f32((qw @ b1v[:96]).reshape(3, 96).T)  # [96, 3]

    qdw = p["qkv_dw_w"][:, 0]  # [288,3,3]
    qdw_d = np.zeros((96, 27 * 96), np.float32)
    for t in range(9):
        dy, dx = t // 3 - 1, t % 3 - 1
        for j in range(3):
            qdw_d[:, (t * 3 + j) * 96:(t * 3 + j + 1) * 96] = \
                np.diag(qdw[j * 96:(j + 1) * 96, dy + 1, dx + 1])
    w["qdw_diag"] = f32r(qdw_d)

    gw1 = p["gate_w1"][:, :, 0, 0]  # [96, 192]
    gw1_eff = gw1 * g1v[None, :]
    w["gate_w1T"] = f32r(np.concatenate(
        [gw1_eff[:, cg * 96:(cg + 1) * 96].T for cg in range(2)], axis=1))  # [96, 192]
    w["gate_b1"] = f32((p["gate_b1"] + gw1 @ b1v).reshape(96, 1))
    w["gate_w2T"] = f32r(p["gate_w2"][:, :, 0, 0].T.copy())  # [96,1]
    w["gate_b2"] = f32(p["gate_b2"].reshape(1, 1))

    pj = p["proj_w"][:, :, 0, 0]
    pj1, pj2 = pj[:, :96], pj[:, 96:] * g1v[None, 96:]
    w["proj1T"] = f32r(np.concatenate(
        [pj1[cg * 96:(cg + 1) * 96].T for cg in range(2)], axis=1))  # [96, 192]
    w["proj2T"] = f32r(np.concatenate(
        [pj2[cg * 96:(cg + 1) * 96].T for cg in range(2)], axis=1))
    w["proj_bias"] = f32((pj[:, 96:] @ b1v[96:]).reshape(2, 96).T)  # [96, 2]

    attn_scale = float(p["attn1"][0] + p["attn2"][0] + p["attn3"][0] + p["attn4"][0])
    w["_attn_scale"] = (attn_scale, None)
    w["tempvec"] = f32(np.repeat(p["temperature"].reshape(HEADS), CPH).reshape(96, 1))

    g2v, b2v = p["ln2_g"], p["ln2_b"]
    f1 = p["fc1_w"][:, :, 0, 0]  # [256, 192]
    f1_eff = f1 * g2v[None, :]
    fc1 = np.zeros((96, 4 * 128), np.float32)
    for mg in range(2):
        for cg in range(2):
            fc1[:, (mg * 2 + cg) * 128:(mg * 2 + cg + 1) * 128] = \
                f1_eff[mg * 128:(mg + 1) * 128, cg * 96:(cg + 1) * 96].T
    w["fc1T"] = f32r(fc1)
    w["fc1_bias"] = f32((f1 @ b2v).reshape(2, 128).T)  # [128, 2]

    s1 = p["bn1_g"] / np.sqrt(p["bn1_v"] + eps_bn)
    t1 = p["bn1_b"] - p["bn1_m"] * s1
    s2 = p["bn2_g"] / np.sqrt(p["bn2_v"] + eps_bn)
    t2 = p["bn2_b"] - p["bn2_m"] * s2
    s3 = p["bn3_g"] / np.sqrt(p["bn3_v"] + eps_bn)
    t3 = p["bn3_b"] - p["bn3_m"] * s3

    dw1w, dw2w, dw3w = p["dw1_w"][:, 0], p["dw2_w"][:, 0], p["dw3_w"][:, 0]
    dw1b, dw2b, dw3b = p["dw1_b"], p["dw2_b"], p["dw3_b"]
    s1g = [s1[i * 64:(i + 1) * 64] for i in range(4)]
    t1g = [t1[i * 64:(i + 1) * 64] for i in range(4)]

    pair_d = np.zeros((128, 25 * 128), np.float32)
    for t in range(25):
        dy, dx = t // 5 - 2, t % 5 - 2
        blk = np.zeros((128, 128), np.float32)
        d2 = dw2w[:, dy + 2, dx + 2] * s1g[2]
        if dy == 0 and dx == 0:
            d2 = d2 + s1g[2]
        blk[64:, 64:] = np.diag(d2)
        if -1 <= dy <= 1 and -1 <= dx <= 1:
            d1 = dw1w[:, dy + 1, dx + 1] * s1g[1]
            if dy == 0 and dx == 0:
                d1 = d1 + s1g[1]
            blk[:64, :64] = np.diag(d1)
        pair_d[:, t * 128:(t + 1) * 128] = blk
    w["pair_diag"] = f32r(pair_d)
    bc1 = t1g[1] * dw1w.sum((1, 2)) + dw1b + t1g[1]
    bc2 = t1g[2] * dw2w.sum((1, 2)) + dw2b + t1g[2]
    w["pair_bias"] = f32(np.concatenate([bc1, bc2]).reshape(128, 1))

    # rows 64:128 of v0t2 hold the same data stored shifted +1, so a read at
    # AP offset (dy, dxa) yields tap (dy, dxa-1) for those rows.
    dw3_passes = []
    for dy in range(-3, 4):
        for dxa in (-2, 0, 2):
            dw3_passes.append((dy, dxa, True))
        dw3_passes.append((dy, 3, False))
    dw3_d = np.zeros((128, len(dw3_passes) * 64), np.float32)
    for i, (dy, dxa, hasb) in enumerate(dw3_passes):
        wa = dw3w[:, dy + 3, dxa + 3] * s1g[3]
        if dy == 0 and dxa == 0:
            wa = wa + s1g[3]
        dw3_d[:64, i * 64:(i + 1) * 64] = np.diag(wa)
        if hasb:
            wb = dw3w[:, dy + 3, dxa - 1 + 3] * s1g[3]
            if dy == 0 and dxa - 1 == 0:
                wb = wb + s1g[3]
            dw3_d[64:, i * 64:(i + 1) * 64] = np.diag(wb)
    w["dw3_diag"] = f32r(dw3_d)
    w["_dw3_passes"] = (dw3_passes, None)
    w["dw3_bias"] = f32((t1g[3] * dw3w.sum((1, 2)) + dw3b + t1g[3]).reshape(64, 1))

    d0w, d0b = p["dw0_w"][:, 0, 0, 0], p["dw0_b"]
    w["g0_scale"] = f32(((d0w + 1.0) * s1g[0]).reshape(64, 1))
    w["g0_bias"] = f32(((d0w + 1.0) * t1g[0] + d0b).reshape(64, 1))

    f2 = p["fc2_w"][:, :, 0, 0]  # [192, 256]
    f2a = f2 * s2[None, :]
    f2b = f2 * (t2 * s1)[None, :]
    cstv = f2 @ (t2 * t1)
    fc2a = np.zeros((128, 4 * 96), np.float32)
    for cg in range(2):
        for kg in range(2):
            fc2a[:, (cg * 2 + kg) * 96:(cg * 2 + kg + 1) * 96] = \
                f2a[cg * 96:(cg + 1) * 96, kg * 128:(kg + 1) * 128].T
    w["fc2aT"] = f32r(fc2a)
    w["fc2bT_g0"] = f32r(np.concatenate(
        [f2b[cg * 96:(cg + 1) * 96, 0:64].T for cg in range(2)], axis=1))    # [64, 192]
    w["fc2bT_g12"] = f32r(np.concatenate(
        [f2b[cg * 96:(cg + 1) * 96, 64:192].T for cg in range(2)], axis=1))  # [128, 192]
    w["fc2bT_g3"] = f32r(np.concatenate(
        [f2b[cg * 96:(cg + 1) * 96, 192:256].T for cg in range(2)], axis=1))  # [64, 192]
    w["s3v"] = f32(np.stack([s3[:96], s3[96:]], axis=1))          # [96, 2]
    w["out_bias"] = f32(np.stack([(s3 * 0 + t3 + s3 * cstv * 0)[:96], (t3)[96:]], axis=1))
    # careful: out = s3*(psum + cst) + t3 + xc' ; psum excludes cst, so bias = s3*cst + t3
    ob = s3 * cstv + t3
    w["out_bias"] = f32(np.stack([ob[:96], ob[96:]], axis=1))     # [96, 2]

    sg = np.where(s1 == 0, 1.0, s1)
    padv = -t1 / sg
    w["padv1"] = f32(np.concatenate([padv[64:128], padv[128:192]]).reshape(128, 1))
    w["padv2"] = f32(np.concatenate([padv[192:256], padv[192:256]]).reshape(128, 1))
    w["s1a"] = f32(s1[:128].reshape(128, 1))
    w["s1b"] = f32(s1[128:].reshape(128, 1))
    w["t1a"] = f32(t1[:128].reshape(128, 1))
    w["t1b"] = f32(t1[128:].reshape(128, 1))

    w["ones_st"] = f32r(np.full((96, 128), 1.0 / EMBED, np.float32))
    w["epsv"] = f32(np.full((128, 1), 1e-6, np.float32))
    vm = np.zeros((96, 96), np.float32)
    for h in range(HEADS):
        vm[h * CPH:(h + 1) * CPH, h * CPH:(h + 1) * CPH] = 1.0
    w["vmask"] = f32(vm)
    return w


WSPEC = {
    "ident": ([128, 128], F32), "identb": ([128, 128], BF16),
    "pos_diag": ([96, 18 * 96], F32R),
    "pos_b": ([96, 2], F32), "qkv_wT": ([96, 3 * 96], F32R),
    "qkv_bias": ([96, 3], F32), "qdw_diag": ([96, 27 * 96], F32R),
    "gate_w1T": ([96, 192], F32R), "gate_b1": ([96, 1], F32),
    "gate_w2T": ([96, 1], F32R), "gate_b2": ([1, 1], F32),
    "proj1T": ([96, 192], F32R), "proj2T": ([96, 192], F32R),
    "proj_bias": ([96, 2], F32), "tempvec": ([96, 1], F32),
    "fc1T": ([96, 4 * 128], F32R), "fc1_bias": ([128, 2], F32),
    "pair_diag": ([128, 25 * 128], F32R), "pair_bias": ([128, 1], F32),
    "dw3_diag": ([128, 28 * 64], F32R), "dw3_bias": ([64, 1], F32),
    "g0_scale": ([64, 1], F32), "g0_bias": ([64, 1], F32),
    "fc2aT": ([128, 4 * 96], F32R), "fc2bT_g0": ([64, 192], F32R),
    "fc2bT_g12": ([128, 192], F32R), "fc2bT_g3": ([64, 192], F32R),
    "s3v": ([96, 2], F32), "out_bias": ([96, 2], F32),
    "padv1": ([128, 1], F32),
    "padv2": ([128, 1], F32),
    "s1a": ([128, 1], F32), "s1b": ([128, 1], F32),
    "t1a": ([128, 1], F32), "t1b": ([128, 1], F32),
    "ones_st": ([96, 128], F32R),
    "epsv": ([128, 1], F32),
    "vmask": ([96, 96], F32),
}


# ----------------------------------------------------------------------------
# device kernel
# ----------------------------------------------------------------------------

def build(nc, H, W, n_cores, attn_scale, dw3_passes):
    S = H * W
    Wp1 = W + 2
    P1B = (BR + 2) * Wp1 + 2 * SLOP   # band buffer (pad1)
    Wp3, Hp3 = W + 6, H + 6
    P3 = Hp3 * Wp3 + 2 * SLOP
    NCH = _ceil(H, RC)
    NB = _ceil(H, BR)
    NSC = _ceil(S, 512)
    GCH = 512 // W                    # gate chunk rows (512 cols)
    NGC_PER_BAND = _ceil(BR, GCH)

    x_t = nc.dram_tensor("x", [H, W, EMBED], F32, kind="ExternalInput")
    # channel-major output: [192, S]; host does the final (H,W,C) transpose
    out_t = nc.dram_tensor("out", [EMBED, S], F32, kind="ExternalOutput")
    wt = {k: nc.dram_tensor("w_" + k, shp, d, kind="ExternalInput")
          for k, (shp, d) in WSPEC.items()}

    def pd3(r):
        return SLOP + r * Wp3

    with tile.TileContext(nc) as tc:
        C_ONLY_W = ['fc1T', 'fc1_bias', 'pair_diag', 'pair_bias', 'dw3_diag', 'dw3_bias', 'g0_scale', 'g0_bias', 'fc2aT', 'fc2bT_g0', 'fc2bT_g12', 'fc2bT_g3', 's3v', 'out_bias', 's1a', 's1b', 't1a', 't1b', 'padv1', 'padv2']
        with (
            tc.tile_pool(name="dram", bufs=1, space="DRAM") as dram,
            tc.tile_pool(name="persist", bufs=1) as pers,
        ):
            ws = {}

            def _load_w(pool, names):
                for k in names:
                    shp, d = WSPEC[k]
                    tl = pool.tile(shp, d, tag="w_" + k, name="w_" + k)
                    nc.sync.dma_start(out=tl[:], in_=wt[k][:])
                    ws[k] = tl


            yn1_sp = dram.tile([96, S], F32R)
            yn2_sp = dram.tile([96, S], F32R)
            xc_sp = [dram.tile([96, S], F32R, name=f"xc_sp{i}") for i in range(2)]
            v_sp = dram.tile([96, S], F32R)
            xcp_sp = [dram.tile([96, S], F32R, name=f"xcp_sp{i}") for i in range(2)]
            mu2_sp = dram.tile([128, S], F32)
            rstd2_sp = dram.tile([128, S], F32)
            vg0_sp = dram.tile([64, P3], F32R)
            ug0_sp = dram.tile([64, P3], F32R)
            dbg_sp = {nm: dram.tile([128, S], F32, name="dbg_" + nm)
                      for nm in ("uga", "ugb", "vba", "vbb", "z1a", "z1b")} \
                if getattr(build, "DEBUG", False) else None
            cc_in = dram.tile([1, 1], F32)
            cc_out = dram.tile([1, 1], F32)

            gsum = pers.tile([1, NB * NGC_PER_BAND + 8], F32)
            nc.vector.memset(gsum[:], 0.0)
            dynk = pers.tile([96, 1], F32)
            probsT = pers.tile([96, 96], F32R)

            # ================= PHASE A =================
            _wpab_cm = tc.tile_pool(name="wpAB", bufs=1)
            wpab = _wpab_cm.__enter__()
            _load_w(wpab, [k for k in WSPEC if k not in C_ONLY_W])
            ident = ws["ident"]
            with (
                tc.tile_pool(name="pa_band", bufs=2) as pab,
                tc.tile_pool(name="pa_rot", bufs=3) as par,
                tc.tile_pool(name="pa_ps", bufs=2, space="PSUM") as paps,
                tc.tile_pool(name="pa_ps2", bufs=2, space="PSUM") as paps2,
            ):
                for b in range(NB):
                    r0, r1 = b * BR, min((b + 1) * BR, H)
                    ylo, yhi = max(r0 - 1, 0), min(r1 + 1, H)
                    nrb = yhi - ylo
                    xband = [pab.tile([96, P1B], F32R, tag=f"xb{cg}", name=f"xb{cg}") for cg in range(2)]
                    for cg in range(2):
                        nc.vector.memset(xband[cg][:].bitcast(F32), 0.0)
                    xraw = pab.tile([W, (BR + 2) * EMBED], F32, tag="xraw")
                    nc.sync.dma_start(
                        out=xraw[:, :nrb * EMBED].rearrange("w (r c) -> w r c", c=EMBED),
                        in_=x_t[ylo:yhi].rearrange("r w c -> w r c"))
                    for rr in range(ylo, yhi):
                        boff = SLOP + (rr - (r0 - 1)) * Wp1 + 1
                        for cg in range(2):
                            tps = paps2.tile([96, W], F32, tag="tps")
                            nc.tensor.transpose(
                                tps[:],
                                xraw[:, (rr - ylo) * EMBED + cg * 96:
                                     (rr - ylo) * EMBED + (cg + 1) * 96],
                                ident[:W, :W])
                            nc.scalar.copy(xband[cg][:, boff:boff + W], tps[:])
                    xcband = [pab.tile([96, BR * W], F32R, tag=f"xcb{cg}", name=f"xcb{cg}")
                              for cg in range(2)]
                    ynband = [pab.tile([96, BR * W], F32R, tag=f"ynb{cg}", name=f"ynb{cg}")
                              for cg in range(2)]
                    for c0 in range(r0, r1, RC):
                        nr_c = min(RC, H - c0)
                        N = nr_c * Wp1
                        NN = nr_c * W
                        bo = (c0 - r0) * W
                        sb0 = SLOP + (c0 - r0 + 1) * Wp1
                        xc_ch = [xcband[cg][:, bo:bo + NN] for cg in range(2)]
                        xsq = [par.tile([96, RC * W], F32R, tag=f"xq{cg}", name=f"xq{cg}") for cg in range(2)]
                        for cg in range(2):
                            ps = paps.tile([96, RC * Wp1], F32, tag="posps")
                            for t in range(9):
                                dy, dx = t // 3 - 1, t % 3 - 1
                                o = sb0 + dy * Wp1 + dx
                                nc.tensor.matmul(
                                    ps[:, :N],
                                    ws["pos_diag"][:, (t * 2 + cg) * 96:(t * 2 + cg + 1) * 96],
                                    xband[cg][:, o:o + N],
                                    start=(t == 0), stop=(t == 8))
                            ps_int = ps[:, :N].rearrange("p (r w) -> p r w", w=Wp1)[:, :, 1:1 + W]
                            xb_int = xband[cg][:, sb0:sb0 + N] \
                                .rearrange("p (r w) -> p r w", w=Wp1)[:, :, 1:1 + W]
                            xcv = xc_ch[cg].rearrange("p (r w) -> p r w", w=W)
                            nc.vector.scalar_tensor_tensor(
                                out=xcv, in0=ps_int, scalar=ws["pos_b"][:, cg:cg + 1],
                                in1=xb_int, op0=Alu.add, op1=Alu.add)
                            nc.scalar.square(xsq[cg][:, :NN], xc_ch[cg])
                        mu_ps = paps.tile([128, RC * W], F32, tag="mups")
                        m2_ps = paps.tile([128, RC * W], F32, tag="m2ps")
                        for cg in range(2):
                            nc.tensor.matmul(mu_ps[:, :NN], ws["ones_st"], xc_ch[cg],
                                             start=(cg == 0), stop=(cg == 1))
                            nc.tensor.matmul(m2_ps[:, :NN], ws["ones_st"], xsq[cg][:, :NN],
                                             start=(cg == 0), stop=(cg == 1))
                        musq = par.tile([128, RC * W], F32, tag="musq")
                        nc.scalar.square(musq[:, :NN], mu_ps[:, :NN])
                        var = par.tile([128, RC * W], F32, tag="var")
                        nc.vector.tensor_tensor(out=var[:, :NN], in0=m2_ps[:, :NN],
                                                in1=musq[:, :NN], op=Alu.subtract)
                        sd = par.tile([128, RC * W], F32, tag="sd")
                        nc.scalar.activation(sd[:, :NN], var[:, :NN], Act.Sqrt, bias=ws["epsv"])
                        rstd = par.tile([128, RC * W], F32, tag="rstd")
                        nc.vector.reciprocal(rstd[:, :NN], sd[:, :NN])
                        for cg in range(2):
                            tdf = par.tile([96, RC * W], F32, tag=f"td{cg}")
                            nc.vector.tensor_tensor(out=tdf[:, :NN], in0=xc_ch[cg],
                                                    in1=mu_ps[:96, :NN], op=Alu.subtract)
                            nc.vector.tensor_tensor(out=ynband[cg][:, bo:bo + NN],
                                                    in0=tdf[:, :NN],
                                                    in1=rstd[:96, :NN], op=Alu.mult)
                    NBW = (r1 - r0) * W
                    for cg in range(2):
                        sp = yn1_sp if cg == 0 else yn2_sp
                        nc.sync.dma_start(out=sp[:, r0 * W:r0 * W + NBW],
                                          in_=ynband[cg][:, :NBW])
                        nc.sync.dma_start(out=xc_sp[cg][:, r0 * W:r0 * W + NBW],
                                          in_=xcband[cg][:, :NBW])

            # ================= PHASE B =================
            with (
                tc.tile_pool(name="pb_band", bufs=1) as pbb,
                tc.tile_pool(name="pb_rot", bufs=3) as pbr,
                tc.tile_pool(name="gram_ps", bufs=1, space="PSUM") as gpsp,
            ):
                g1_ps = gpsp.tile([96, 384], F32)
                g2_ps = gpsp.tile([96, 288], F32)
                with (
                    tc.tile_pool(name="pb_psg", bufs=1, space="PSUM") as pbpsg,
                    tc.tile_pool(name="pb_ps", bufs=2, space="PSUM") as pbps,
                    tc.tile_pool(name="pb_tps", bufs=2, space="PSUM") as tpsp,
                ):
                    for b in range(NB):
                        r0, r1 = b * BR, min((b + 1) * BR, H)
                        ylo, yhi = max(r0 - 1, 0), min(r1 + 1, H)
                        ynb = [pbb.tile([96, (BR + 2) * W], F32R, tag=f"ynb{cg}", name=f"ynb{cg}")
                               for cg in range(2)]
                        for cg in range(2):
                            sp = yn1_sp if cg == 0 else yn2_sp
                            nc.sync.dma_start(
                                out=ynb[cg][:, (ylo - r0 + 1) * W:(yhi - r0 + 1) * W],
                                in_=sp[:, ylo * W:yhi * W])
                        # gate (512-col chunks over rows [r0, r1))
                        for gi in range(NGC_PER_BAND):
                            gr0 = r0 + gi * GCH
                            if gr0 >= r1:
                                break
                            ngr = min(GCH, r1 - gr0)
                            NG = ngr * W
                            yo = (gr0 - r0 + 1) * W
                            gps = pbpsg.tile([96, 512], F32, tag="gps")
                            for cg in range(2):
                                nc.tensor.matmul(gps[:, :NG],
                                                 ws["gate_w1T"][:, cg * 96:(cg + 1) * 96],
                                                 ynb[cg][:, yo:yo + NG],
                                                 start=(cg == 0), stop=(cg == 1))
                            g1s = pbr.tile([96, 512], F32R, tag="g1s")
                            nc.scalar.activation(g1s[:, :NG], gps[:, :NG], Act.Relu,
                                                 bias=ws["gate_b1"])
                            g2ps = pbpsg.tile([1, 512], F32, tag="g2ps")
                            nc.tensor.matmul(g2ps[:, :NG], ws["gate_w2T"], g1s[:, :NG],
                                             start=True, stop=True)
                            sgt = pbr.tile([1, 512], F32, tag="sgt")
                            idx = b * NGC_PER_BAND + gi
                            nc.scalar.activation(sgt[:, :NG], g2ps[:, :NG], Act.Sigmoid,
                                                 bias=ws["gate_b2"],
                                                 accum_out=gsum[0:1, idx:idx + 1])
                        # qkv0 band
                        qkv0 = [pbb.tile([96, P1B], F32R, tag=f"qk0{j}", name=f"qk0{j}") for j in range(3)]
                        for j in range(3):
                            nc.vector.memset(qkv0[j][:].bitcast(F32), 0.0)
                        for rr in range(ylo, yhi, 2):
                            nrw = min(2, yhi - rr)
                            NQ = nrw * W
                            for j in range(3):
                                qps = pbps.tile([96, RC * Wp1], F32, tag="ps")
                                nc.tensor.matmul(qps[:, :NQ],
                                                 ws["qkv_wT"][:, j * 96:(j + 1) * 96],
                                                 ynb[0][:, (rr - r0 + 1) * W:(rr - r0 + 1) * W + NQ],
                                                 start=True, stop=True)
                                dst = SLOP + (rr - r0 + 1) * Wp1 + 1
                                dview = qkv0[j][:, dst:dst + nrw * Wp1] \
                                    .rearrange("p (r w) -> p r w", w=Wp1)[:, :, 0:W]
                                nc.scalar.activation(
                                    dview, qps[:, :NQ].rearrange("p (r w) -> p r w", w=W),
                                    Act.Identity, bias=ws["qkv_bias"][:, j:j + 1])
                        # depthwise + hi/lo + transpose staging
                        qkband = pbr.tile([W, BR * 384], BF16, tag="qkband")
                        for c0 in range(r0, r1, RC):
                            nr_c = min(RC, H - c0)
                            N = nr_c * Wp1
                            NN = nr_c * W
                            sb0 = SLOP + (c0 - r0 + 1) * Wp1
                            hilo = {}
                            for j in range(3):
                                ps = pbps.tile([96, RC * Wp1], F32, tag="ps")
                                for t in range(9):
                                    dy, dx = t // 3 - 1, t % 3 - 1
                                    o = sb0 + dy * Wp1 + dx
                                    nc.tensor.matmul(
                                        ps[:, :N],
                                        ws["qdw_diag"][:, (t * 3 + j) * 96:(t * 3 + j + 1) * 96],
                                        qkv0[j][:, o:o + N],
                                        start=(t == 0), stop=(t == 8))
                                ps_int = ps[:, :N].rearrange("p (r w) -> p r w", w=Wp1)[:, :, 1:1 + W]
                                if j == 2:
                                    vch = pbr.tile([96, RC * W], F32R, tag="vch")
                                    nc.scalar.copy(
                                        vch[:, :NN].rearrange("p (r w) -> p r w", w=W), ps_int)
                                    nc.sync.dma_start(out=v_sp[:, c0 * W:c0 * W + NN],
                                                      in_=vch[:, :NN])
                                else:
                                    hi = pbr.tile([96, RC * W], BF16, tag=f"hi{j}")
                                    lo = pbr.tile([96, RC * W], BF16, tag=f"lo{j}")
                                    hiv = hi[:, :NN].rearrange("p (r w) -> p r w", w=W)
                                    nc.scalar.copy(hiv, ps_int)
                                    nc.vector.tensor_tensor(
                                        out=lo[:, :NN].rearrange("p (r w) -> p r w", w=W),
                                        in0=ps_int, in1=hiv, op=Alu.subtract)
                                    hilo[j] = (hi, lo)
                            for rr in range(c0, c0 + nr_c):
                                ro = (rr - r0) * 384
                                rl = (rr - c0) * W
                                tq = tpsp.tile([128, 384], BF16, tag="tq")
                                for idx, src in enumerate((hilo[0][0], hilo[1][0],
                                                           hilo[1][1], hilo[0][1])):
                                    nc.tensor.transpose(
                                        tq[:, idx * 96:(idx + 1) * 96],
                                        src[:, rl:rl + W], ws["identb"][:96, :96])
                                nc.scalar.copy(qkband[:, ro:ro + 384], tq[:])
                        for rr in range(r0, r1):
                            ro = (rr - r0) * 384
                            nc.tensor.matmul(g1_ps[:], qkband[:, ro:ro + 96],
                                             qkband[:, ro:ro + 384],
                                             start=(rr == 0), stop=(rr == H - 1))
                            nc.tensor.matmul(g2_ps[:], qkband[:, ro + 96:ro + 192],
                                             qkband[:, ro + 96:ro + 384],
                                             start=(rr == 0), stop=(rr == H - 1))

                # ---- gate mean -> AllReduce -> dynk ----
                gred = pers.tile([1, 1], F32)
                nc.vector.reduce_sum(gred[:], gsum[0:1, 0:NB * NGC_PER_BAND], axis=AX)
                gsc = pers.tile([1, 1], F32)
                nc.vector.tensor_scalar_mul(gsc[:], gred[:], float(CPH) / (n_cores * S))
                nc.sync.dma_start(out=cc_in[:], in_=gsc[:])
                nc.gpsimd.collective_compute(
                    "AllReduce", Alu.add, replica_groups=[list(range(n_cores))],
                    ins=[cc_in.opt()], outs=[cc_out.opt()])
                nc.sync.dma_start(out=dynk[:], in_=cc_out[:].partition_broadcast(96))

                # ---- attn block ----
                with (
                    tc.tile_pool(name="at_ps", bufs=2, space="PSUM") as atps,
                    tc.tile_pool(name="at_sb", bufs=1) as ab,
                ):
                    g1sb = ab.tile([96, 384], F32)
                    nc.scalar.copy(g1sb[:], g1_ps[:])
                    g2sb = ab.tile([96, 288], F32)
                    nc.scalar.copy(g2sb[:], g2_ps[:])
                    lohi_ps = atps.tile([96, 96], F32, tag="atp")
                    nc.tensor.transpose(lohi_ps[:], g2sb[:, 192:288], ident[:96, :96])
                    gq = ab.tile([96, 96], F32)
                    nc.vector.tensor_tensor(out=gq[:], in0=g1sb[:, 96:192],
                                            in1=g1sb[:, 192:288], op=Alu.add)
                    gqk = ab.tile([96, 96], F32)
                    nc.vector.tensor_tensor(out=gqk[:], in0=gq[:], in1=lohi_ps[:], op=Alu.add)
                    idm = ident[:96, :96]
                    tq = ab.tile([96, 96], F32)
                    nc.vector.tensor_tensor(out=tq[:], in0=g1sb[:, 0:96], in1=idm, op=Alu.mult)
                    nq2 = ab.tile([96, 1], F32)
                    nc.vector.reduce_sum(nq2[:], tq[:], axis=AX)
                    ksm = ab.tile([96, 96], F32)
                    nc.vector.scalar_tensor_tensor(out=ksm[:], in0=g2sb[:, 96:192], scalar=2.0,
                                                   in1=g2sb[:, 0:96], op0=Alu.mult, op1=Alu.add)
                    tk = ab.tile([96, 96], F32)
                    nc.vector.tensor_tensor(out=tk[:], in0=ksm[:], in1=idm, op=Alu.mult)
                    nk2 = ab.tile([96, 1], F32)
                    nc.vector.reduce_sum(nk2[:], tk[:], axis=AX)

                    def rsqrt_clamped(nm, src):
                        sq = ab.tile([96, 1], F32, tag=nm + "sq")
                        nc.scalar.sqrt(sq[:], src[:])
                        cl = ab.tile([96, 1], F32, tag=nm + "cl")
                        nc.vector.tensor_scalar_max(cl[:], sq[:], 1e-12)
                        rvv = ab.tile([96, 1], F32, tag=nm)
                        nc.vector.reciprocal(rvv[:], cl[:])
                        return rvv

                    rq = rsqrt_clamped("rq", nq2)
                    rk = rsqrt_clamped("rk", nk2)
                    rqt = ab.tile([96, 1], F32)
                    nc.vector.tensor_tensor(out=rqt[:], in0=rq[:], in1=ws["tempvec"][:],
                                            op=Alu.mult)
                    asr = ab.tile([96, 96], F32)
                    nc.vector.tensor_scalar_mul(asr[:], gqk[:], rqt[:])
                    as_ps = atps.tile([96, 96], F32, tag="atp")
                    nc.tensor.transpose(as_ps[:], asr[:], ident[:96, :96])
                    ast = ab.tile([96, 96], F32)
                    nc.vector.tensor_scalar_mul(ast[:], as_ps[:], rk[:])
                    as2_ps = atps.tile([96, 96], F32, tag="atp")
                    nc.tensor.transpose(as2_ps[:], ast[:], ident[:96, :96])
                    as2 = ab.tile([96, 96], F32)
                    nc.scalar.copy(as2[:], as2_ps[:])
                    # mask off-head-block entries to -60
                    t60 = ab.tile([96, 96], F32)
                    nc.vector.tensor_scalar_add(t60[:], as2[:], 60.0)
                    amf = ab.tile([96, 96], F32)
                    nc.vector.tensor_tensor(out=amf[:], in0=t60[:], in1=ws["vmask"][:],
                                            op=Alu.mult)
                    nc.vector.tensor_scalar_add(amf[:], amf[:], -60.0)
                    # rank+1 over full row via pairwise is_ge
                    rnk3 = ab.tile([96, 96 * 96], F32)
                    a_i = amf[:].unsqueeze(1).broadcast_to([96, 96, 96])
                    a_d = amf[:].unsqueeze(2).broadcast_to([96, 96, 96])
                    rvw = rnk3[:].rearrange("p (i d) -> p i d", d=96)
                    nc.vector.tensor_tensor(out=rvw, in0=a_i, in1=a_d, op=Alu.is_ge)
                    rank1 = ab.tile([96, 96], F32)
                    nc.vector.reduce_sum(rank1[:].unsqueeze(2), rvw, axis=AX)
                    sel = ab.tile([96, 96], F32)
                    nc.vector.tensor_tensor(out=sel[:], in0=rank1[:],
                                            in1=dynk[:].broadcast_to([96, 96]), op=Alu.is_le)
                    am = ab.tile([96, 96], F32)
                    t60b = ab.tile([96, 96], F32)
                    nc.vector.tensor_scalar_add(t60b[:], amf[:], 60.0)
                    nc.vector.tensor_tensor(out=am[:], in0=t60b[:], in1=sel[:], op=Alu.mult)
                    nc.vector.tensor_scalar_add(am[:], am[:], -60.0)
                    mx = ab.tile([96, 1], F32)
                    nc.vector.reduce_max(mx[:], am[:], axis=AX)
                    nmx = ab.tile([96, 1], F32)
                    nc.vector.tensor_scalar_mul(nmx[:], mx[:], -1.0)
                    ex = ab.tile([96, 96], F32)
                    nc.scalar.activation(ex[:], am[:], Act.Exp, bias=nmx[:])
                    sme = ab.tile([96, 1], F32)
                    nc.vector.reduce_sum(sme[:], ex[:], axis=AX)
                    rsm = ab.tile([96, 1], F32)
                    nc.vector.reciprocal(rsm[:], sme[:])
                    probs = ab.tile([96, 96], F32)
                    nc.vector.tensor_scalar_mul(probs[:], ex[:], rsm[:])
                    pt_ps = atps.tile([96, 96], F32, tag="atp2")
                    nc.tensor.transpose(pt_ps[:], probs[:], ident[:96, :96])
                    nc.scalar.copy(probsT[:], pt_ps[:])

            # ================= PHASE B5 =================
            with (
                tc.tile_pool(name="b5_rot", bufs=3) as b5r,
                tc.tile_pool(name="b5_ps", bufs=1, space="PSUM") as b5ps,
            ):
                for ci in range(NSC):
                    o0 = ci * 512
                    NN = min(512, S - o0)
                    vch = b5r.tile([96, 512], F32R, tag="vch")
                    nc.sync.dma_start(out=vch[:, :NN], in_=v_sp[:, o0:o0 + NN])
                    av_ps = b5ps.tile([96, 512], F32, tag="avps")
                    nc.tensor.matmul(av_ps[:, :NN], probsT[:], vch[:, :NN],
                                     start=True, stop=True)
                    avs = b5r.tile([96, 512], F32R, tag="avs")
                    nc.scalar.activation(avs[:, :NN], av_ps[:, :NN], Act.Copy,
                                         scale=attn_scale)
                    x2ch = b5r.tile([96, 512], F32R, tag="x2ch")
                    nc.sync.dma_start(out=x2ch[:, :NN], in_=yn2_sp[:, o0:o0 + NN])
                    xpch = [b5r.tile([96, 512], F32R, tag=f"xp{cg}", name=f"xp{cg}") for cg in range(2)]
                    xsq = [b5r.tile([96, 512], F32R, tag=f"xs{cg}", name=f"xs{cg}") for cg in range(2)]
                    for cg in range(2):
                        xcch = b5r.tile([96, 512], F32R, tag=f"xcc{cg}")
                        nc.sync.dma_start(out=xcch[:, :NN], in_=xc_sp[cg][:, o0:o0 + NN])
                        pj_ps = b5ps.tile([96, 512], F32, tag=f"pjps{cg}")
                        nc.tensor.matmul(pj_ps[:, :NN],
                                         ws["proj1T"][:, cg * 96:(cg + 1) * 96],
                                         avs[:, :NN], start=True, stop=False)
                        nc.tensor.matmul(pj_ps[:, :NN],
                                         ws["proj2T"][:, cg * 96:(cg + 1) * 96],
                                         x2ch[:, :NN], start=False, stop=True)
                        nc.vector.scalar_tensor_tensor(
                            out=xpch[cg][:, :NN], in0=pj_ps[:, :NN],
                            scalar=ws["proj_bias"][:, cg:cg + 1], in1=xcch[:, :NN],
                            op0=Alu.add, op1=Alu.add)
                        nc.sync.dma_start(out=xcp_sp[cg][:, o0:o0 + NN],
                                          in_=xpch[cg][:, :NN])
                        nc.scalar.square(xsq[cg][:, :NN], xpch[cg][:, :NN])
                    mu_ps = b5ps.tile([128, 512], F32, tag="mu2ps")
                    m2_ps = b5ps.tile([128, 512], F32, tag="m22ps")
                    for cg in range(2):
                        nc.tensor.matmul(mu_ps[:, :NN], ws["ones_st"], xpch[cg][:, :NN],
                                         start=(cg == 0), stop=(cg == 1))
                        nc.tensor.matmul(m2_ps[:, :NN], ws["ones_st"], xsq[cg][:, :NN],
                                         start=(cg == 0), stop=(cg == 1))
                    musq = b5r.tile([128, 512], F32, tag="musq2")
                    nc.scalar.square(musq[:, :NN], mu_ps[:, :NN])
                    var = b5r.tile([128, 512], F32, tag="var2")
                    nc.vector.tensor_tensor(out=var[:, :NN], in0=m2_ps[:, :NN],
                                            in1=musq[:, :NN], op=Alu.subtract)
                    sd = b5r.tile([128, 512], F32, tag="sd2")
                    nc.scalar.activation(sd[:, :NN], var[:, :NN], Act.Sqrt, bias=ws["epsv"])
                    rstd = b5r.tile([128, 512], F32, tag="rstd2")
                    nc.vector.reciprocal(rstd[:, :NN], sd[:, :NN])
                    muc = b5r.tile([128, 512], F32, tag="muc")
                    nc.vector.tensor_copy(out=muc[:, :NN], in_=mu_ps[:, :NN])
                    nc.sync.dma_start(out=mu2_sp[:, o0:o0 + NN], in_=muc[:, :NN])
                    nc.sync.dma_start(out=rstd2_sp[:, o0:o0 + NN], in_=rstd[:, :NN])

            _wpab_cm.__exit__(None, None, None)
            # ================= PHASE C =================
            _wpc_cm = tc.tile_pool(name="wpC", bufs=1)
            wpc = _wpc_cm.__enter__()
            _load_w(wpc, C_ONLY_W)
            with tc.tile_pool(name="c_v0", bufs=1) as cv0:
                v0t1 = cv0.tile([128, P3], F32R)
                v0t2 = cv0.tile([128, P3], F32R)
                with (
                    tc.tile_pool(name="c1_rot", bufs=2) as c1r,
                    tc.tile_pool(name="c1_ps", bufs=2, space="PSUM") as c1ps,
                ):
                    nc.vector.memset(v0t1[:].bitcast(F32), 0.0)
                    nc.vector.memset(v0t2[:].bitcast(F32), 0.0)
                    # pad cells must hold -t1/s1 so the bn-folded depthwise
                    # reads zeros in v0_bn space at image borders
                    nc.vector.tensor_scalar_add(v0t1[:], v0t1[:], ws["padv1"])
                    nc.vector.tensor_scalar_add(v0t2[:], v0t2[:], ws["padv2"])
                    for ci in range(NCH):
                        c0 = ci * RC
                        nr_c = min(RC, H - c0)
                        NN = nr_c * W
                        o0 = c0 * W
                        xpch = [c1r.tile([96, RC * W], F32R, tag=f"cx{cg}", name=f"cx{cg}") for cg in range(2)]
                        for cg in range(2):
                            nc.sync.dma_start(out=xpch[cg][:, :NN],
                                              in_=xcp_sp[cg][:, o0:o0 + NN])
                        mub = c1r.tile([128, RC * W], F32, tag="cmu")
                        rsb = c1r.tile([128, RC * W], F32, tag="crs")
                        nc.sync.dma_start(out=mub[:, :NN], in_=mu2_sp[:, o0:o0 + NN])
                        nc.sync.dma_start(out=rsb[:, :NN], in_=rstd2_sp[:, o0:o0 + NN])
                        yn2t = [c1r.tile([96, RC * W], F32R, tag=f"cy{cg}", name=f"cy{cg}") for cg in range(2)]
                        for cg in range(2):
                            td = c1r.tile([96, RC * W], F32, tag=f"ctd{cg}")
                            nc.vector.tensor_tensor(out=td[:, :NN], in0=xpch[cg][:, :NN],
                                                    in1=mub[:96, :NN], op=Alu.subtract)
                            nc.vector.tensor_tensor(out=yn2t[cg][:, :NN], in0=td[:, :NN],
                                                    in1=rsb[:96, :NN], op=Alu.mult)
                        for mg in range(2):
                            fps = c1ps.tile([128, RC * W], F32, tag="fps")
                            for cg in range(2):
                                nc.tensor.matmul(
                                    fps[:, :NN],
                                    ws["fc1T"][:, (mg * 2 + cg) * 128:(mg * 2 + cg + 1) * 128],
                                    yn2t[cg][:, :NN], start=(cg == 0), stop=(cg == 1))
                            if mg == 0:
                                vg0 = c1r.tile([64, RC * W], F32R, tag="vg0")
                                nc.scalar.activation(vg0[:, :NN], fps[0:64, :NN], Act.Gelu,
                                                     bias=ws["fc1_bias"][0:64, 0:1])
                                ug0 = c1r.tile([64, RC * W], F32R, tag="ug0")
                                nc.scalar.activation(ug0[:, :NN], vg0[:, :NN], Act.Gelu,
                                                     bias=ws["g0_bias"], scale=ws["g0_scale"])
                                d0 = pd3(3 + c0) + 3
                                nc.sync.dma_start(
                                    out=vg0_sp[:, d0:d0 + nr_c * Wp3].rearrange(
                                        "p (r w) -> p r w", w=Wp3)[:, :, 0:W],
                                    in_=vg0[:, :NN].rearrange("p (r w) -> p r w", w=W))
                                nc.sync.dma_start(
                                    out=ug0_sp[:, d0:d0 + nr_c * Wp3].rearrange(
                                        "p (r w) -> p r w", w=Wp3)[:, :, 0:W],
                                    in_=ug0[:, :NN].rearrange("p (r w) -> p r w", w=W))
                                for r in range(nr_c):
                                    d0 = pd3(3 + c0 + r) + 3
                                    nc.scalar.activation(
                                        v0t1[0:64, d0:d0 + W],
                                        fps[64:128, r * W:(r + 1) * W], Act.Gelu,
                                        bias=ws["fc1_bias"][64:128, 0:1])
                            else:
                                for r in range(nr_c):
                                    d0 = pd3(3 + c0 + r) + 3
                                    nc.scalar.activation(
                                        v0t1[64:128, d0:d0 + W],
                                        fps[0:64, r * W:(r + 1) * W], Act.Gelu,
                                        bias=ws["fc1_bias"][0:64, 1:2])
                                    nc.scalar.activation(
                                        v0t2[0:64, d0:d0 + W],
                                        fps[64:128, r * W:(r + 1) * W], Act.Gelu,
                                        bias=ws["fc1_bias"][64:128, 1:2])
                                    nc.scalar.activation(
                                        v0t2[64:128, d0 + 1:d0 + 1 + W],
                                        fps[64:128, r * W:(r + 1) * W], Act.Gelu,
                                        bias=ws["fc1_bias"][64:128, 1:2])

                with (
                    tc.tile_pool(name="c2_rot", bufs=2) as c2r,
                    tc.tile_pool(name="c2_ps", bufs=2, space="PSUM") as c2ps,
                ):
                    for ci in range(NCH):
                        c0 = ci * RC
                        nr_c = min(RC, H - c0)
                        N = nr_c * Wp3
                        NN = nr_c * W
                        sb0 = pd3(3 + c0)
                        ps_a = c2ps.tile([128, RC * Wp3], F32, tag="psa")
                        for t in range(25):
                            dy, dx = t // 5 - 2, t % 5 - 2
                            o = sb0 + dy * Wp3 + dx
                            nc.tensor.matmul(ps_a[:, :N],
                                             ws["pair_diag"][:, t * 128:(t + 1) * 128],
                                             v0t1[:, o:o + N],
                                             start=(t == 0), stop=(t == 24))
                        ps_b = c2ps.tile([64, RC * Wp3], F32, tag="psb")
                        for i, (dy, dxa, hasb) in enumerate(dw3_passes):
                            o = sb0 + dy * Wp3 + dxa
                            nc.tensor.matmul(ps_b[:, :N],
                                             ws["dw3_diag"][:, i * 64:(i + 1) * 64],
                                             v0t2[:, o:o + N],
                                             start=(i == 0), stop=(i == len(dw3_passes) - 1))

                        def inner(ap_flat, lo, hi):
                            # interior view of a PSUM chunk (starts at free 0)
                            return ap_flat[lo:hi, :N].rearrange(
                                "p (r w) -> p r w", w=Wp3)[:, :, 3:3 + W]

                        def inner_v0(ap_flat, lo, hi):
                            # interior view of the padded v0 buffers at this chunk
                            return ap_flat[lo:hi, sb0:sb0 + N].rearrange(
                                "p (r w) -> p r w", w=Wp3)[:, :, 3:3 + W]

                        ug_a = c2r.tile([128, RC * W], F32R, tag="uga")
                        ug_b = c2r.tile([128, RC * W], F32R, tag="ugb")
                        vb_a = c2r.tile([128, RC * W], F32, tag="vba")
                        vb_b = c2r.tile([128, RC * W], F32, tag="vbb")
                        g0v = c2r.tile([64, RC * W], F32R, tag="g0v")
                        src3 = vg0_sp[:, sb0:sb0 + N].rearrange(
                            "p (r w) -> p r w", w=Wp3)[:, :, 3:3 + W]
                        nc.sync.dma_start(
                            out=g0v[:, :NN].rearrange("p (r w) -> p r w", w=W), in_=src3)
                        usrc3 = ug0_sp[:, sb0:sb0 + N].rearrange(
                            "p (r w) -> p r w", w=Wp3)[:, :, 3:3 + W]
                        nc.sync.dma_start(
                            out=ug_a[0:64, :NN].rearrange("p (r w) -> p r w", w=W), in_=usrc3)
                        nc.scalar.activation(
                            ug_a[64:128, :NN].rearrange("p (r w) -> p r w", w=W),
                            inner(ps_a, 0, 64), Act.Gelu, bias=ws["pair_bias"][0:64])
                        nc.scalar.activation(
                            ug_b[0:64, :NN].rearrange("p (r w) -> p r w", w=W),
                            inner(ps_a, 64, 128), Act.Gelu, bias=ws["pair_bias"][64:128])
                        nc.scalar.activation(
                            ug_b[64:128, :NN].rearrange("p (r w) -> p r w", w=W),
                            inner(ps_b, 0, 64), Act.Gelu, bias=ws["dw3_bias"])
                        nc.vector.tensor_scalar(out=vb_a[0:64, :NN], in0=g0v[:, :NN],
                                                scalar1=ws["s1a"][0:64],
                                                scalar2=ws["t1a"][0:64],
                                                op0=Alu.mult, op1=Alu.add)
                        nc.vector.tensor_scalar(out=vb_a[64:128, :NN],
                                                in0=inner_v0(v0t1, 0, 64),
                                                scalar1=ws["s1a"][64:128],
                                                scalar2=ws["t1a"][64:128],
                                                op0=Alu.mult, op1=Alu.add)
                        nc.vector.tensor_scalar(out=vb_b[0:64, :NN],
                                                in0=inner_v0(v0t1, 64, 128),
                                                scalar1=ws["s1b"][0:64],
                                                scalar2=ws["t1b"][0:64],
                                                op0=Alu.mult, op1=Alu.add)
                        nc.vector.tensor_scalar(out=vb_b[64:128, :NN],
                                                in0=inner_v0(v0t2, 0, 64),
                                                scalar1=ws["s1b"][64:128],
                                                scalar2=ws["t1b"][64:128],
                                                op0=Alu.mult, op1=Alu.add)
                        z1a = c2r.tile([128, RC * W], F32R, tag="z1a")
                        z1b = c2r.tile([128, RC * W], F32R, tag="z1b")
                        nc.vector.tensor_tensor(out=z1a[:, :NN], in0=ug_a[:, :NN],
                                                in1=vb_a[:, :NN], op=Alu.mult)
                        nc.vector.tensor_tensor(out=z1b[:, :NN], in0=ug_b[:, :NN],
                                                in1=vb_b[:, :NN], op=Alu.mult)
                        if dbg_sp is not None:
                            for nm, tl in (("uga", ug_a), ("ugb", ug_b), ("vba", vb_a),
                                           ("vbb", vb_b), ("z1a", z1a), ("z1b", z1b)):
                                nc.sync.dma_start(
                                    out=dbg_sp[nm][:, c0 * W:c0 * W + NN],
                                    in_=tl[:, :NN].bitcast(F32))
                        for cg in range(2):
                            ops = c2ps.tile([96, RC * W], F32, tag=f"ops{cg}")
                            nc.tensor.matmul(ops[:, :NN],
                                             ws["fc2aT"][:, (cg * 2) * 96:(cg * 2 + 1) * 96],
                                             z1a[:, :NN], start=True, stop=False)
                            nc.tensor.matmul(ops[:, :NN],
                                             ws["fc2aT"][:, (cg * 2 + 1) * 96:(cg * 2 + 2) * 96],
                                             z1b[:, :NN], start=False, stop=False)
                            nc.tensor.matmul(ops[:, :NN],
                                             ws["fc2bT_g0"][:, cg * 96:(cg + 1) * 96],
                                             g0v[:, :NN], start=False, stop=False)
                            opsv = ops[:, :NN].rearrange("p (r w) -> p r w", w=W)
                            nc.tensor.matmul(opsv,
                                             ws["fc2bT_g12"][:, cg * 96:(cg + 1) * 96],
                                             inner_v0(v0t1, 0, 128), start=False, stop=False)
                            nc.tensor.matmul(opsv,
                                             ws["fc2bT_g3"][:, cg * 96:(cg + 1) * 96],
                                             inner_v0(v0t2, 0, 64), start=False, stop=True)
                            xrch = c2r.tile([96, RC * W], F32R, tag=f"xr{cg}", bufs=1)
                            nc.sync.dma_start(out=xrch[:, :NN],
                                              in_=xcp_sp[cg][:, c0 * W:c0 * W + NN])
                            ob = c2r.tile([96, RC * W], F32, tag=f"ob{cg}", bufs=1)
                            nc.vector.tensor_scalar(out=ob[:, :NN], in0=ops[:, :NN],
                                                    scalar1=ws["s3v"][:, cg:cg + 1],
                                                    scalar2=ws["out_bias"][:, cg:cg + 1],
                                                    op0=Alu.mult, op1=Alu.add)
                            oc = c2r.tile([96, RC * W], F32, tag=f"oc{cg}", bufs=1)
                            nc.vector.tensor_tensor(out=oc[:, :NN], in0=ob[:, :NN],
                                                    in1=xrch[:, :NN], op=Alu.add)
                            nc.sync.dma_start(
                                out=out_t[cg * 96:(cg + 1) * 96, c0 * W:c0 * W + NN],
                                in_=oc[:, :NN])
            _wpc_cm.__exit__(None, None, None)
    return out_t.name


# ----------------------------------------------------------------------------
# host entry
# ----------------------------------------------------------------------------

_CACHE = {}


def make_program(H, W, n_cores, attn_scale, dw3_passes):
    key = (H, W, n_cores, round(attn_scale, 9))
    if key in _CACHE:
        return _CACHE[key]
    nc = bacc.Bacc("TRN2", target_bir_lowering=False, debug=False, num_devices=n_cores)
    out_name = build(nc, H, W, n_cores, attn_scale, dw3_passes)
    nc.compile()
    _CACHE[key] = (nc, out_name)
    return nc, out_name


def make_in_maps(inputs):
    x = np.asarray(inputs["x"], np.float32)
    B = x.shape[0]
    wdict = _prep_weights({k: np.asarray(v) for k, v in inputs.items()})
    base = {}
    for k, (shp, d) in WSPEC.items():
        base["w_" + k] = wdict[k][0].reshape(shp)
    in_maps = []
    for b in range(B):
        m = dict(base)
        m["x"] = np.ascontiguousarray(x[b])
        in_maps.append(m)
    return in_maps, wdict


def kernel(**inputs):
    x = np.asarray(inputs["x"], np.float32)
    B, H, W, C = x.shape
    in_maps, wdict = make_in_maps(inputs)
    nc, out_name = make_program(H, W, B, wdict["_attn_scale"][0],
                                wdict["_dw3_passes"][0])
    res = bass_utils.run_bass_kernel_spmd(nc, in_maps, core_ids=list(range(B)))
    return np.stack([
        np.asarray(res.results[b][out_name]).reshape(C, H * W).T.reshape(H, W, C)
        for b in range(B)])



# revision 31
# speedup vs baseline: 7.6009x; 1.5570x over previous
"""Trainium2 Bass kernel for nn_Block_87351044866235 (sparse_attention).

Data-parallel over batch: 8 samples -> 8 NeuronCores. Channel-major
layout [C, H*W] on chip; depthwise convs as diagonal fp32r matmuls on
TensorE; 1x1 convs as fp32r matmuls; LN stats via ones-matmuls; q/k gram
via hi/lo bf16 split + DMA-xbar transposes; dynamic-k gate mean via a
scalar AllReduce.
"""
import sys, os

for _p in ("/opt/trn_rl_repo", "/root/.axon_site/_ro/trn_rl_repo"):
    if os.path.isdir(_p) and _p not in sys.path:
        sys.path.append(_p)

import numpy as np
import ml_dtypes
import concourse.bass as bass
import concourse.bacc as bacc
import concourse.tile as tile
from concourse import mybir
from concourse import bass_utils

try:
    from concourse import tile_utils as _tu
    _tu.max_sbuf_usage = 208 * 1024
except Exception:
    pass

dt = mybir.dt
Alu = mybir.AluOpType
Act = mybir.ActivationFunctionType
AX = mybir.AxisListType.X

EMBED, PDIM, HEADS, HID = 192, 96, 8, 256
CPH = PDIM // HEADS  # 12
SLOP = 8
RC = 3    # conv output rows per chunk
BR = 12   # rows per band

F32, F32R, BF16 = dt.float32, dt.float32r, dt.bfloat16


def _ceil(a, b):
    return (a + b - 1) // b


# ----------------------------------------------------------------------------
# host-side weight prep: everything 2D [partitions, free]
# ----------------------------------------------------------------------------

def _prep_weights(p):
    w = {}
    f32r = lambda a: (np.ascontiguousarray(a, np.float32), F32R)
    f32 = lambda a: (np.ascontiguousarray(a, np.float32), F32)
    eps_bn = 1e-5

    w["ident"] = f32(np.eye(128, dtype=np.float32))
    w["identb"] = (np.eye(128, dtype=np.float32).astype(ml_dtypes.bfloat16), BF16)

    # pos depthwise diag: [96, (t*2+cg)*96]
    pw = p["pos_w"][:, 0]  # [192,3,3]
    pos_d = np.zeros((96, 18 * 96), np.float32)
    for t in range(9):
        dy, dx = t // 3 - 1, t % 3 - 1
        for cg in range(2):
            pos_d[:, (t * 2 + cg) * 96:(t * 2 + cg + 1) * 96] = \
                np.diag(pw[cg * 96:(cg + 1) * 96, dy + 1, dx + 1])
    w["pos_diag"] = f32r(pos_d)
    w["pos_b"] = f32(p["pos_b"].reshape(2, 96).T)  # [96, 2]

    g1v, b1v = p["ln1_g"], p["ln1_b"]
    qw = p["qkv_w"][:, :, 0, 0]  # [288, 96]
    qw_eff = qw * g1v[None, :96]
    w["qkv_wT"] = f32r(np.concatenate(
        [qw_eff[j * 96:(j + 1) * 96].T for j in range(3)], axis=1))  # [96, 3*96]
    w["qkv_bias"] = f32((qw @ b1v[:96]).reshape(3, 96).T)  # [96, 3]

    qdw = p["qkv_dw_w"][:, 0]  # [288,3,3]
    qdw_d = np.zeros((96, 27 * 96), np.float32)
    for t in range(9):
        dy, dx = t // 3 - 1, t % 3 - 1
        for j in range(3):
            qdw_d[:, (t * 3 + j) * 96:(t * 3 + j + 1) * 96] = \
                np.diag(qdw[j * 96:(j + 1) * 96, dy + 1, dx + 1])
    w["qdw_diag"] = f32r(qdw_d)

    gw1 = p["gate_w1"][:, :, 0, 0]  # [96, 192]
    gw1_eff = gw1 * g1v[None, :]
    w["gate_w1T"] = f32r(np.concatenate(
        [gw1_eff[:, cg * 96:(cg + 1) * 96].T for cg in range(2)], axis=1))  # [96, 192]
    w["gate_b1"] = f32((p["gate_b1"] + gw1 @ b1v).reshape(96, 1))
    w["gate_w2T"] = f32r(p["gate_w2"][:, :, 0, 0].T.copy())  # [96,1]
    w["gate_b2"] = f32(p["gate_b2"].reshape(1, 1))

    pj = p["proj_w"][:, :, 0, 0]
    pj1, pj2 = pj[:, :96], pj[:, 96:] * g1v[None, 96:]
    w["proj1T"] = f32r(np.concatenate(
        [pj1[cg * 96:(cg + 1) * 96].T for cg in range(2)], axis=1))  # [96, 192]
    w["proj2T"] = f32r(np.concatenate(
        [pj2[cg * 96:(cg + 1) * 96].T for cg in range(2)], axis=1))
    w["proj_bias"] = f32((pj[:, 96:] @ b1v[96:]).reshape(2, 96).T)  # [96, 2]

    attn_scale = float(p["attn1"][0] + p["attn2"][0] + p["attn3"][0] + p["attn4"][0])
    w["_attn_scale"] = (attn_scale, None)
    w["tempvec"] = f32(np.repeat(p["temperature"].reshape(HEADS), CPH).reshape(96, 1))

    g2v, b2v = p["ln2_g"], p["ln2_b"]
    f1 = p["fc1_w"][:, :, 0, 0]  # [256, 192]
    f1_eff = f1 * g2v[None, :]
    fc1 = np.zeros((96, 4 * 128), np.float32)
    for mg in range(2):
        for cg in range(2):
            fc1[:, (mg * 2 + cg) * 128:(mg * 2 + cg + 1) * 128] = \
                f1_eff[mg * 128:(mg + 1) * 128, cg * 96:(cg + 1) * 96].T
    w["fc1T"] = f32r(fc1)
    w["fc1_bias"] = f32((f1 @ b2v).reshape(2, 128).T)  # [128, 2]

    s1 = p["bn1_g"] / np.sqrt(p["bn1_v"] + eps_bn)
    t1 = p["bn1_b"] - p["bn1_m"] * s1
    s2 = p["bn2_g"] / np.sqrt(p["bn2_v"] + eps_bn)
    t2 = p["bn2_b"] - p["bn2_m"] * s2
    s3 = p["bn3_g"] / np.sqrt(p["bn3_v"] + eps_bn)
    t3 = p["bn3_b"] - p["bn3_m"] * s3

    dw1w, dw2w, dw3w = p["dw1_w"][:, 0], p["dw2_w"][:, 0], p["dw3_w"][:, 0]
    dw1b, dw2b, dw3b = p["dw1_b"], p["dw2_b"], p["dw3_b"]
    s1g = [s1[i * 64:(i + 1) * 64] for i in range(4)]
    t1g = [t1[i * 64:(i + 1) * 64] for i in range(4)]

    pair_d = np.zeros((128, 25 * 128), np.float32)
    for t in range(25):
        dy, dx = t // 5 - 2, t % 5 - 2
        blk = np.zeros((128, 128), np.float32)
        d2 = dw2w[:, dy + 2, dx + 2] * s1g[2]
        if dy == 0 and dx == 0:
            d2 = d2 + s1g[2]
        blk[64:, 64:] = np.diag(d2)
        if -1 <= dy <= 1 and -1 <= dx <= 1:
            d1 = dw1w[:, dy + 1, dx + 1] * s1g[1]
            if dy == 0 and dx == 0:
                d1 = d1 + s1g[1]
            blk[:64, :64] = np.diag(d1)
        pair_d[:, t * 128:(t + 1) * 128] = blk
    w["pair_diag"] = (pair_d.astype(ml_dtypes.bfloat16), BF16)
    bc1 = t1g[1] * dw1w.sum((1, 2)) + dw1b + t1g[1]
    bc2 = t1g[2] * dw2w.sum((1, 2)) + dw2b + t1g[2]
    w["pair_bias"] = f32(np.concatenate([bc1, bc2]).reshape(128, 1))

    # rows 64:128 of v0t2 hold the same data stored shifted +1, so a read at
    # AP offset (dy, dxa) yields tap (dy, dxa-1) for those rows.
    dw3_passes = []
    for dy in range(-3, 4):
        for dxa in (-2, 0, 2):
            dw3_passes.append((dy, dxa, True))
        dw3_passes.append((dy, 3, False))
    dw3_d = np.zeros((128, len(dw3_passes) * 64), np.float32)
    for i, (dy, dxa, hasb) in enumerate(dw3_passes):
        wa = dw3w[:, dy + 3, dxa + 3] * s1g[3]
        if dy == 0 and dxa == 0:
            wa = wa + s1g[3]
        dw3_d[:64, i * 64:(i + 1) * 64] = np.diag(wa)
        if hasb:
            wb = dw3w[:, dy + 3, dxa - 1 + 3] * s1g[3]
            if dy == 0 and dxa - 1 == 0:
                wb = wb + s1g[3]
            dw3_d[64:, i * 64:(i + 1) * 64] = np.diag(wb)
    w["dw3_diag"] = (dw3_d.astype(ml_dtypes.bfloat16), BF16)
    w["_dw3_passes"] = (dw3_passes, None)
    w["dw3_bias"] = f32((t1g[3] * dw3w.sum((1, 2)) + dw3b + t1g[3]).reshape(64, 1))

    d0w, d0b = p["dw0_w"][:, 0, 0, 0], p["dw0_b"]
    w["g0_scale"] = f32(((d0w + 1.0) * s1g[0]).reshape(64, 1))
    w["g0_bias"] = f32(((d0w + 1.0) * t1g[0] + d0b).reshape(64, 1))

    f2 = p["fc2_w"][:, :, 0, 0]  # [192, 256]
    # z = (s2*gelu(u)+t2) * v0bn is computed explicitly on DVE, so fc2 is plain
    fc2a = np.zeros((128, 4 * 96), np.float32)
    for cg in range(2):
        for kg in range(2):
            fc2a[:, (cg * 2 + kg) * 96:(cg * 2 + kg + 1) * 96] = \
                f2[cg * 96:(cg + 1) * 96, kg * 128:(kg + 1) * 128].T
    w["fc2aT"] = f32r(fc2a)
    w["s3v"] = f32(np.stack([s3[:96], s3[96:]], axis=1))          # [96, 2]
    w["out_bias"] = f32(np.stack([t3[:96], t3[96:]], axis=1))     # [96, 2]
    w["s2a"] = f32(s2[:128].reshape(128, 1))
    w["s2b"] = f32(s2[128:].reshape(128, 1))
    w["t2a"] = f32(t2[:128].reshape(128, 1))
    w["t2b"] = f32(t2[128:].reshape(128, 1))

    sg = np.where(s1 == 0, 1.0, s1)
    padv = -t1 / sg
    w["padv1"] = f32(np.concatenate([padv[64:128], padv[128:192]]).reshape(128, 1))
    w["padv2"] = f32(np.concatenate([padv[192:256], padv[192:256]]).reshape(128, 1))
    w["s1a"] = f32(s1[:128].reshape(128, 1))
    w["s1b"] = f32(s1[128:].reshape(128, 1))
    w["t1a"] = f32(t1[:128].reshape(128, 1))
    w["t1b"] = f32(t1[128:].reshape(128, 1))

    w["ones_st"] = f32r(np.full((96, 128), 1.0 / EMBED, np.float32))
    w["epsv"] = f32(np.full((128, 1), 1e-6, np.float32))
    vm = np.zeros((96, 96), np.float32)
    for h in range(HEADS):
        vm[h * CPH:(h + 1) * CPH, h * CPH:(h + 1) * CPH] = 1.0
    w["vmask"] = f32(vm)
    return w


WSPEC = {
    "ident": ([128, 128], F32), "identb": ([128, 128], BF16),
    "pos_diag": ([96, 18 * 96], F32R),
    "pos_b": ([96, 2], F32), "qkv_wT": ([96, 3 * 96], F32R),
    "qkv_bias": ([96, 3], F32), "qdw_diag": ([96, 27 * 96], F32R),
    "gate_w1T": ([96, 192], F32R), "gate_b1": ([96, 1], F32),
    "gate_w2T": ([96, 1], F32R), "gate_b2": ([1, 1], F32),
    "proj1T": ([96, 192], F32R), "proj2T": ([96, 192], F32R),
    "proj_bias": ([96, 2], F32), "tempvec": ([96, 1], F32),
    "fc1T": ([96, 4 * 128], F32R), "fc1_bias": ([128, 2], F32),
    "pair_diag": ([128, 25 * 128], BF16), "pair_bias": ([128, 1], F32),
    "dw3_diag": ([128, 28 * 64], BF16), "dw3_bias": ([64, 1], F32),
    "g0_scale": ([64, 1], F32), "g0_bias": ([64, 1], F32),
    "fc2aT": ([128, 4 * 96], F32R),
    "s3v": ([96, 2], F32), "out_bias": ([96, 2], F32),
    "s2a": ([128, 1], F32), "s2b": ([128, 1], F32),
    "t2a": ([128, 1], F32), "t2b": ([128, 1], F32),
    "padv1": ([128, 1], F32),
    "padv2": ([128, 1], F32),
    "s1a": ([128, 1], F32), "s1b": ([128, 1], F32),
    "t1a": ([128, 1], F32), "t1b": ([128, 1], F32),
    "ones_st": ([96, 128], F32R),
    "epsv": ([128, 1], F32),
    "vmask": ([96, 96], F32),
}


# ----------------------------------------------------------------------------
# device kernel
# ----------------------------------------------------------------------------

def build(nc, H, W, n_cores, attn_scale, dw3_passes):
    S = H * W
    Wp1 = W + 2
    P1B = (BR + 2) * Wp1 + 2 * SLOP   # band buffer (pad1)
    Wp3, Hp3 = W + 6, H + 6
    P3 = Hp3 * Wp3 + 2 * SLOP
    NCH = _ceil(H, RC)
    NB = _ceil(H, BR)
    NSC = _ceil(S, 512)
    GCH = 512 // W                    # gate chunk rows (512 cols)
    NGC_PER_BAND = _ceil(BR, GCH)

    x_t = nc.dram_tensor("x", [H, W, EMBED], F32, kind="ExternalInput")
    # channel-major output: [192, S]; host does the final (H,W,C) transpose
    out_t = nc.dram_tensor("out", [EMBED, S], F32, kind="ExternalOutput")
    wt = {k: nc.dram_tensor("w_" + k, shp, d, kind="ExternalInput")
          for k, (shp, d) in WSPEC.items()}

    def pd3(r):
        return SLOP + r * Wp3

    with tile.TileContext(nc) as tc:
        PERS_W = ['ident', 'identb', 'ones_st', 'epsv', 'vmask', 'tempvec']
        C_ONLY_W = ['fc1T', 'fc1_bias', 'pair_diag', 'pair_bias', 'dw3_diag',
                    'dw3_bias', 'g0_scale', 'g0_bias', 'fc2aT', 's3v', 'out_bias',
                    's1a', 's1b', 't1a', 't1b', 's2a', 's2b', 't2a', 't2b',
                    'padv1', 'padv2', 'proj2T', 'proj_bias']
        with (
            tc.tile_pool(name="dram", bufs=1, space="DRAM") as dram,
            tc.tile_pool(name="persist", bufs=1) as pers,
        ):
            ws = {}

            def _load_w(pool, names):
                for k in names:
                    shp, d = WSPEC[k]
                    tl = pool.tile(shp, d, tag="w_" + k, name="w_" + k)
                    nc.sync.dma_start(out=tl[:], in_=wt[k][:])
                    ws[k] = tl


            yn1_sp = dram.tile([96, S], F32R)
            yn2_sp = dram.tile([96, S], F32R)
            xc_sp = [dram.tile([96, S], F32R, name=f"xc_sp{i}") for i in range(2)]
            v_sp = dram.tile([96, S], F32R)
            xcp_sp = [dram.tile([96, S], F32R, name=f"xcp_sp{i}") for i in range(2)]
            vg0_sp = dram.tile([64, P3], F32R)
            ug0_sp = dram.tile([64, P3], F32R)
            cc_in = dram.tile([1, 1], F32)
            cc_out = dram.tile([1, 1], F32)

            gsum = pers.tile([1, NB * NGC_PER_BAND + 8], F32)
            nc.vector.memset(gsum[:], 0.0)
            dynk = pers.tile([96, 1], F32)
            ppt = pers.tile([96, 192], F32R)   # (proj1 @ (attn_scale*P))^T

            # ================= PHASE A =================
            _load_w(pers, PERS_W)
            _wpab_cm = tc.tile_pool(name="wpAB", bufs=1)
            wpab = _wpab_cm.__enter__()
            _load_w(wpab, [k for k in WSPEC
                           if k not in C_ONLY_W and k not in PERS_W])
            ident = ws["ident"]
            with (
                tc.tile_pool(name="pa_band", bufs=2) as pab,
                tc.tile_pool(name="pa_rot", bufs=3) as par,
                tc.tile_pool(name="pa_ps", bufs=2, space="PSUM") as paps,
                tc.tile_pool(name="pa_ps2", bufs=2, space="PSUM") as paps2,
            ):
                for b in range(NB):
                    r0, r1 = b * BR, min((b + 1) * BR, H)
                    ylo, yhi = max(r0 - 1, 0), min(r1 + 1, H)
                    nrb = yhi - ylo
                    xband = [pab.tile([96, P1B], F32R, tag=f"xb{cg}", name=f"xb{cg}") for cg in range(2)]
                    for cg in range(2):
                        nc.vector.memset(xband[cg][:].bitcast(F32), 0.0)
                    xraw = pab.tile([W, (BR + 2) * EMBED], F32, tag="xraw")
                    nc.sync.dma_start(
                        out=xraw[:, :nrb * EMBED].rearrange("w (r c) -> w r c", c=EMBED),
                        in_=x_t[ylo:yhi].rearrange("r w c -> w r c"))
                    for rr in range(ylo, yhi):
                        boff = SLOP + (rr - (r0 - 1)) * Wp1 + 1
                        for cg in range(2):
                            tps = paps2.tile([96, W], F32, tag="tps")
                            nc.tensor.transpose(
                                tps[:],
                                xraw[:, (rr - ylo) * EMBED + cg * 96:
                                     (rr - ylo) * EMBED + (cg + 1) * 96],
                                ident[:W, :W])
                            nc.scalar.copy(xband[cg][:, boff:boff + W], tps[:])
                    xcband = [pab.tile([96, BR * W], F32R, tag=f"xcb{cg}", name=f"xcb{cg}")
                              for cg in range(2)]
                    ynband = [pab.tile([96, BR * W], F32R, tag=f"ynb{cg}", name=f"ynb{cg}")
                              for cg in range(2)]
                    for c0 in range(r0, r1, RC):
                        nr_c = min(RC, H - c0)
                        N = nr_c * Wp1
                        NN = nr_c * W
                        bo = (c0 - r0) * W
                        sb0 = SLOP + (c0 - r0 + 1) * Wp1
                        xc_ch = [xcband[cg][:, bo:bo + NN] for cg in range(2)]
                        xsq = [par.tile([96, RC * W], F32R, tag=f"xq{cg}", name=f"xq{cg}") for cg in range(2)]
                        for cg in range(2):
                            ps = paps.tile([96, RC * Wp1], F32, tag="posps")
                            for t in range(9):
                                dy, dx = t // 3 - 1, t % 3 - 1
                                o = sb0 + dy * Wp1 + dx
                                nc.tensor.matmul(
                                    ps[:, :N],
                                    ws["pos_diag"][:, (t * 2 + cg) * 96:(t * 2 + cg + 1) * 96],
                                    xband[cg][:, o:o + N],
                                    start=(t == 0), stop=(t == 8))
                            ps_int = ps[:, :N].rearrange("p (r w) -> p r w", w=Wp1)[:, :, 1:1 + W]
                            xb_int = xband[cg][:, sb0:sb0 + N] \
                                .rearrange("p (r w) -> p r w", w=Wp1)[:, :, 1:1 + W]
                            xcv = xc_ch[cg].rearrange("p (r w) -> p r w", w=W)
                            nc.vector.scalar_tensor_tensor(
                                out=xcv, in0=ps_int, scalar=ws["pos_b"][:, cg:cg + 1],
                                in1=xb_int, op0=Alu.add, op1=Alu.add)
                            nc.scalar.square(xsq[cg][:, :NN], xc_ch[cg])
                        mu_ps = paps.tile([128, RC * W], F32, tag="mups")
                        m2_ps = paps.tile([128, RC * W], F32, tag="m2ps")
                        for cg in range(2):
                            nc.tensor.matmul(mu_ps[:, :NN], ws["ones_st"], xc_ch[cg],
                                             start=(cg == 0), stop=(cg == 1))
                            nc.tensor.matmul(m2_ps[:, :NN], ws["ones_st"], xsq[cg][:, :NN],
                                             start=(cg == 0), stop=(cg == 1))
                        musq = par.tile([128, RC * W], F32, tag="musq")
                        nc.scalar.square(musq[:, :NN], mu_ps[:, :NN])
                        var = par.tile([128, RC * W], F32, tag="var")
                        nc.vector.tensor_tensor(out=var[:, :NN], in0=m2_ps[:, :NN],
                                                in1=musq[:, :NN], op=Alu.subtract)
                        sd = par.tile([128, RC * W], F32, tag="sd")
                        nc.scalar.activation(sd[:, :NN], var[:, :NN], Act.Sqrt, bias=ws["epsv"])
                        rstd = par.tile([128, RC * W], F32, tag="rstd")
                        nc.vector.reciprocal_approx_fast(rstd[:, :NN], sd[:, :NN])
                        for cg in range(2):
                            tdf = par.tile([96, RC * W], F32, tag=f"td{cg}")
                            nc.vector.tensor_tensor(out=tdf[:, :NN], in0=xc_ch[cg],
                                                    in1=mu_ps[:96, :NN], op=Alu.subtract)
                            nc.vector.tensor_tensor(out=ynband[cg][:, bo:bo + NN],
                                                    in0=tdf[:, :NN],
                                                    in1=rstd[:96, :NN], op=Alu.mult)
                    NBW = (r1 - r0) * W
                    for cg in range(2):
                        sp = yn1_sp if cg == 0 else yn2_sp
                        nc.sync.dma_start(out=sp[:, r0 * W:r0 * W + NBW],
                                          in_=ynband[cg][:, :NBW])
                        nc.sync.dma_start(out=xc_sp[cg][:, r0 * W:r0 * W + NBW],
                                          in_=xcband[cg][:, :NBW])

            # ================= PHASE B =================
            with (
                tc.tile_pool(name="pb_band", bufs=1) as pbb,
                tc.tile_pool(name="pb_rot", bufs=3) as pbr,
                tc.tile_pool(name="gram_ps", bufs=1, space="PSUM") as gpsp,
            ):
                g1_ps = gpsp.tile([96, 384], F32)
                g2_ps = gpsp.tile([96, 288], F32)
                with (
                    tc.tile_pool(name="pb_psg", bufs=1, space="PSUM") as pbpsg,
                    tc.tile_pool(name="pb_ps", bufs=2, space="PSUM") as pbps,
                    tc.tile_pool(name="pb_tps", bufs=2, space="PSUM") as tpsp,
                ):
                    for b in range(NB):
                        r0, r1 = b * BR, min((b + 1) * BR, H)
                        ylo, yhi = max(r0 - 1, 0), min(r1 + 1, H)
                        ynb = [pbb.tile([96, (BR + 2) * W], F32R, tag=f"ynb{cg}", name=f"ynb{cg}")
                               for cg in range(2)]
                        for cg in range(2):
                            sp = yn1_sp if cg == 0 else yn2_sp
                            nc.sync.dma_start(
                                out=ynb[cg][:, (ylo - r0 + 1) * W:(yhi - r0 + 1) * W],
                                in_=sp[:, ylo * W:yhi * W])
                        # gate (512-col chunks over rows [r0, r1))
                        for gi in range(NGC_PER_BAND):
                            gr0 = r0 + gi * GCH
                            if gr0 >= r1:
                                break
                            ngr = min(GCH, r1 - gr0)
                            NG = ngr * W
                            yo = (gr0 - r0 + 1) * W
                            gps = pbpsg.tile([96, 512], F32, tag="gps")
                            for cg in range(2):
                                nc.tensor.matmul(gps[:, :NG],
                                                 ws["gate_w1T"][:, cg * 96:(cg + 1) * 96],
                                                 ynb[cg][:, yo:yo + NG],
                                                 start=(cg == 0), stop=(cg == 1))
                            g1s = pbr.tile([96, 512], F32R, tag="g1s")
                            nc.scalar.activation(g1s[:, :NG], gps[:, :NG], Act.Relu,
                                                 bias=ws["gate_b1"])
                            g2ps = pbpsg.tile([1, 512], F32, tag="g2ps")
                            nc.tensor.matmul(g2ps[:, :NG], ws["gate_w2T"], g1s[:, :NG],
                                             start=True, stop=True)
                            sgt = pbr.tile([1, 512], F32, tag="sgt")
                            idx = b * NGC_PER_BAND + gi
                            nc.scalar.activation(sgt[:, :NG], g2ps[:, :NG], Act.Sigmoid,
                                                 bias=ws["gate_b2"],
                                                 accum_out=gsum[0:1, idx:idx + 1])
                        # qkv0 band
                        qkv0 = [pbb.tile([96, P1B], F32R, tag=f"qk0{j}", name=f"qk0{j}") for j in range(3)]
                        for j in range(3):
                            nc.vector.memset(qkv0[j][:].bitcast(F32), 0.0)
                        for rr in range(ylo, yhi, 2):
                            nrw = min(2, yhi - rr)
                            NQ = nrw * W
                            for j in range(3):
                                qps = pbps.tile([96, RC * Wp1], F32, tag="ps")
                                nc.tensor.matmul(qps[:, :NQ],
                                                 ws["qkv_wT"][:, j * 96:(j + 1) * 96],
                                                 ynb[0][:, (rr - r0 + 1) * W:(rr - r0 + 1) * W + NQ],
                                                 start=True, stop=True)
                                dst = SLOP + (rr - r0 + 1) * Wp1 + 1
                                dview = qkv0[j][:, dst:dst + nrw * Wp1] \
                                    .rearrange("p (r w) -> p r w", w=Wp1)[:, :, 0:W]
                                nc.scalar.activation(
                                    dview, qps[:, :NQ].rearrange("p (r w) -> p r w", w=W),
                                    Act.Identity, bias=ws["qkv_bias"][:, j:j + 1])
                        # depthwise + hi/lo + transpose staging
                        qkband = pbr.tile([W, BR * 384], BF16, tag="qkband")
                        for c0 in range(r0, r1, RC):
                            nr_c = min(RC, H - c0)
                            N = nr_c * Wp1
                            NN = nr_c * W
                            sb0 = SLOP + (c0 - r0 + 1) * Wp1
                            hilo = {}
                            for j in range(3):
                                ps = pbps.tile([96, RC * Wp1], F32, tag="ps")
                                for t in range(9):
                                    dy, dx = t // 3 - 1, t % 3 - 1
                                    o = sb0 + dy * Wp1 + dx
                                    nc.tensor.matmul(
                                        ps[:, :N],
                                        ws["qdw_diag"][:, (t * 3 + j) * 96:(t * 3 + j + 1) * 96],
                                        qkv0[j][:, o:o + N],
                                        start=(t == 0), stop=(t == 8))
                                ps_int = ps[:, :N].rearrange("p (r w) -> p r w", w=Wp1)[:, :, 1:1 + W]
                                if j == 2:
                                    vch = pbr.tile([96, RC * W], F32R, tag="vch")
                                    nc.scalar.copy(
                                        vch[:, :NN].rearrange("p (r w) -> p r w", w=W), ps_int)
                                    nc.sync.dma_start(out=v_sp[:, c0 * W:c0 * W + NN],
                                                      in_=vch[:, :NN])
                                else:
                                    hi = pbr.tile([96, RC * W], BF16, tag=f"hi{j}")
                                    lo = pbr.tile([96, RC * W], BF16, tag=f"lo{j}")
                                    hiv = hi[:, :NN].rearrange("p (r w) -> p r w", w=W)
                                    nc.scalar.copy(hiv, ps_int)
                                    nc.vector.tensor_tensor(
                                        out=lo[:, :NN].rearrange("p (r w) -> p r w", w=W),
                                        in0=ps_int, in1=hiv, op=Alu.subtract)
                                    hilo[j] = (hi, lo)
                            for rr in range(c0, c0 + nr_c):
                                ro = (rr - r0) * 384
                                rl = (rr - c0) * W
                                tq = tpsp.tile([128, 384], BF16, tag="tq")
                                for idx, src in enumerate((hilo[0][0], hilo[1][0],
                                                           hilo[1][1], hilo[0][1])):
                                    nc.tensor.transpose(
                                        tq[:, idx * 96:(idx + 1) * 96],
                                        src[:, rl:rl + W], ws["identb"][:96, :96])
                                nc.scalar.copy(qkband[:, ro:ro + 384], tq[:])
                        for rr in range(r0, r1):
                            ro = (rr - r0) * 384
                            nc.tensor.matmul(g1_ps[:], qkband[:, ro:ro + 96],
                                             qkband[:, ro:ro + 384],
                                             start=(rr == 0), stop=(rr == H - 1))
                            nc.tensor.matmul(g2_ps[:], qkband[:, ro + 96:ro + 192],
                                             qkband[:, ro + 96:ro + 384],
                                             start=(rr == 0), stop=(rr == H - 1))

                # ---- gate mean -> AllReduce -> dynk ----
                gred = pers.tile([1, 1], F32)
                nc.vector.reduce_sum(gred[:], gsum[0:1, 0:NB * NGC_PER_BAND], axis=AX)
                gsc = pers.tile([1, 1], F32)
                nc.vector.tensor_scalar_mul(gsc[:], gred[:], float(CPH) / (n_cores * S))
                nc.sync.dma_start(out=cc_in[:], in_=gsc[:])
                nc.gpsimd.collective_compute(
                    "AllReduce", Alu.add, replica_groups=[list(range(n_cores))],
                    ins=[cc_in.opt()], outs=[cc_out.opt()])
                nc.sync.dma_start(out=dynk[:], in_=cc_out[:].partition_broadcast(96))

                # ---- attn block ----
                with (
                    tc.tile_pool(name="at_ps", bufs=2, space="PSUM") as atps,
                    tc.tile_pool(name="at_sb", bufs=1) as ab,
                ):
                    g1sb = ab.tile([96, 384], F32)
                    nc.scalar.copy(g1sb[:], g1_ps[:])
                    g2sb = ab.tile([96, 288], F32)
                    nc.scalar.copy(g2sb[:], g2_ps[:])
                    lohi_ps = atps.tile([96, 96], F32, tag="atp")
                    nc.tensor.transpose(lohi_ps[:], g2sb[:, 192:288], ident[:96, :96])
                    gq = ab.tile([96, 96], F32)
                    nc.vector.tensor_tensor(out=gq[:], in0=g1sb[:, 96:192],
                                            in1=g1sb[:, 192:288], op=Alu.add)
                    gqk = ab.tile([96, 96], F32)
                    nc.vector.tensor_tensor(out=gqk[:], in0=gq[:], in1=lohi_ps[:], op=Alu.add)
                    idm = ident[:96, :96]
                    tq = ab.tile([96, 96], F32)
                    nc.vector.tensor_tensor(out=tq[:], in0=g1sb[:, 0:96], in1=idm, op=Alu.mult)
                    nq2 = ab.tile([96, 1], F32)
                    nc.vector.reduce_sum(nq2[:], tq[:], axis=AX)
                    ksm = ab.tile([96, 96], F32)
                    nc.vector.scalar_tensor_tensor(out=ksm[:], in0=g2sb[:, 96:192], scalar=2.0,
                                                   in1=g2sb[:, 0:96], op0=Alu.mult, op1=Alu.add)
                    tk = ab.tile([96, 96], F32)
                    nc.vector.tensor_tensor(out=tk[:], in0=ksm[:], in1=idm, op=Alu.mult)
                    nk2 = ab.tile([96, 1], F32)
                    nc.vector.reduce_sum(nk2[:], tk[:], axis=AX)

                    def rsqrt_clamped(nm, src):
                        sq = ab.tile([96, 1], F32, tag=nm + "sq")
                        nc.scalar.sqrt(sq[:], src[:])
                        cl = ab.tile([96, 1], F32, tag=nm + "cl")
                        nc.vector.tensor_scalar_max(cl[:], sq[:], 1e-12)
                        rvv = ab.tile([96, 1], F32, tag=nm)
                        nc.vector.reciprocal(rvv[:], cl[:])
                        return rvv

                    rq = rsqrt_clamped("rq", nq2)
                    rk = rsqrt_clamped("rk", nk2)
                    rqt = ab.tile([96, 1], F32)
                    nc.vector.tensor_tensor(out=rqt[:], in0=rq[:], in1=ws["tempvec"][:],
                                            op=Alu.mult)
                    asr = ab.tile([96, 96], F32)
                    nc.vector.tensor_scalar_mul(asr[:], gqk[:], rqt[:])
                    as_ps = atps.tile([96, 96], F32, tag="atp")
                    nc.tensor.transpose(as_ps[:], asr[:], ident[:96, :96])
                    ast = ab.tile([96, 96], F32)
                    nc.vector.tensor_scalar_mul(ast[:], as_ps[:], rk[:])
                    as2_ps = atps.tile([96, 96], F32, tag="atp")
                    nc.tensor.transpose(as2_ps[:], ast[:], ident[:96, :96])
                    as2 = ab.tile([96, 96], F32)
                    nc.scalar.copy(as2[:], as2_ps[:])
                    # mask off-head-block entries to -60
                    t60 = ab.tile([96, 96], F32)
                    nc.vector.tensor_scalar_add(t60[:], as2[:], 60.0)
                    amf = ab.tile([96, 96], F32)
                    nc.vector.tensor_tensor(out=amf[:], in0=t60[:], in1=ws["vmask"][:],
                                            op=Alu.mult)
                    nc.vector.tensor_scalar_add(amf[:], amf[:], -60.0)
                    # rank+1 over full row via pairwise is_ge
                    rnk3 = ab.tile([96, 96 * 96], F32)
                    a_i = amf[:].unsqueeze(1).broadcast_to([96, 96, 96])
                    a_d = amf[:].unsqueeze(2).broadcast_to([96, 96, 96])
                    rvw = rnk3[:].rearrange("p (i d) -> p i d", d=96)
                    nc.vector.tensor_tensor(out=rvw, in0=a_i, in1=a_d, op=Alu.is_ge)
                    rank1 = ab.tile([96, 96], F32)
                    nc.vector.reduce_sum(rank1[:].unsqueeze(2), rvw, axis=AX)
                    sel = ab.tile([96, 96], F32)
                    nc.vector.tensor_tensor(out=sel[:], in0=rank1[:],
                                            in1=dynk[:].broadcast_to([96, 96]), op=Alu.is_le)
                    am = ab.tile([96, 96], F32)
                    t60b = ab.tile([96, 96], F32)
                    nc.vector.tensor_scalar_add(t60b[:], amf[:], 60.0)
                    nc.vector.tensor_tensor(out=am[:], in0=t60b[:], in1=sel[:], op=Alu.mult)
                    nc.vector.tensor_scalar_add(am[:], am[:], -60.0)
                    mx = ab.tile([96, 1], F32)
                    nc.vector.reduce_max(mx[:], am[:], axis=AX)
                    nmx = ab.tile([96, 1], F32)
                    nc.vector.tensor_scalar_mul(nmx[:], mx[:], -1.0)
                    ex = ab.tile([96, 96], F32)
                    nc.scalar.activation(ex[:], am[:], Act.Exp, bias=nmx[:])
                    sme = ab.tile([96, 1], F32)
                    nc.vector.reduce_sum(sme[:], ex[:], axis=AX)
                    rsm = ab.tile([96, 1], F32)
                    nc.vector.reciprocal(rsm[:], sme[:])
                    # probs scaled by attn_scale; then fold proj1 into the
                    # attention matmul: ppt = (proj1 @ (scale*P))^T = P's^T @ proj1T
                    nc.vector.tensor_scalar_mul(rsm[:], rsm[:], attn_scale)
                    probs = ab.tile([96, 96], F32R)
                    nc.vector.tensor_scalar_mul(probs[:], ex[:], rsm[:])
                    pp_ps = atps.tile([96, 192], F32, tag="atp2")
                    nc.tensor.matmul(pp_ps[:], probs[:], ws["proj1T"][:],
                                     start=True, stop=True)
                    nc.scalar.copy(ppt[:], pp_ps[:])

            # ============ PHASE C (fused attnV+proj+LN2+fc1, then C2) ============
            _wpab_cm.__exit__(None, None, None)
            _wpc_cm = tc.tile_pool(name="wpC", bufs=1)
            wpc = _wpc_cm.__enter__()
            _load_w(wpc, C_ONLY_W)
            with tc.tile_pool(name="c_v0", bufs=1) as cv0:
                v0t1 = cv0.tile([128, P3], BF16)
                v0t2 = cv0.tile([128, P3], BF16)
                with (
                    tc.tile_pool(name="f_rot", bufs=2) as fr,
                    tc.tile_pool(name="f_ps", bufs=1, space="PSUM") as fpp,
                ):
                    nc.vector.memset(v0t1[:], 0.0)
                    nc.vector.memset(v0t2[:], 0.0)
                    # pad cells must hold -t1/s1 so the bn-folded depthwise
                    # reads zeros in v0_bn space at image borders
                    nc.vector.tensor_scalar_add(v0t1[:], v0t1[:], ws["padv1"])
                    nc.vector.tensor_scalar_add(v0t2[:], v0t2[:], ws["padv2"])
                    for ci in range(NSC):
                        o0 = ci * 512
                        NN = min(512, S - o0)
                        c0 = o0 // W
                        nr_c = NN // W
                        vch = fr.tile([96, 512], F32R, tag="vch")
                        nc.sync.dma_start(out=vch[:, :NN], in_=v_sp[:, o0:o0 + NN])
                        x2ch = fr.tile([96, 512], F32R, tag="x2ch")
                        nc.sync.dma_start(out=x2ch[:, :NN], in_=yn2_sp[:, o0:o0 + NN])
                        xpch = [fr.tile([96, 512], F32R, tag=f"xp{cg}", name=f"xp{cg}") for cg in range(2)]
                        xsq = [fr.tile([96, 512], F32R, tag=f"xs{cg}", name=f"xs{cg}") for cg in range(2)]
                        for cg in range(2):
                            xcch = fr.tile([96, 512], F32R, tag=f"xcc{cg}")
                            nc.sync.dma_start(out=xcch[:, :NN], in_=xc_sp[cg][:, o0:o0 + NN])
                            pj_ps = fpp.tile([96, 512], F32, tag=f"pj{cg}", bufs=2)
                            nc.tensor.matmul(pj_ps[:, :NN],
                                             ppt[:, cg * 96:(cg + 1) * 96],
                                             vch[:, :NN], start=True, stop=False)
                            nc.tensor.matmul(pj_ps[:, :NN],
                                             ws["proj2T"][:, cg * 96:(cg + 1) * 96],
                                             x2ch[:, :NN], start=False, stop=True)
                            nc.vector.scalar_tensor_tensor(
                                out=xpch[cg][:, :NN], in0=pj_ps[:, :NN],
                                scalar=ws["proj_bias"][:, cg:cg + 1], in1=xcch[:, :NN],
                                op0=Alu.add, op1=Alu.add)
                            nc.sync.dma_start(out=xcp_sp[cg][:, o0:o0 + NN],
                                              in_=xpch[cg][:, :NN])
                            nc.scalar.square(xsq[cg][:, :NN], xpch[cg][:, :NN])
                        mu_ps = fpp.tile([128, 512], F32, tag="mu")
                        m2_ps = fpp.tile([128, 512], F32, tag="m2")
                        for cg in range(2):
                            nc.tensor.matmul(mu_ps[:, :NN], ws["ones_st"], xpch[cg][:, :NN],
                                             start=(cg == 0), stop=(cg == 1))
                            nc.tensor.matmul(m2_ps[:, :NN], ws["ones_st"], xsq[cg][:, :NN],
                                             start=(cg == 0), stop=(cg == 1))
                        musq = fr.tile([128, 512], F32, tag="musq")
                        nc.scalar.square(musq[:, :NN], mu_ps[:, :NN])
                        var = fr.tile([128, 512], F32, tag="var")
                        nc.vector.tensor_tensor(out=var[:, :NN], in0=m2_ps[:, :NN],
                                                in1=musq[:, :NN], op=Alu.subtract)
                        sd2 = fr.tile([128, 512], F32, tag="sd2")
                        nc.scalar.activation(sd2[:, :NN], var[:, :NN], Act.Sqrt,
                                             bias=ws["epsv"])
                        rstd = fr.tile([128, 512], F32, tag="rstd")
                        nc.vector.reciprocal_approx_fast(rstd[:, :NN], sd2[:, :NN])
                        yn2t = [fr.tile([96, 512], F32R, tag=f"cy{cg}", name=f"cy{cg}") for cg in range(2)]
                        for cg in range(2):
                            td = fr.tile([96, 512], F32, tag=f"ctd{cg}")
                            nc.vector.tensor_tensor(out=td[:, :NN], in0=xpch[cg][:, :NN],
                                                    in1=mu_ps[:96, :NN], op=Alu.subtract)
                            nc.vector.tensor_tensor(out=yn2t[cg][:, :NN], in0=td[:, :NN],
                                                    in1=rstd[:96, :NN], op=Alu.mult)
                        d0 = pd3(3 + c0) + 3
                        pv1 = v0t1[:, d0:d0 + nr_c * Wp3].rearrange(
                            "p (r w) -> p r w", w=Wp3)[:, :, 0:W]
                        pv2 = v0t2[:, d0:d0 + nr_c * Wp3].rearrange(
                            "p (r w) -> p r w", w=Wp3)[:, :, 0:W]
                        pv2s = v0t2[:, d0 + 1:d0 + 1 + nr_c * Wp3].rearrange(
                            "p (r w) -> p r w", w=Wp3)[:, :, 0:W]
                        for mg in range(2):
                            fch = fpp.tile([128, 512], F32, tag="fc", bufs=2)
                            for cg in range(2):
                                nc.tensor.matmul(
                                    fch[:, :NN],
                                    ws["fc1T"][:, (mg * 2 + cg) * 128:(mg * 2 + cg + 1) * 128],
                                    yn2t[cg][:, :NN], start=(cg == 0), stop=(cg == 1))
                            fv = fch[:, :NN].rearrange("p (r w) -> p r w", w=W)
                            if mg == 0:
                                vg0 = fr.tile([64, 512], F32R, tag="vg0")
                                nc.scalar.activation(vg0[:, :NN], fch[0:64, :NN], Act.Gelu,
                                                     bias=ws["fc1_bias"][0:64, 0:1])
                                ug0 = fr.tile([64, 512], F32R, tag="ug0")
                                nc.scalar.activation(ug0[:, :NN], vg0[:, :NN], Act.Gelu,
                                                     bias=ws["g0_bias"], scale=ws["g0_scale"])
                                nc.sync.dma_start(
                                    out=vg0_sp[:, d0:d0 + nr_c * Wp3].rearrange(
                                        "p (r w) -> p r w", w=Wp3)[:, :, 0:W],
                                    in_=vg0[:, :NN].rearrange("p (r w) -> p r w", w=W))
                                nc.sync.dma_start(
                                    out=ug0_sp[:, d0:d0 + nr_c * Wp3].rearrange(
                                        "p (r w) -> p r w", w=Wp3)[:, :, 0:W],
                                    in_=ug0[:, :NN].rearrange("p (r w) -> p r w", w=W))
                                nc.scalar.activation(pv1[0:64], fv[64:128], Act.Gelu,
                                                     bias=ws["fc1_bias"][64:128, 0:1])
                            else:
                                nc.scalar.activation(pv1[64:128], fv[0:64], Act.Gelu,
                                                     bias=ws["fc1_bias"][0:64, 1:2])
                                nc.scalar.activation(pv2[0:64], fv[64:128], Act.Gelu,
                                                     bias=ws["fc1_bias"][64:128, 1:2])
                                nc.scalar.activation(pv2s[64:128], fv[64:128], Act.Gelu,
                                                     bias=ws["fc1_bias"][64:128, 1:2])

                with (
                    tc.tile_pool(name="c2_rot", bufs=2) as c2r,
                    tc.tile_pool(name="c2_ps", bufs=2, space="PSUM") as c2ps,
                ):
                    for ci in range(NCH):
                        c0 = ci * RC
                        nr_c = min(RC, H - c0)
                        N = nr_c * Wp3
                        NN = nr_c * W
                        sb0 = pd3(3 + c0)
                        ps_a = c2ps.tile([128, RC * Wp3], F32, tag="psa")
                        for t in range(25):
                            dy, dx = t // 5 - 2, t % 5 - 2
                            o = sb0 + dy * Wp3 + dx
                            nc.tensor.matmul(ps_a[:, :N],
                                             ws["pair_diag"][:, t * 128:(t + 1) * 128],
                                             v0t1[:, o:o + N],
                                             start=(t == 0), stop=(t == 24))
                        ps_b = c2ps.tile([64, RC * Wp3], F32, tag="psb")
                        for i, (dy, dxa, hasb) in enumerate(dw3_passes):
                            o = sb0 + dy * Wp3 + dxa
                            nc.tensor.matmul(ps_b[:, :N],
                                             ws["dw3_diag"][:, i * 64:(i + 1) * 64],
                                             v0t2[:, o:o + N],
                                             start=(i == 0), stop=(i == len(dw3_passes) - 1))

                        def inner(ap_flat, lo, hi):
                            # interior view of a PSUM chunk (starts at free 0)
                            return ap_flat[lo:hi, :N].rearrange(
                                "p (r w) -> p r w", w=Wp3)[:, :, 3:3 + W]

                        def inner_v0(ap_flat, lo, hi):
                            # interior view of the padded v0 buffers at this chunk
                            return ap_flat[lo:hi, sb0:sb0 + N].rearrange(
                                "p (r w) -> p r w", w=Wp3)[:, :, 3:3 + W]

                        ug_a = c2r.tile([128, RC * W], F32R, tag="uga")
                        ug_b = c2r.tile([128, RC * W], F32R, tag="ugb")
                        vb_a = c2r.tile([128, RC * W], F32, tag="vba")
                        vb_b = c2r.tile([128, RC * W], F32, tag="vbb")
                        g0v = c2r.tile([64, RC * W], F32R, tag="g0v")
                        src3 = vg0_sp[:, sb0:sb0 + N].rearrange(
                            "p (r w) -> p r w", w=Wp3)[:, :, 3:3 + W]
                        nc.sync.dma_start(
                            out=g0v[:, :NN].rearrange("p (r w) -> p r w", w=W), in_=src3)
                        usrc3 = ug0_sp[:, sb0:sb0 + N].rearrange(
                            "p (r w) -> p r w", w=Wp3)[:, :, 3:3 + W]
                        nc.sync.dma_start(
                            out=ug_a[0:64, :NN].rearrange("p (r w) -> p r w", w=W), in_=usrc3)
                        nc.scalar.activation(
                            ug_a[64:128, :NN].rearrange("p (r w) -> p r w", w=W),
                            inner(ps_a, 0, 64), Act.Gelu, bias=ws["pair_bias"][0:64])
                        nc.scalar.activation(
                            ug_b[0:64, :NN].rearrange("p (r w) -> p r w", w=W),
                            inner(ps_a, 64, 128), Act.Gelu, bias=ws["pair_bias"][64:128])
                        nc.scalar.activation(
                            ug_b[64:128, :NN].rearrange("p (r w) -> p r w", w=W),
                            inner(ps_b, 0, 64), Act.Gelu, bias=ws["dw3_bias"])
                        nc.vector.tensor_scalar(out=vb_a[0:64, :NN], in0=g0v[:, :NN],
                                                scalar1=ws["s1a"][0:64],
                                                scalar2=ws["t1a"][0:64],
                                                op0=Alu.mult, op1=Alu.add)
                        nc.vector.tensor_scalar(out=vb_a[64:128, :NN],
                                                in0=inner_v0(v0t1, 0, 64),
                                                scalar1=ws["s1a"][64:128],
                                                scalar2=ws["t1a"][64:128],
                                                op0=Alu.mult, op1=Alu.add)
                        nc.vector.tensor_scalar(out=vb_b[0:64, :NN],
                                                in0=inner_v0(v0t1, 64, 128),
                                                scalar1=ws["s1b"][0:64],
                                                scalar2=ws["t1b"][0:64],
                                                op0=Alu.mult, op1=Alu.add)
                        nc.vector.tensor_scalar(out=vb_b[64:128, :NN],
                                                in0=inner_v0(v0t2, 0, 64),
                                                scalar1=ws["s1b"][64:128],
                                                scalar2=ws["t1b"][64:128],
                                                op0=Alu.mult, op1=Alu.add)
                        # u_bn2 = s2*gelu(u)+t2 made explicit so fc2 is 2 matmuls/cg
                        ub_a = c2r.tile([128, RC * W], F32, tag="uba")
                        ub_b = c2r.tile([128, RC * W], F32, tag="ubb")
                        nc.vector.tensor_scalar(out=ub_a[:, :NN], in0=ug_a[:, :NN],
                                                scalar1=ws["s2a"], scalar2=ws["t2a"],
                                                op0=Alu.mult, op1=Alu.add)
                        nc.vector.tensor_scalar(out=ub_b[:, :NN], in0=ug_b[:, :NN],
                                                scalar1=ws["s2b"], scalar2=ws["t2b"],
                                                op0=Alu.mult, op1=Alu.add)
                        z1a = c2r.tile([128, RC * W], F32R, tag="z1a")
                        z1b = c2r.tile([128, RC * W], F32R, tag="z1b")
                        nc.vector.tensor_tensor(out=z1a[:, :NN], in0=ub_a[:, :NN],
                                                in1=vb_a[:, :NN], op=Alu.mult)
                        nc.vector.tensor_tensor(out=z1b[:, :NN], in0=ub_b[:, :NN],
                                                in1=vb_b[:, :NN], op=Alu.mult)
                        for cg in range(2):
                            ops = c2ps.tile([96, RC * W], F32, tag=f"ops{cg}")
                            nc.tensor.matmul(ops[:, :NN],
                                             ws["fc2aT"][:, (cg * 2) * 96:(cg * 2 + 1) * 96],
                                             z1a[:, :NN], start=True, stop=False)
                            nc.tensor.matmul(ops[:, :NN],
                                             ws["fc2aT"][:, (cg * 2 + 1) * 96:(cg * 2 + 2) * 96],
                                             z1b[:, :NN], start=False, stop=True)
                            xrch = c2r.tile([96, RC * W], F32R, tag=f"xr{cg}", bufs=1)
                            nc.sync.dma_start(out=xrch[:, :NN],
                                              in_=xcp_sp[cg][:, c0 * W:c0 * W + NN])
                            ob = c2r.tile([96, RC * W], F32, tag=f"ob{cg}", bufs=1)
                            nc.vector.tensor_scalar(out=ob[:, :NN], in0=ops[:, :NN],
                                                    scalar1=ws["s3v"][:, cg:cg + 1],
                                                    scalar2=ws["out_bias"][:, cg:cg + 1],
                                                    op0=Alu.mult, op1=Alu.add)
                            oc = c2r.tile([96, RC * W], F32, tag=f"oc{cg}", bufs=1)
                            nc.vector.tensor_tensor(out=oc[:, :NN], in0=ob[:, :NN],
                                                    in1=xrch[:, :NN], op=Alu.add)
                            nc.sync.dma_start(
                                out=out_t[cg * 96:(cg + 1) * 96, c0 * W:c0 * W + NN],
                                in_=oc[:, :NN])
            _wpc_cm.__exit__(None, None, None)
    return out_t.name


# ----------------------------------------------------------------------------
# host entry
# ----------------------------------------------------------------------------

_CACHE = {}


def make_program(H, W, n_cores, attn_scale, dw3_passes):
    key = (H, W, n_cores, round(attn_scale, 9))
    if key in _CACHE:
        return _CACHE[key]
    nc = bacc.Bacc("TRN2", target_bir_lowering=False, debug=False, num_devices=n_cores)
    out_name = build(nc, H, W, n_cores, attn_scale, dw3_passes)
    nc.compile()
    _CACHE[key] = (nc, out_name)
    return nc, out_name


def make_in_maps(inputs):
    x = np.asarray(inputs["x"], np.float32)
    B = x.shape[0]
    wdict = _prep_weights({k: np.asarray(v) for k, v in inputs.items()})
    base = {}
    for k, (shp, d) in WSPEC.items():
        base["w_" + k] = wdict[k][0].reshape(shp)
    in_maps = []
    for b in range(B):
        m = dict(base)
        m["x"] = np.ascontiguousarray(x[b])
        in_maps.append(m)
    return in_maps, wdict


def kernel(**inputs):
    x = np.asarray(inputs["x"], np.float32)
    B, H, W, C = x.shape
    in_maps, wdict = make_in_maps(inputs)
    nc, out_name = make_program(H, W, B, wdict["_attn_scale"][0],
                                wdict["_dw3_passes"][0])
    res = bass_utils.run_bass_kernel_spmd(nc, in_maps, core_ids=list(range(B)))
    return np.stack([
        np.asarray(res.results[b][out_name]).reshape(C, H * W).T.reshape(H, W, C)
        for b in range(B)])



# revision 37
# speedup vs baseline: 8.0023x; 1.0528x over previous
"""Trainium2 Bass kernel for nn_Block_87351044866235 (sparse_attention).

Data-parallel over batch: 8 samples -> 8 NeuronCores. Channel-major
layout [C, H*W] on chip; depthwise convs as diagonal fp32r matmuls on
TensorE; 1x1 convs as fp32r matmuls; LN stats via ones-matmuls; q/k gram
via hi/lo bf16 split + DMA-xbar transposes; dynamic-k gate mean via a
scalar AllReduce.
"""
import sys, os

for _p in ("/opt/trn_rl_repo", "/root/.axon_site/_ro/trn_rl_repo"):
    if os.path.isdir(_p) and _p not in sys.path:
        sys.path.append(_p)

import numpy as np
import ml_dtypes
import concourse.bass as bass
import concourse.bacc as bacc
import concourse.tile as tile
from concourse import mybir
from concourse import bass_utils

try:
    from concourse import tile_utils as _tu
    _tu.max_sbuf_usage = 208 * 1024
except Exception:
    pass

dt = mybir.dt
Alu = mybir.AluOpType
Act = mybir.ActivationFunctionType
AX = mybir.AxisListType.X

EMBED, PDIM, HEADS, HID = 192, 96, 8, 256
CPH = PDIM // HEADS  # 12
SLOP = 8
RC = 3    # conv output rows per chunk
BR = 12   # rows per band

F32, F32R, BF16 = dt.float32, dt.float32r, dt.bfloat16


def _ceil(a, b):
    return (a + b - 1) // b


# ----------------------------------------------------------------------------
# host-side weight prep: everything 2D [partitions, free]
# ----------------------------------------------------------------------------

def _prep_weights(p):
    w = {}
    f32r = lambda a: (np.ascontiguousarray(a, np.float32), F32R)
    f32 = lambda a: (np.ascontiguousarray(a, np.float32), F32)
    eps_bn = 1e-5

    w["ident"] = f32(np.eye(128, dtype=np.float32))
    w["identb"] = (np.eye(128, dtype=np.float32).astype(ml_dtypes.bfloat16), BF16)

    # pos depthwise diag: [96, (t*2+cg)*96]
    pw = p["pos_w"][:, 0]  # [192,3,3]
    pos_d = np.zeros((96, 18 * 96), np.float32)
    for t in range(9):
        dy, dx = t // 3 - 1, t % 3 - 1
        for cg in range(2):
            pos_d[:, (t * 2 + cg) * 96:(t * 2 + cg + 1) * 96] = \
                np.diag(pw[cg * 96:(cg + 1) * 96, dy + 1, dx + 1])
    w["pos_diag"] = f32r(pos_d)
    w["pos_b"] = f32(p["pos_b"].reshape(2, 96).T)  # [96, 2]

    g1v, b1v = p["ln1_g"], p["ln1_b"]
    qw = p["qkv_w"][:, :, 0, 0]  # [288, 96]
    qw_eff = qw * g1v[None, :96]
    qdw = p["qkv_dw_w"][:, 0]  # [288,3,3]
    # 1x1 folded into each depthwise tap (dense per-tap weights); row 96 is
    # the ones-channel carrying the position-dependent bias
    b0 = qw @ b1v[:96]  # [288]
    qkv_full = np.zeros((97, 27 * 96), np.float32)
    for t in range(9):
        dy, dx = t // 3 - 1, t % 3 - 1
        for j in range(3):
            wj = qw_eff[j * 96:(j + 1) * 96, :]           # [96 out, 96 in]
            dj = qdw[j * 96:(j + 1) * 96, dy + 1, dx + 1]  # [96]
            qkv_full[:96, (t * 3 + j) * 96:(t * 3 + j + 1) * 96] = (wj * dj[:, None]).T
            qkv_full[96, (t * 3 + j) * 96:(t * 3 + j + 1) * 96] = b0[j * 96:(j + 1) * 96] * dj
    w["qkv_fullT"] = f32r(qkv_full)

    gw1 = p["gate_w1"][:, :, 0, 0]  # [96, 192]
    gw1_eff = gw1 * g1v[None, :]
    w["gate_w1T"] = f32r(np.concatenate(
        [gw1_eff[:, cg * 96:(cg + 1) * 96].T for cg in range(2)], axis=1))  # [96, 192]
    w["gate_b1"] = f32((p["gate_b1"] + gw1 @ b1v).reshape(96, 1))
    w["gate_w2T"] = f32r(p["gate_w2"][:, :, 0, 0].T.copy())  # [96,1]
    w["gate_b2"] = f32(p["gate_b2"].reshape(1, 1))

    pj = p["proj_w"][:, :, 0, 0]
    pj1, pj2 = pj[:, :96], pj[:, 96:] * g1v[None, 96:]
    w["proj1T"] = f32r(np.concatenate(
        [pj1[cg * 96:(cg + 1) * 96].T for cg in range(2)], axis=1))  # [96, 192]
    w["proj2T"] = f32r(np.concatenate(
        [pj2[cg * 96:(cg + 1) * 96].T for cg in range(2)], axis=1))
    w["proj_bias"] = f32((pj[:, 96:] @ b1v[96:]).reshape(2, 96).T)  # [96, 2]

    attn_scale = float(p["attn1"][0] + p["attn2"][0] + p["attn3"][0] + p["attn4"][0])
    w["_attn_scale"] = (attn_scale, None)
    w["tempvec"] = f32(np.repeat(p["temperature"].reshape(HEADS), CPH).reshape(96, 1))

    g2v, b2v = p["ln2_g"], p["ln2_b"]
    f1 = p["fc1_w"][:, :, 0, 0]  # [256, 192]
    f1_eff = f1 * g2v[None, :]
    fc1 = np.zeros((96, 4 * 128), np.float32)
    for mg in range(2):
        for cg in range(2):
            fc1[:, (mg * 2 + cg) * 128:(mg * 2 + cg + 1) * 128] = \
                f1_eff[mg * 128:(mg + 1) * 128, cg * 96:(cg + 1) * 96].T
    w["fc1T"] = f32r(fc1)
    w["fc1_bias"] = f32((f1 @ b2v).reshape(2, 128).T)  # [128, 2]

    s1 = p["bn1_g"] / np.sqrt(p["bn1_v"] + eps_bn)
    t1 = p["bn1_b"] - p["bn1_m"] * s1
    s2 = p["bn2_g"] / np.sqrt(p["bn2_v"] + eps_bn)
    t2 = p["bn2_b"] - p["bn2_m"] * s2
    s3 = p["bn3_g"] / np.sqrt(p["bn3_v"] + eps_bn)
    t3 = p["bn3_b"] - p["bn3_m"] * s3

    dw1w, dw2w, dw3w = p["dw1_w"][:, 0], p["dw2_w"][:, 0], p["dw3_w"][:, 0]
    dw1b, dw2b, dw3b = p["dw1_b"], p["dw2_b"], p["dw3_b"]
    s1g = [s1[i * 64:(i + 1) * 64] for i in range(4)]
    t1g = [t1[i * 64:(i + 1) * 64] for i in range(4)]

    pair_d = np.zeros((128, 25 * 128), np.float32)
    for t in range(25):
        dy, dx = t // 5 - 2, t % 5 - 2
        blk = np.zeros((128, 128), np.float32)
        d2 = dw2w[:, dy + 2, dx + 2] * s1g[2]
        if dy == 0 and dx == 0:
            d2 = d2 + s1g[2]
        blk[64:, 64:] = np.diag(d2)
        if -1 <= dy <= 1 and -1 <= dx <= 1:
            d1 = dw1w[:, dy + 1, dx + 1] * s1g[1]
            if dy == 0 and dx == 0:
                d1 = d1 + s1g[1]
            blk[:64, :64] = np.diag(d1)
        pair_d[:, t * 128:(t + 1) * 128] = blk
    w["pair_diag"] = (pair_d.astype(ml_dtypes.bfloat16), BF16)
    bc1 = t1g[1] * dw1w.sum((1, 2)) + dw1b + t1g[1]
    bc2 = t1g[2] * dw2w.sum((1, 2)) + dw2b + t1g[2]
    w["pair_bias"] = f32(np.concatenate([bc1, bc2]).reshape(128, 1))

    # rows 64:128 of v0t2 hold the same data stored shifted +1, so a read at
    # AP offset (dy, dxa) yields tap (dy, dxa-1) for those rows.
    dw3_passes = []
    for dy in range(-3, 4):
        for dxa in (-2, 0, 2):
            dw3_passes.append((dy, dxa, True))
        dw3_passes.append((dy, 3, False))
    dw3_d = np.zeros((128, len(dw3_passes) * 64), np.float32)
    for i, (dy, dxa, hasb) in enumerate(dw3_passes):
        wa = dw3w[:, dy + 3, dxa + 3] * s1g[3]
        if dy == 0 and dxa == 0:
            wa = wa + s1g[3]
        dw3_d[:64, i * 64:(i + 1) * 64] = np.diag(wa)
        if hasb:
            wb = dw3w[:, dy + 3, dxa - 1 + 3] * s1g[3]
            if dy == 0 and dxa - 1 == 0:
                wb = wb + s1g[3]
            dw3_d[64:, i * 64:(i + 1) * 64] = np.diag(wb)
    w["dw3_diag"] = (dw3_d.astype(ml_dtypes.bfloat16), BF16)
    w["_dw3_passes"] = (dw3_passes, None)
    w["dw3_bias"] = f32((t1g[3] * dw3w.sum((1, 2)) + dw3b + t1g[3]).reshape(64, 1))

    d0w, d0b = p["dw0_w"][:, 0, 0, 0], p["dw0_b"]
    w["g0_scale"] = f32(((d0w + 1.0) * s1g[0]).reshape(64, 1))
    w["g0_bias"] = f32(((d0w + 1.0) * t1g[0] + d0b).reshape(64, 1))

    f2 = p["fc2_w"][:, :, 0, 0]  # [192, 256]
    # z = (s2*gelu(u)+t2) * v0bn is computed explicitly on DVE, so fc2 is plain
    fc2a = np.zeros((128, 4 * 96), np.float32)
    for cg in range(2):
        for kg in range(2):
            fc2a[:, (cg * 2 + kg) * 96:(cg * 2 + kg + 1) * 96] = \
                f2[cg * 96:(cg + 1) * 96, kg * 128:(kg + 1) * 128].T
    w["fc2aT"] = f32r(fc2a)
    w["s3v"] = f32(np.stack([s3[:96], s3[96:]], axis=1))          # [96, 2]
    w["out_bias"] = f32(np.stack([t3[:96], t3[96:]], axis=1))     # [96, 2]
    w["s2a"] = f32(s2[:128].reshape(128, 1))
    w["s2b"] = f32(s2[128:].reshape(128, 1))
    w["t2a"] = f32(t2[:128].reshape(128, 1))
    w["t2b"] = f32(t2[128:].reshape(128, 1))

    sg = np.where(s1 == 0, 1.0, s1)
    padv = -t1 / sg
    w["padv1"] = f32(np.concatenate([padv[64:128], padv[128:192]]).reshape(128, 1))
    w["padv2"] = f32(np.concatenate([padv[192:256], padv[192:256]]).reshape(128, 1))
    w["s1a"] = f32(s1[:128].reshape(128, 1))
    w["s1b"] = f32(s1[128:].reshape(128, 1))
    w["t1a"] = f32(t1[:128].reshape(128, 1))
    w["t1b"] = f32(t1[128:].reshape(128, 1))

    w["ones_st"] = f32r(np.full((96, 128), 1.0 / EMBED, np.float32))
    w["epsv"] = f32(np.full((128, 1), 1e-6, np.float32))
    vm = np.zeros((96, 96), np.float32)
    for h in range(HEADS):
        vm[h * CPH:(h + 1) * CPH, h * CPH:(h + 1) * CPH] = 1.0
    w["vmask"] = f32(vm)
    return w


WSPEC = {
    "ident": ([128, 128], F32), "identb": ([128, 128], BF16),
    "pos_diag": ([96, 18 * 96], F32R),
    "pos_b": ([96, 2], F32), "qkv_fullT": ([97, 27 * 96], F32R),
    "gate_w1T": ([96, 192], F32R), "gate_b1": ([96, 1], F32),
    "gate_w2T": ([96, 1], F32R), "gate_b2": ([1, 1], F32),
    "proj1T": ([96, 192], F32R), "proj2T": ([96, 192], F32R),
    "proj_bias": ([96, 2], F32), "tempvec": ([96, 1], F32),
    "fc1T": ([96, 4 * 128], F32R), "fc1_bias": ([128, 2], F32),
    "pair_diag": ([128, 25 * 128], BF16), "pair_bias": ([128, 1], F32),
    "dw3_diag": ([128, 28 * 64], BF16), "dw3_bias": ([64, 1], F32),
    "g0_scale": ([64, 1], F32), "g0_bias": ([64, 1], F32),
    "fc2aT": ([128, 4 * 96], F32R),
    "s3v": ([96, 2], F32), "out_bias": ([96, 2], F32),
    "s2a": ([128, 1], F32), "s2b": ([128, 1], F32),
    "t2a": ([128, 1], F32), "t2b": ([128, 1], F32),
    "padv1": ([128, 1], F32),
    "padv2": ([128, 1], F32),
    "s1a": ([128, 1], F32), "s1b": ([128, 1], F32),
    "t1a": ([128, 1], F32), "t1b": ([128, 1], F32),
    "ones_st": ([96, 128], F32R),
    "epsv": ([128, 1], F32),
    "vmask": ([96, 96], F32),
}


# ----------------------------------------------------------------------------
# device kernel
# ----------------------------------------------------------------------------

def build(nc, H, W, n_cores, attn_scale, dw3_passes):
    S = H * W
    Wp1 = W + 2
    P1B = (BR + 2) * Wp1 + 2 * SLOP   # band buffer (pad1)
    Wp3, Hp3 = W + 6, H + 6
    P3 = Hp3 * Wp3 + 2 * SLOP
    NCH = _ceil(H, RC)
    NB = _ceil(H, BR)
    NSC = _ceil(S, 512)
    GCH = 512 // W                    # gate chunk rows (512 cols)
    NGC_PER_BAND = _ceil(BR, GCH)

    x_t = nc.dram_tensor("x", [H, W, EMBED], F32, kind="ExternalInput")
    # channel-major output: [192, S]; host does the final (H,W,C) transpose
    out_t = nc.dram_tensor("out", [EMBED, S], F32, kind="ExternalOutput")
    wt = {k: nc.dram_tensor("w_" + k, shp, d, kind="ExternalInput")
          for k, (shp, d) in WSPEC.items()}

    def pd3(r):
        return SLOP + r * Wp3

    def band_pad_memsets(tile_, r0, r1):
        # zero only the pad cells of a [*, P1B] band buffer:
        # SLOP head (+ leading pad col), SLOP tail, the (129, 0) pad-col pair
        # between consecutive row slots, and missing halo row slots at the
        # image top/bottom.
        Wp1_ = W + 2
        nrows = BR + 2
        nc.vector.memset(tile_[:, 0:SLOP + 1].bitcast(F32), 0.0)
        nc.vector.memset(tile_[:, P1B - SLOP - 1:P1B].bitcast(F32), 0.0)
        pv = tile_[:, SLOP + Wp1_ - 1:SLOP + Wp1_ - 1 + (nrows - 1) * Wp1_] \
            .rearrange("p (r w) -> p r w", w=Wp1_)[:, :, 0:2]
        nc.vector.memset(pv.bitcast(F32), 0.0)
        if r0 == 0:
            nc.vector.memset(tile_[:, SLOP:SLOP + Wp1_].bitcast(F32), 0.0)
        if r1 >= H:
            sl = r1 - r0 + 1
            nc.vector.memset(
                tile_[:, SLOP + sl * Wp1_:SLOP + (sl + 1) * Wp1_].bitcast(F32), 0.0)

    with tile.TileContext(nc) as tc:
        PERS_W = ['ident', 'identb', 'ones_st', 'epsv', 'vmask', 'tempvec']
        C_ONLY_W = ['fc1T', 'fc1_bias', 'pair_diag', 'pair_bias', 'dw3_diag',
                    'dw3_bias', 'g0_scale', 'g0_bias', 'fc2aT', 's3v', 'out_bias',
                    's1a', 's1b', 't1a', 't1b', 's2a', 's2b', 't2a', 't2b',
                    'padv1', 'padv2', 'proj2T', 'proj_bias']
        with (
            tc.tile_pool(name="dram", bufs=1, space="DRAM") as dram,
            tc.tile_pool(name="persist", bufs=1) as pers,
        ):
            ws = {}

            def _load_w(pool, names):
                for k in names:
                    shp, d = WSPEC[k]
                    tl = pool.tile(shp, d, tag="w_" + k, name="w_" + k)
                    nc.sync.dma_start(out=tl[:], in_=wt[k][:])
                    ws[k] = tl


            yn1_sp = dram.tile([96, S], F32R)
            yn2_sp = dram.tile([96, S], F32R)
            xc_sp = [dram.tile([96, S], F32R, name=f"xc_sp{i}") for i in range(2)]
            v_sp = dram.tile([96, S], F32R)
            xcp_sp = [dram.tile([96, S], F32R, name=f"xcp_sp{i}") for i in range(2)]
            vg0_sp = dram.tile([64, P3], F32R)
            ug0_sp = dram.tile([64, P3], F32R)
            cc_in = dram.tile([1, 1], F32)
            cc_out = dram.tile([1, 1], F32)

            gsum = pers.tile([1, NB * NGC_PER_BAND + 8], F32)
            nc.vector.memset(gsum[:], 0.0)
            dynk = pers.tile([96, 1], F32)
            ppt = pers.tile([96, 192], F32R)   # (proj1 @ (attn_scale*P))^T

            # ================= PHASE A =================
            _load_w(pers, PERS_W)
            _wpab_cm = tc.tile_pool(name="wpAB", bufs=1)
            wpab = _wpab_cm.__enter__()
            _load_w(wpab, [k for k in WSPEC
                           if k not in C_ONLY_W and k not in PERS_W])
            ident = ws["ident"]
            with (
                tc.tile_pool(name="pa_band", bufs=2) as pab,
                tc.tile_pool(name="pa_rot", bufs=3) as par,
                tc.tile_pool(name="pa_ps", bufs=2, space="PSUM") as paps,
                tc.tile_pool(name="pa_ps2", bufs=2, space="PSUM") as paps2,
            ):
                for b in range(NB):
                    r0, r1 = b * BR, min((b + 1) * BR, H)
                    ylo, yhi = max(r0 - 1, 0), min(r1 + 1, H)
                    nrb = yhi - ylo
                    xband = [pab.tile([96, P1B], F32R, tag=f"xb{cg}", name=f"xb{cg}") for cg in range(2)]
                    for cg in range(2):
                        band_pad_memsets(xband[cg], r0, r1)
                    xraw = pab.tile([W, (BR + 2) * EMBED], F32, tag="xraw")
                    nc.sync.dma_start(
                        out=xraw[:, :nrb * EMBED].rearrange("w (r c) -> w r c", c=EMBED),
                        in_=x_t[ylo:yhi].rearrange("r w c -> w r c"))
                    for rr in range(ylo, yhi):
                        boff = SLOP + (rr - (r0 - 1)) * Wp1 + 1
                        for cg in range(2):
                            tps = paps2.tile([96, W], F32, tag="tps")
                            nc.tensor.transpose(
                                tps[:],
                                xraw[:, (rr - ylo) * EMBED + cg * 96:
                                     (rr - ylo) * EMBED + (cg + 1) * 96],
                                ident[:W, :W])
                            nc.scalar.copy(xband[cg][:, boff:boff + W], tps[:])
                    xcband = [pab.tile([96, BR * W], F32R, tag=f"xcb{cg}", name=f"xcb{cg}")
                              for cg in range(2)]
                    ynband = [pab.tile([96, BR * W], F32R, tag=f"ynb{cg}", name=f"ynb{cg}")
                              for cg in range(2)]
                    for c0 in range(r0, r1, RC):
                        nr_c = min(RC, H - c0)
                        N = nr_c * Wp1
                        NN = nr_c * W
                        bo = (c0 - r0) * W
                        sb0 = SLOP + (c0 - r0 + 1) * Wp1
                        xc_ch = [xcband[cg][:, bo:bo + NN] for cg in range(2)]
                        xsq = [par.tile([96, RC * W], F32R, tag=f"xq{cg}", name=f"xq{cg}") for cg in range(2)]
                        for cg in range(2):
                            ps = paps.tile([96, RC * Wp1], F32, tag="posps")
                            for t in range(9):
                                dy, dx = t // 3 - 1, t % 3 - 1
                                o = sb0 + dy * Wp1 + dx
                                nc.tensor.matmul(
                                    ps[:, :N],
                                    ws["pos_diag"][:, (t * 2 + cg) * 96:(t * 2 + cg + 1) * 96],
                                    xband[cg][:, o:o + N],
                                    start=(t == 0), stop=(t == 8))
                            ps_int = ps[:, :N].rearrange("p (r w) -> p r w", w=Wp1)[:, :, 1:1 + W]
                            xb_int = xband[cg][:, sb0:sb0 + N] \
                                .rearrange("p (r w) -> p r w", w=Wp1)[:, :, 1:1 + W]
                            xcv = xc_ch[cg].rearrange("p (r w) -> p r w", w=W)
                            nc.vector.scalar_tensor_tensor(
                                out=xcv, in0=ps_int, scalar=ws["pos_b"][:, cg:cg + 1],
                                in1=xb_int, op0=Alu.add, op1=Alu.add)
                            nc.scalar.square(xsq[cg][:, :NN], xc_ch[cg])
                        mu_ps = paps.tile([128, RC * W], F32, tag="mups")
                        m2_ps = paps.tile([128, RC * W], F32, tag="m2ps")
                        for cg in range(2):
                            nc.tensor.matmul(mu_ps[:, :NN], ws["ones_st"], xc_ch[cg],
                                             start=(cg == 0), stop=(cg == 1))
                            nc.tensor.matmul(m2_ps[:, :NN], ws["ones_st"], xsq[cg][:, :NN],
                                             start=(cg == 0), stop=(cg == 1))
                        musq = par.tile([128, RC * W], F32, tag="musq")
                        nc.scalar.square(musq[:, :NN], mu_ps[:, :NN])
                        var = par.tile([128, RC * W], F32, tag="var")
                        nc.vector.tensor_tensor(out=var[:, :NN], in0=m2_ps[:, :NN],
                                                in1=musq[:, :NN], op=Alu.subtract)
                        sd = par.tile([128, RC * W], F32, tag="sd")
                        nc.scalar.activation(sd[:, :NN], var[:, :NN], Act.Sqrt, bias=ws["epsv"])
                        rstd = par.tile([128, RC * W], F32, tag="rstd")
                        nc.vector.reciprocal_approx_fast(rstd[:, :NN], sd[:, :NN])
                        for cg in range(2):
                            tdf = par.tile([96, RC * W], F32, tag=f"td{cg}")
                            nc.vector.tensor_tensor(out=tdf[:, :NN], in0=xc_ch[cg],
                                                    in1=mu_ps[:96, :NN], op=Alu.subtract)
                            nc.vector.tensor_tensor(out=ynband[cg][:, bo:bo + NN],
                                                    in0=tdf[:, :NN],
                                                    in1=rstd[:96, :NN], op=Alu.mult)
                    NBW = (r1 - r0) * W
                    for cg in range(2):
                        sp = yn1_sp if cg == 0 else yn2_sp
                        nc.sync.dma_start(out=sp[:, r0 * W:r0 * W + NBW],
                                          in_=ynband[cg][:, :NBW])
                        nc.sync.dma_start(out=xc_sp[cg][:, r0 * W:r0 * W + NBW],
                                          in_=xcband[cg][:, :NBW])

            # ================= PHASE B =================
            with (
                tc.tile_pool(name="pb_band", bufs=1) as pbb,
                tc.tile_pool(name="pb_rot", bufs=3) as pbr,
                tc.tile_pool(name="gram_ps", bufs=1, space="PSUM") as gpsp,
            ):
                g1_ps = gpsp.tile([96, 384], F32)
                g2_ps = gpsp.tile([96, 288], F32)
                with (
                    tc.tile_pool(name="pb_psg", bufs=1, space="PSUM") as pbpsg,
                    tc.tile_pool(name="pb_ps", bufs=2, space="PSUM") as pbps,
                    tc.tile_pool(name="pb_tps", bufs=2, space="PSUM") as tpsp,
                ):
                    for b in range(NB):
                        r0, r1 = b * BR, min((b + 1) * BR, H)
                        ylo, yhi = max(r0 - 1, 0), min(r1 + 1, H)
                        # padded yn1 band (row 96 = ones-channel for folded bias)
                        ynp = pbb.tile([97, P1B], F32R, tag="ynp", name="ynp")
                        band_pad_memsets(ynp, r0, r1)
                        iview = ynp[0:96, SLOP + (ylo - r0 + 1) * Wp1 + 1:
                                    SLOP + (yhi - r0 + 1) * Wp1 + 1] \
                            .rearrange("p (r w) -> p r w", w=Wp1)[:, :, 0:W]
                        nc.sync.dma_start(
                            out=iview,
                            in_=yn1_sp[:, ylo * W:yhi * W].rearrange(
                                "p (r w) -> p r w", w=W))
                        ov = ynp[96:97, SLOP + (ylo - r0 + 1) * Wp1 + 1:
                                 SLOP + (yhi - r0 + 1) * Wp1 + 1] \
                            .rearrange("p (r w) -> p r w", w=Wp1)[:, :, 0:W]
                        nc.vector.memset(ov.bitcast(F32), 1.0)
                        ynb2 = pbb.tile([96, (BR + 2) * W], F32R, tag="ynb2", name="ynb2")
                        nc.sync.dma_start(
                            out=ynb2[:, (ylo - r0 + 1) * W:(yhi - r0 + 1) * W],
                            in_=yn2_sp[:, ylo * W:yhi * W])
                        # gate (512-col chunks over rows [r0, r1))
                        for gi in range(NGC_PER_BAND):
                            gr0 = r0 + gi * GCH
                            if gr0 >= r1:
                                break
                            ngr = min(GCH, r1 - gr0)
                            NG = ngr * W
                            yo = (gr0 - r0 + 1) * W
                            y1v = ynp[0:96, SLOP + (gr0 - r0 + 1) * Wp1 + 1:
                                      SLOP + (gr0 - r0 + 1 + ngr) * Wp1 + 1] \
                                .rearrange("p (r w) -> p r w", w=Wp1)[:, :, 0:W]
                            gps = pbpsg.tile([96, 512], F32, tag="gps")
                            nc.tensor.matmul(gps[:, :NG].rearrange(
                                "p (r w) -> p r w", w=W),
                                ws["gate_w1T"][:, 0:96], y1v,
                                start=True, stop=False)
                            nc.tensor.matmul(gps[:, :NG],
                                             ws["gate_w1T"][:, 96:192],
                                             ynb2[:, yo:yo + NG],
                                             start=False, stop=True)
                            g1s = pbr.tile([96, 512], F32R, tag="g1s")
                            nc.scalar.activation(g1s[:, :NG], gps[:, :NG], Act.Relu,
                                                 bias=ws["gate_b1"])
                            g2ps = pbpsg.tile([1, 512], F32, tag="g2ps")
                            nc.tensor.matmul(g2ps[:, :NG], ws["gate_w2T"], g1s[:, :NG],
                                             start=True, stop=True)
                            sgt = pbr.tile([1, 512], F32, tag="sgt")
                            idx = b * NGC_PER_BAND + gi
                            nc.scalar.activation(sgt[:, :NG], g2ps[:, :NG], Act.Sigmoid,
                                                 bias=ws["gate_b2"],
                                                 accum_out=gsum[0:1, idx:idx + 1])
                        # folded 1x1+depthwise -> hi/lo + transpose staging
                        qkband = pbr.tile([W, BR * 384], BF16, tag="qkband")
                        for c0 in range(r0, r1, RC):
                            nr_c = min(RC, H - c0)
                            N = nr_c * Wp1
                            NN = nr_c * W
                            sb0 = SLOP + (c0 - r0 + 1) * Wp1
                            hilo = {}
                            for j in range(3):
                                ps = pbps.tile([96, RC * Wp1], F32, tag="ps")
                                for t in range(9):
                                    dy, dx = t // 3 - 1, t % 3 - 1
                                    o = sb0 + dy * Wp1 + dx
                                    nc.tensor.matmul(
                                        ps[:, :N],
                                        ws["qkv_fullT"][:, (t * 3 + j) * 96:(t * 3 + j + 1) * 96],
                                        ynp[:, o:o + N],
                                        start=(t == 0), stop=(t == 8))
                                ps_int = ps[:, :N].rearrange("p (r w) -> p r w", w=Wp1)[:, :, 1:1 + W]
                                if j == 2:
                                    vch = pbr.tile([96, RC * W], F32R, tag="vch")
                                    nc.scalar.copy(
                                        vch[:, :NN].rearrange("p (r w) -> p r w", w=W), ps_int)
                                    nc.sync.dma_start(out=v_sp[:, c0 * W:c0 * W + NN],
                                                      in_=vch[:, :NN])
                                else:
                                    hi = pbr.tile([96, RC * W], BF16, tag=f"hi{j}")
                                    lo = pbr.tile([96, RC * W], BF16, tag=f"lo{j}")
                                    hiv = hi[:, :NN].rearrange("p (r w) -> p r w", w=W)
                                    nc.scalar.copy(hiv, ps_int)
                                    nc.vector.tensor_tensor(
                                        out=lo[:, :NN].rearrange("p (r w) -> p r w", w=W),
                                        in0=ps_int, in1=hiv, op=Alu.subtract)
                                    hilo[j] = (hi, lo)
                            for rr in range(c0, c0 + nr_c):
                                ro = (rr - r0) * 384
                                rl = (rr - c0) * W
                                tq = tpsp.tile([128, 384], BF16, tag="tq")
                                for idx, src in enumerate((hilo[0][0], hilo[1][0],
                                                           hilo[1][1], hilo[0][1])):
                                    nc.tensor.transpose(
                                        tq[:, idx * 96:(idx + 1) * 96],
                                        src[:, rl:rl + W], ws["identb"][:96, :96])
                                nc.scalar.copy(qkband[:, ro:ro + 384], tq[:])
                        for rr in range(r0, r1):
                            ro = (rr - r0) * 384
                            nc.tensor.matmul(g1_ps[:], qkband[:, ro:ro + 96],
                                             qkband[:, ro:ro + 384],
                                             start=(rr == 0), stop=(rr == H - 1))
                            nc.tensor.matmul(g2_ps[:], qkband[:, ro + 96:ro + 192],
                                             qkband[:, ro + 96:ro + 384],
                                             start=(rr == 0), stop=(rr == H - 1))

                # ---- gate mean -> AllReduce -> dynk ----
                gred = pers.tile([1, 1], F32)
                nc.vector.reduce_sum(gred[:], gsum[0:1, 0:NB * NGC_PER_BAND], axis=AX)
                gsc = pers.tile([1, 1], F32)
                nc.vector.tensor_scalar_mul(gsc[:], gred[:], float(CPH) / (n_cores * S))
                nc.sync.dma_start(out=cc_in[:], in_=gsc[:])
                nc.gpsimd.collective_compute(
                    "AllReduce", Alu.add, replica_groups=[list(range(n_cores))],
                    ins=[cc_in.opt()], outs=[cc_out.opt()])
                nc.sync.dma_start(out=dynk[:], in_=cc_out[:].partition_broadcast(96))

                # ---- attn block ----
                with (
                    tc.tile_pool(name="at_ps", bufs=2, space="PSUM") as atps,
                    tc.tile_pool(name="at_sb", bufs=1) as ab,
                ):
                    g1sb = ab.tile([96, 384], F32)
                    nc.scalar.copy(g1sb[:], g1_ps[:])
                    g2sb = ab.tile([96, 288], F32)
                    nc.scalar.copy(g2sb[:], g2_ps[:])
                    lohi_ps = atps.tile([96, 96], F32, tag="atp")
                    nc.tensor.transpose(lohi_ps[:], g2sb[:, 192:288], ident[:96, :96])
                    gq = ab.tile([96, 96], F32)
                    nc.vector.tensor_tensor(out=gq[:], in0=g1sb[:, 96:192],
                                            in1=g1sb[:, 192:288], op=Alu.add)
                    gqk = ab.tile([96, 96], F32)
                    nc.vector.tensor_tensor(out=gqk[:], in0=gq[:], in1=lohi_ps[:], op=Alu.add)
                    idm = ident[:96, :96]
                    tq = ab.tile([96, 96], F32)
                    nc.vector.tensor_tensor(out=tq[:], in0=g1sb[:, 0:96], in1=idm, op=Alu.mult)
                    nq2 = ab.tile([96, 1], F32)
                    nc.vector.reduce_sum(nq2[:], tq[:], axis=AX)
                    ksm = ab.tile([96, 96], F32)
                    nc.vector.scalar_tensor_tensor(out=ksm[:], in0=g2sb[:, 96:192], scalar=2.0,
                                                   in1=g2sb[:, 0:96], op0=Alu.mult, op1=Alu.add)
                    tk = ab.tile([96, 96], F32)
                    nc.vector.tensor_tensor(out=tk[:], in0=ksm[:], in1=idm, op=Alu.mult)
                    nk2 = ab.tile([96, 1], F32)
                    nc.vector.reduce_sum(nk2[:], tk[:], axis=AX)

                    def rsqrt_clamped(nm, src):
                        sq = ab.tile([96, 1], F32, tag=nm + "sq")
                        nc.scalar.sqrt(sq[:], src[:])
                        cl = ab.tile([96, 1], F32, tag=nm + "cl")
                        nc.vector.tensor_scalar_max(cl[:], sq[:], 1e-12)
                        rvv = ab.tile([96, 1], F32, tag=nm)
                        nc.vector.reciprocal(rvv[:], cl[:])
                        return rvv

                    rq = rsqrt_clamped("rq", nq2)
                    rk = rsqrt_clamped("rk", nk2)
                    rqt = ab.tile([96, 1], F32)
                    nc.vector.tensor_tensor(out=rqt[:], in0=rq[:], in1=ws["tempvec"][:],
                                            op=Alu.mult)
                    asr = ab.tile([96, 96], F32)
                    nc.vector.tensor_scalar_mul(asr[:], gqk[:], rqt[:])
                    as_ps = atps.tile([96, 96], F32, tag="atp")
                    nc.tensor.transpose(as_ps[:], asr[:], ident[:96, :96])
                    ast = ab.tile([96, 96], F32)
                    nc.vector.tensor_scalar_mul(ast[:], as_ps[:], rk[:])
                    as2_ps = atps.tile([96, 96], F32, tag="atp")
                    nc.tensor.transpose(as2_ps[:], ast[:], ident[:96, :96])
                    as2 = ab.tile([96, 96], F32)
                    nc.scalar.copy(as2[:], as2_ps[:])
                    # mask off-head-block entries to -60
                    t60 = ab.tile([96, 96], F32)
                    nc.vector.tensor_scalar_add(t60[:], as2[:], 60.0)
                    amf = ab.tile([96, 96], F32)
                    nc.vector.tensor_tensor(out=amf[:], in0=t60[:], in1=ws["vmask"][:],
                                            op=Alu.mult)
                    nc.vector.tensor_scalar_add(amf[:], amf[:], -60.0)
                    # rank+1 over full row via pairwise is_ge
                    rnk3 = ab.tile([96, 96 * 96], F32)
                    a_i = amf[:].unsqueeze(1).broadcast_to([96, 96, 96])
                    a_d = amf[:].unsqueeze(2).broadcast_to([96, 96, 96])
                    rvw = rnk3[:].rearrange("p (i d) -> p i d", d=96)
                    nc.vector.tensor_tensor(out=rvw, in0=a_i, in1=a_d, op=Alu.is_ge)
                    rank1 = ab.tile([96, 96], F32)
                    nc.vector.reduce_sum(rank1[:].unsqueeze(2), rvw, axis=AX)
                    sel = ab.tile([96, 96], F32)
                    nc.vector.tensor_tensor(out=sel[:], in0=rank1[:],
                                            in1=dynk[:].broadcast_to([96, 96]), op=Alu.is_le)
                    am = ab.tile([96, 96], F32)
                    t60b = ab.tile([96, 96], F32)
                    nc.vector.tensor_scalar_add(t60b[:], amf[:], 60.0)
                    nc.vector.tensor_tensor(out=am[:], in0=t60b[:], in1=sel[:], op=Alu.mult)
                    nc.vector.tensor_scalar_add(am[:], am[:], -60.0)
                    mx = ab.tile([96, 1], F32)
                    nc.vector.reduce_max(mx[:], am[:], axis=AX)
                    nmx = ab.tile([96, 1], F32)
                    nc.vector.tensor_scalar_mul(nmx[:], mx[:], -1.0)
                    ex = ab.tile([96, 96], F32)
                    nc.scalar.activation(ex[:], am[:], Act.Exp, bias=nmx[:])
                    sme = ab.tile([96, 1], F32)
                    nc.vector.reduce_sum(sme[:], ex[:], axis=AX)
                    rsm = ab.tile([96, 1], F32)
                    nc.vector.reciprocal(rsm[:], sme[:])
                    # probs scaled by attn_scale; then fold proj1 into the
                    # attention matmul: ppt = (proj1 @ (scale*P))^T = P's^T @ proj1T
                    nc.vector.tensor_scalar_mul(rsm[:], rsm[:], attn_scale)
                    probs = ab.tile([96, 96], F32R)
                    nc.vector.tensor_scalar_mul(probs[:], ex[:], rsm[:])
                    pp_ps = atps.tile([96, 192], F32, tag="atp2")
                    nc.tensor.matmul(pp_ps[:], probs[:], ws["proj1T"][:],
                                     start=True, stop=True)
                    nc.scalar.copy(ppt[:], pp_ps[:])

            # ============ PHASE C (fused attnV+proj+LN2+fc1, then C2) ============
            _wpab_cm.__exit__(None, None, None)
            _wpc_cm = tc.tile_pool(name="wpC", bufs=1)
            wpc = _wpc_cm.__enter__()
            _load_w(wpc, C_ONLY_W)
            with tc.tile_pool(name="c_v0", bufs=1) as cv0:
                v0t1 = cv0.tile([128, P3], BF16)
                v0t2 = cv0.tile([128, P3], BF16)
                with (
                    tc.tile_pool(name="f_rot", bufs=2) as fr,
                    tc.tile_pool(name="f_ps", bufs=1, space="PSUM") as fpp,
                ):
                    nc.vector.memset(v0t1[:], 0.0)
                    nc.vector.memset(v0t2[:], 0.0)
                    # pad cells must hold -t1/s1 so the bn-folded depthwise
                    # reads zeros in v0_bn space at image borders
                    nc.vector.tensor_scalar_add(v0t1[:], v0t1[:], ws["padv1"])
                    nc.vector.tensor_scalar_add(v0t2[:], v0t2[:], ws["padv2"])
                    for ci in range(NSC):
                        o0 = ci * 512
                        NN = min(512, S - o0)
                        c0 = o0 // W
                        nr_c = NN // W
                        vch = fr.tile([96, 512], F32R, tag="vch")
                        nc.sync.dma_start(out=vch[:, :NN], in_=v_sp[:, o0:o0 + NN])
                        x2ch = fr.tile([96, 512], F32R, tag="x2ch")
                        nc.sync.dma_start(out=x2ch[:, :NN], in_=yn2_sp[:, o0:o0 + NN])
                        xpch = [fr.tile([96, 512], F32R, tag=f"xp{cg}", name=f"xp{cg}") for cg in range(2)]
                        xsq = [fr.tile([96, 512], F32R, tag=f"xs{cg}", name=f"xs{cg}") for cg in range(2)]
                        for cg in range(2):
                            xcch = fr.tile([96, 512], F32R, tag=f"xcc{cg}")
                            nc.sync.dma_start(out=xcch[:, :NN], in_=xc_sp[cg][:, o0:o0 + NN])
                            pj_ps = fpp.tile([96, 512], F32, tag=f"pj{cg}", bufs=2)
                            nc.tensor.matmul(pj_ps[:, :NN],
                                             ppt[:, cg * 96:(cg + 1) * 96],
                                             vch[:, :NN], start=True, stop=False)
                            nc.tensor.matmul(pj_ps[:, :NN],
                                             ws["proj2T"][:, cg * 96:(cg + 1) * 96],
                                             x2ch[:, :NN], start=False, stop=True)
                            nc.vector.scalar_tensor_tensor(
                                out=xpch[cg][:, :NN], in0=pj_ps[:, :NN],
                                scalar=ws["proj_bias"][:, cg:cg + 1], in1=xcch[:, :NN],
                                op0=Alu.add, op1=Alu.add)
                            nc.sync.dma_start(out=xcp_sp[cg][:, o0:o0 + NN],
                                              in_=xpch[cg][:, :NN])
                            nc.scalar.square(xsq[cg][:, :NN], xpch[cg][:, :NN])
                        mu_ps = fpp.tile([128, 512], F32, tag="mu")
                        m2_ps = fpp.tile([128, 512], F32, tag="m2")
                        for cg in range(2):
                            nc.tensor.matmul(mu_ps[:, :NN], ws["ones_st"], xpch[cg][:, :NN],
                                             start=(cg == 0), stop=(cg == 1))
                            nc.tensor.matmul(m2_ps[:, :NN], ws["ones_st"], xsq[cg][:, :NN],
                                             start=(cg == 0), stop=(cg == 1))
                        musq = fr.tile([128, 512], F32, tag="musq")
                        nc.scalar.square(musq[:, :NN], mu_ps[:, :NN])
                        var = fr.tile([128, 512], F32, tag="var")
                        nc.vector.tensor_tensor(out=var[:, :NN], in0=m2_ps[:, :NN],
                                                in1=musq[:, :NN], op=Alu.subtract)
                        sd2 = fr.tile([128, 512], F32, tag="sd2")
                        nc.scalar.activation(sd2[:, :NN], var[:, :NN], Act.Sqrt,
                                             bias=ws["epsv"])
                        rstd = fr.tile([128, 512], F32, tag="rstd")
                        nc.vector.reciprocal_approx_fast(rstd[:, :NN], sd2[:, :NN])
                        yn2t = [fr.tile([96, 512], F32R, tag=f"cy{cg}", name=f"cy{cg}") for cg in range(2)]
                        for cg in range(2):
                            td = fr.tile([96, 512], F32, tag=f"ctd{cg}")
                            nc.vector.tensor_tensor(out=td[:, :NN], in0=xpch[cg][:, :NN],
                                                    in1=mu_ps[:96, :NN], op=Alu.subtract)
                            nc.vector.tensor_tensor(out=yn2t[cg][:, :NN], in0=td[:, :NN],
                                                    in1=rstd[:96, :NN], op=Alu.mult)
                        d0 = pd3(3 + c0) + 3
                        pv1 = v0t1[:, d0:d0 + nr_c * Wp3].rearrange(
                            "p (r w) -> p r w", w=Wp3)[:, :, 0:W]
                        pv2 = v0t2[:, d0:d0 + nr_c * Wp3].rearrange(
                            "p (r w) -> p r w", w=Wp3)[:, :, 0:W]
                        pv2s = v0t2[:, d0 + 1:d0 + 1 + nr_c * Wp3].rearrange(
                            "p (r w) -> p r w", w=Wp3)[:, :, 0:W]
                        for mg in range(2):
                            fch = fpp.tile([128, 512], F32, tag="fc", bufs=2)
                            for cg in range(2):
                                nc.tensor.matmul(
                                    fch[:, :NN],
                                    ws["fc1T"][:, (mg * 2 + cg) * 128:(mg * 2 + cg + 1) * 128],
                                    yn2t[cg][:, :NN], start=(cg == 0), stop=(cg == 1))
                            fv = fch[:, :NN].rearrange("p (r w) -> p r w", w=W)
                            if mg == 0:
                                vg0 = fr.tile([64, 512], F32R, tag="vg0")
                                nc.scalar.activation(vg0[:, :NN], fch[0:64, :NN], Act.Gelu,
                                                     bias=ws["fc1_bias"][0:64, 0:1])
                                ug0 = fr.tile([64, 512], F32R, tag="ug0")
                                nc.scalar.activation(ug0[:, :NN], vg0[:, :NN], Act.Gelu,
                                                     bias=ws["g0_bias"], scale=ws["g0_scale"])
                                nc.sync.dma_start(
                                    out=vg0_sp[:, d0:d0 + nr_c * Wp3].rearrange(
                                        "p (r w) -> p r w", w=Wp3)[:, :, 0:W],
                                    in_=vg0[:, :NN].rearrange("p (r w) -> p r w", w=W))
                                nc.sync.dma_start(
                                    out=ug0_sp[:, d0:d0 + nr_c * Wp3].rearrange(
                                        "p (r w) -> p r w", w=Wp3)[:, :, 0:W],
                                    in_=ug0[:, :NN].rearrange("p (r w) -> p r w", w=W))
                                nc.scalar.activation(pv1[0:64], fv[64:128], Act.Gelu,
                                                     bias=ws["fc1_bias"][64:128, 0:1])
                            else:
                                nc.scalar.activation(pv1[64:128], fv[0:64], Act.Gelu,
                                                     bias=ws["fc1_bias"][0:64, 1:2])
                                nc.scalar.activation(pv2[0:64], fv[64:128], Act.Gelu,
                                                     bias=ws["fc1_bias"][64:128, 1:2])
                                nc.scalar.activation(pv2s[64:128], fv[64:128], Act.Gelu,
                                                     bias=ws["fc1_bias"][64:128, 1:2])

                with (
                    tc.tile_pool(name="c2_rot", bufs=2) as c2r,
                    tc.tile_pool(name="c2_ps", bufs=2, space="PSUM") as c2ps,
                ):
                    for ci in range(NCH):
                        c0 = ci * RC
                        nr_c = min(RC, H - c0)
                        N = nr_c * Wp3
                        NN = nr_c * W
                        sb0 = pd3(3 + c0)
                        ps_a = c2ps.tile([128, RC * Wp3], F32, tag="psa")
                        for t in range(25):
                            dy, dx = t // 5 - 2, t % 5 - 2
                            o = sb0 + dy * Wp3 + dx
                            nc.tensor.matmul(ps_a[:, :N],
                                             ws["pair_diag"][:, t * 128:(t + 1) * 128],
                                             v0t1[:, o:o + N],
                                             start=(t == 0), stop=(t == 24))
                        ps_b = c2ps.tile([64, RC * Wp3], F32, tag="psb")
                        for i, (dy, dxa, hasb) in enumerate(dw3_passes):
                            o = sb0 + dy * Wp3 + dxa
                            nc.tensor.matmul(ps_b[:, :N],
                                             ws["dw3_diag"][:, i * 64:(i + 1) * 64],
                                             v0t2[:, o:o + N],
                                             start=(i == 0), stop=(i == len(dw3_passes) - 1))

                        def inner(ap_flat, lo, hi):
                            # interior view of a PSUM chunk (starts at free 0)
                            return ap_flat[lo:hi, :N].rearrange(
                                "p (r w) -> p r w", w=Wp3)[:, :, 3:3 + W]

                        def inner_v0(ap_flat, lo, hi):
                            # interior view of the padded v0 buffers at this chunk
                            return ap_flat[lo:hi, sb0:sb0 + N].rearrange(
                                "p (r w) -> p r w", w=Wp3)[:, :, 3:3 + W]

                        ug_a = c2r.tile([128, RC * W], F32R, tag="uga")
                        ug_b = c2r.tile([128, RC * W], F32R, tag="ugb")
                        vb_a = c2r.tile([128, RC * W], F32, tag="vba")
                        vb_b = c2r.tile([128, RC * W], F32, tag="vbb")
                        g0v = c2r.tile([64, RC * W], F32R, tag="g0v")
                        src3 = vg0_sp[:, sb0:sb0 + N].rearrange(
                            "p (r w) -> p r w", w=Wp3)[:, :, 3:3 + W]
                        nc.sync.dma_start(
                            out=g0v[:, :NN].rearrange("p (r w) -> p r w", w=W), in_=src3)
                        usrc3 = ug0_sp[:, sb0:sb0 + N].rearrange(
                            "p (r w) -> p r w", w=Wp3)[:, :, 3:3 + W]
                        nc.sync.dma_start(
                            out=ug_a[0:64, :NN].rearrange("p (r w) -> p r w", w=W), in_=usrc3)
                        nc.scalar.activation(
                            ug_a[64:128, :NN].rearrange("p (r w) -> p r w", w=W),
                            inner(ps_a, 0, 64), Act.Gelu, bias=ws["pair_bias"][0:64])
                        nc.scalar.activation(
                            ug_b[0:64, :NN].rearrange("p (r w) -> p r w", w=W),
                            inner(ps_a, 64, 128), Act.Gelu, bias=ws["pair_bias"][64:128])
                        nc.scalar.activation(
                            ug_b[64:128, :NN].rearrange("p (r w) -> p r w", w=W),
                            inner(ps_b, 0, 64), Act.Gelu, bias=ws["dw3_bias"])
                        nc.vector.tensor_scalar(out=vb_a[0:64, :NN], in0=g0v[:, :NN],
                                                scalar1=ws["s1a"][0:64],
                                                scalar2=ws["t1a"][0:64],
                                                op0=Alu.mult, op1=Alu.add)
                        nc.vector.tensor_scalar(out=vb_a[64:128, :NN],
                                                in0=inner_v0(v0t1, 0, 64),
                                                scalar1=ws["s1a"][64:128],
                                                scalar2=ws["t1a"][64:128],
                                                op0=Alu.mult, op1=Alu.add)
                        nc.vector.tensor_scalar(out=vb_b[0:64, :NN],
                                                in0=inner_v0(v0t1, 64, 128),
                                                scalar1=ws["s1b"][0:64],
                                                scalar2=ws["t1b"][0:64],
                                                op0=Alu.mult, op1=Alu.add)
                        nc.vector.tensor_scalar(out=vb_b[64:128, :NN],
                                                in0=inner_v0(v0t2, 0, 64),
                                                scalar1=ws["s1b"][64:128],
                                                scalar2=ws["t1b"][64:128],
                                                op0=Alu.mult, op1=Alu.add)
                        # u_bn2 = s2*gelu(u)+t2 made explicit so fc2 is 2 matmuls/cg
                        ub_a = c2r.tile([128, RC * W], F32, tag="uba")
                        ub_b = c2r.tile([128, RC * W], F32, tag="ubb")
                        nc.vector.tensor_scalar(out=ub_a[:, :NN], in0=ug_a[:, :NN],
                                                scalar1=ws["s2a"], scalar2=ws["t2a"],
                                                op0=Alu.mult, op1=Alu.add)
                        nc.vector.tensor_scalar(out=ub_b[:, :NN], in0=ug_b[:, :NN],
                                                scalar1=ws["s2b"], scalar2=ws["t2b"],
                                                op0=Alu.mult, op1=Alu.add)
                        z1a = c2r.tile([128, RC * W], F32R, tag="z1a")
                        z1b = c2r.tile([128, RC * W], F32R, tag="z1b")
                        nc.vector.tensor_tensor(out=z1a[:, :NN], in0=ub_a[:, :NN],
                                                in1=vb_a[:, :NN], op=Alu.mult)
                        nc.vector.tensor_tensor(out=z1b[:, :NN], in0=ub_b[:, :NN],
                                                in1=vb_b[:, :NN], op=Alu.mult)
                        for cg in range(2):
                            ops = c2ps.tile([96, RC * W], F32, tag=f"ops{cg}")
                            nc.tensor.matmul(ops[:, :NN],
                                             ws["fc2aT"][:, (cg * 2) * 96:(cg * 2 + 1) * 96],
                                             z1a[:, :NN], start=True, stop=False)
                            nc.tensor.matmul(ops[:, :NN],
                                             ws["fc2aT"][:, (cg * 2 + 1) * 96:(cg * 2 + 2) * 96],
                                             z1b[:, :NN], start=False, stop=True)
                            xrch = c2r.tile([96, RC * W], F32R, tag=f"xr{cg}", bufs=1)
                            nc.sync.dma_start(out=xrch[:, :NN],
                                              in_=xcp_sp[cg][:, c0 * W:c0 * W + NN])
                            ob = c2r.tile([96, RC * W], F32, tag=f"ob{cg}", bufs=1)
                            nc.vector.tensor_scalar(out=ob[:, :NN], in0=ops[:, :NN],
                                                    scalar1=ws["s3v"][:, cg:cg + 1],
                                                    scalar2=ws["out_bias"][:, cg:cg + 1],
                                                    op0=Alu.mult, op1=Alu.add)
                            oc = c2r.tile([96, RC * W], F32, tag=f"oc{cg}", bufs=1)
                            nc.vector.tensor_tensor(out=oc[:, :NN], in0=ob[:, :NN],
                                                    in1=xrch[:, :NN], op=Alu.add)
                            nc.sync.dma_start(
                                out=out_t[cg * 96:(cg + 1) * 96, c0 * W:c0 * W + NN],
                                in_=oc[:, :NN])
            _wpc_cm.__exit__(None, None, None)
    return out_t.name


# ----------------------------------------------------------------------------
# host entry
# ----------------------------------------------------------------------------

_CACHE = {}


def make_program(H, W, n_cores, attn_scale, dw3_passes):
    key = (H, W, n_cores, round(attn_scale, 9))
    if key in _CACHE:
        return _CACHE[key]
    nc = bacc.Bacc("TRN2", target_bir_lowering=False, debug=False, num_devices=n_cores)
    out_name = build(nc, H, W, n_cores, attn_scale, dw3_passes)
    nc.compile()
    _CACHE[key] = (nc, out_name)
    return nc, out_name


def make_in_maps(inputs):
    x = np.asarray(inputs["x"], np.float32)
    B = x.shape[0]
    wdict = _prep_weights({k: np.asarray(v) for k, v in inputs.items()})
    base = {}
    for k, (shp, d) in WSPEC.items():
        base["w_" + k] = wdict[k][0].reshape(shp)
    in_maps = []
    for b in range(B):
        m = dict(base)
        m["x"] = np.ascontiguousarray(x[b])
        in_maps.append(m)
    return in_maps, wdict


def kernel(**inputs):
    x = np.asarray(inputs["x"], np.float32)
    B, H, W, C = x.shape
    in_maps, wdict = make_in_maps(inputs)
    nc, out_name = make_program(H, W, B, wdict["_attn_scale"][0],
                                wdict["_dw3_passes"][0])
    res = bass_utils.run_bass_kernel_spmd(nc, in_maps, core_ids=list(range(B)))
    return np.stack([
        np.asarray(res.results[b][out_name]).reshape(C, H * W).T.reshape(H, W, C)
        for b in range(B)])

